# revision 35
# baseline (speedup 1.0000x reference)
"""Trainium2 Bass kernel for nn_EncoderLayer (multiplicative-attention encoder layer).

Sharding: 8 cores; core c handles batch b=c//2, head-group hg=c%2 (4 of 8 heads).
The reference's head-major reshape bug maps head h exactly to output rows
[256h, 256h+256), so each core owns 1024 complete output rows -> no collectives.

v3: - big GEMMs (QKV proj, Wo, FFN1, FFN2) in fp8e4 DoubleRow perf mode
      (0.5 PE cycles/out-col, K=256/pass) with host-prepacked weights and
      power-of-2 pre-scales folded into writer ops / the exp scale.
    - software-pipelined emission: chain work for head h's 256 output tokens
      is interleaved (generator round-robin) with head h+1's attention, so
      Act (exp-bound) and PE (GEMM-bound) run concurrently.
    - softmax tail: recip straight off PSUM den rows, Pool partition_broadcast
      replaces the PE broadcast matmul, xn multiply reads PSUM directly.

Per-token chain independence: LN1/FFN/LN2 normalize over features, so the
chain runs on 256-token blocks (one attention head's scrambled rows each).
FFN1 runs per 512-token pair to halve writer-instruction overhead.
"""

import numpy as np
import ml_dtypes

import concourse.bass as bass
import concourse.tile as tile
import concourse.bacc as bacc
from concourse import mybir
from concourse import bass_utils
from concourse import hw_specs as _hw_specs

_real_gat = _hw_specs.get_activation_tables


def _gat_pinned(arch):
    tabs = _real_gat(arch)
    return {name: (fns if name == "natural_log_exp_and_others" else set())
            for name, fns in tabs.items()}


bacc.get_activation_tables = _gat_pinned

B, S, HID, H, PF, D = 4, 2048, 512, 8, 2048, 64
N_CORES = 8
HPC = H // 2          # heads per core (4)
R = HPC * 256         # output rows per core (1024)
F32 = mybir.dt.float32
BF16 = mybir.dt.bfloat16
FP8 = mybir.dt.float8e4
AF = mybir.ActivationFunctionType
OP = mybir.AluOpType
DR = mybir.MatmulPerfMode.DoubleRow
NEG_BIG = -87.0
LN_EPS = 1e-5
SQ = 64.0             # wq scale
SK = 32.0             # wk scale
SV = 32.0             # wv scale
SX = 64.0             # xn scale
SW = 32.0             # wo/w1/w2 scale
fp8np = ml_dtypes.float8_e4m3

_built_cache = {}
last_results = None
run_kwargs = {}


def _bcast_ap(ap_1d, parts):
    return bass.AP(tensor=ap_1d.tensor, offset=ap_1d.offset,
                   ap=[[0, parts], *ap_1d.ap])


def _pack_dr(wT):
    """[K, M] (K mult of 256) -> DR-packed [K//256 * 128, 2 * M] host layout."""
    K, M = wT.shape
    return np.ascontiguousarray(
        wT.reshape(K // 256, 2, 128, M).transpose(0, 2, 1, 3)
    ).reshape(K // 2, 2 * M)


import os as _os
_RATIO = int(_os.environ.get("KRATIO", "1"))


def _interleave(*gens, ratio=None):
    # first generator gets `ratio` bursts per single burst of the others
    r = ratio if ratio is not None else _RATIO
    active = [iter(g) for g in gens]
    while active:
        for i, g in enumerate(list(active)):
            n = r if (i == 0 and len(active) > 1) else 1
            for _ in range(n):
                try:
                    next(g)
                except StopIteration:
                    if g in active:
                        active.remove(g)
                    break


def build_bass(sup):
    """Per-core module. sup = padded unmasked key count (mult of 128)."""
    KT = sup // 128
    nc = bacc.Bacc("TRN2", target_bir_lowering=False, debug=False,
                   num_devices=N_CORES)

    def inp(name, shape, dt=F32):
        return nc.dram_tensor(name, shape, dt, kind="ExternalInput").ap()

    src8_d = inp("src8", [HID, S], FP8)
    src_res_d = inp("src_res", [HID, R])         # fp32 src.T slice + bo
    srcu8_d = inp("srcu8", [HID, sup], FP8)
    wq_d = inp("wq", [2 * 128, 2 * 256], FP8)    # DR-packed SQ*(Wm@Wq).T
    wk_d = inp("wk", [2 * 128, 2 * 256], FP8)
    wv_d = inp("wv", [HID, 256], FP8)
    wo_d = inp("wo", [2 * 128, 2 * 512], FP8)
    w1_d = inp("w1", [2 * 128, 2 * PF], FP8)
    w2_d = inp("w2", [8 * 128, 2 * 512], FP8)
    bq_d = inp("bq", [2, 128])
    bk_d = inp("bk", [2, 128])
    bv_d = inp("bv", [256])
    b1_d = inp("b1", [16, 128])                  # SW*b1
    bt1_d = inp("bt1", [4, 128])                 # ln1_b
    bt1f_d = inp("bt1f", [4, 128])               # ln1_b + b2
    g1_d = inp("g1", [4, 128])
    g2_d = inp("g2", [4, 128])
    bt2_d = inp("bt2", [4, 128])
    mb_d = inp("mb", [KT, 128])
    out_d = nc.dram_tensor("out_t", [HID, R], F32, kind="ExternalOutput").ap()

    from contextlib import ExitStack
    with tile.TileContext(nc) as tc, ExitStack() as ctx:
        con = ctx.enter_context(tc.tile_pool(name="con", bufs=1))
        ppool = ctx.enter_context(tc.tile_pool(name="ps", bufs=2, space="PSUM"))
        pe_e = ctx.enter_context(tc.tile_pool(name="pe", bufs=2, space="PSUM"))
        pe_av = ctx.enter_context(tc.tile_pool(name="pav", bufs=2, space="PSUM"))
        att_pool = ctx.enter_context(tc.tile_pool(name="att", bufs=4))
        xn_pool = ctx.enter_context(tc.tile_pool(name="xn", bufs=2))
        rep_pool = ctx.enter_context(tc.tile_pool(name="rep", bufs=3))
        h1_pool = ctx.enter_context(tc.tile_pool(name="h1", bufs=2))
        tmp_pool = ctx.enter_context(tc.tile_pool(name="tmp", bufs=3))
        z_pool = ctx.enter_context(tc.tile_pool(name="z", bufs=2))
        o_pool = ctx.enter_context(tc.tile_pool(name="o", bufs=4))

        mm = nc.tensor.matmul
        act = nc.scalar.activation
        dve = nc.vector
        gps = nc.gpsimd

        def dma(out, in_):
            nc.sync.dma_start(out=out, in_=in_)

        def ctile(shape, dt, tag):
            return con.tile(shape, dt, tag=tag, name=tag)

        # ---- constants / weights ----
        srcu8 = ctile([128, 4, sup], FP8, "srcu8")
        wq8 = ctile([128, 2, 2, 256], FP8, "wq8")
        wk8 = ctile([128, 2, 2, 256], FP8, "wk8")
        wv8 = ctile([128, 4, 256], FP8, "wv8")
        dma(wk8, wk_d.rearrange("(t p) (i m) -> p t i m", t=2, i=2))
        dma(srcu8, srcu8_d.rearrange("(c p) n -> p c n", p=128))
        dma(wv8, wv_d.rearrange("(c p) m -> p c m", p=128))
        src8 = ctile([128, 4, S], FP8, "src8")
        dma(src8, src8_d.rearrange("(c p) n -> p c n", p=128))
        dma(wq8, wq_d.rearrange("(t p) (i m) -> p t i m", t=2, i=2))
        src_res = [ctile([128, R], F32, f"srcres{i}") for i in range(4)]
        wo8 = ctile([128, 2, 2, 512], FP8, "wo8")
        w18 = ctile([128, 2, 2, PF], FP8, "w18")
        w28 = ctile([128, 8, 2, 512], FP8, "w28")

        def load_chain_weights():
            dma(wo8, wo_d.rearrange("(t p) (i m) -> p t i m", t=2, i=2))
            for i in range(4):
                dma(src_res[i], src_res_d[128 * i:128 * (i + 1), :])
            dma(w18, w1_d.rearrange("(t p) (i m) -> p t i m", t=2, i=2))
            dma(w28, w2_d.rearrange("(t p) (i m) -> p t i m", t=8, i=2))

        def vec_in(dram, n, tag):
            t = ctile([128, n], F32, tag)
            dma(t, dram.rearrange("m p -> p m"))
            return t

        bq_sb = vec_in(bq_d, 2, "bq")
        bk_sb = vec_in(bk_d, 2, "bk")
        b1_sb = vec_in(b1_d, 16, "b1")
        g1_sb = vec_in(g1_d, 4, "g1")
        bt1_sb = vec_in(bt1_d, 4, "bt1")
        bt1f_sb = vec_in(bt1f_d, 4, "bt1f")
        g2_sb = vec_in(g2_d, 4, "g2")
        bt2_sb = vec_in(bt2_d, 4, "bt2")
        mb_sb = vec_in(mb_d, KT, "mb")
        bv_rep = ctile([128, 256], F32, "bvrep")
        dma(bv_rep, _bcast_ap(bv_d, 128))

        ones_bf = ctile([128, 128], BF16, "onesbf")
        dve.memset(ones_bf, 1.0)
        eps_t = ctile([128, 1], F32, "eps")
        dve.memset(eps_t, LN_EPS)

        q_sb = [ctile([128, S], BF16, f"q{m}") for m in range(2)]
        k_sb = [ctile([128, sup], BF16, f"k{m}") for m in range(2)]

        def gen_proj(w8, bias_sb, src_t, n_total, out_tiles, mt):
            n0 = 0
            while n0 < n_total:
                nq = min(512, n_total - n0)
                for half in range(2):
                    ps = ppool.tile([64, 512], F32, tag="ps", name="psp")
                    mcol = 64 * (2 * mt + half)
                    for t in range(2):
                        mm(ps[:, :nq],
                           w8[:, t, :, mcol:mcol + 64],
                           src_t[:, 2 * t:2 * t + 2, n0:n0 + nq],
                           start=(t == 0), stop=(t == 1), perf_mode=DR)
                    dve.tensor_scalar_add(
                        out_tiles[mt][64 * half:64 * half + 64, n0:n0 + nq],
                        ps[:, :nq],
                        bias_sb[64 * half:64 * half + 64, mt:mt + 1])
                    yield
                n0 += nq

        # ---- V natural [keys, 4*68] bf16 with ones cols (fp8 matmul) ----
        v_sb = ctile([128, KT * 4 * 68], BF16, "v")
        v_v = v_sb.rearrange("p (kt h e) -> p kt h e", kt=KT, h=4)

        def gen_vproj():
            # ones cols hold 1/SX so den rows accumulate den/SX and
            # rep4 = recip(den/SX) = SX/den (xn lands mid-range for fp8)
            dve.memset(v_v[:, :, :, 64:68], 1.0 / SX)
            for kt in range(KT):
                ps = ppool.tile([128, 512], F32, tag="ps", name="psv")
                for ct in range(4):
                    mm(ps[:, :256], srcu8[:, ct, 128 * kt:128 * (kt + 1)],
                       wv8[:, ct, :], start=(ct == 0), stop=(ct == 3))
                dve.scalar_tensor_tensor(
                    out=v_v[:, kt, :, 0:64],
                    in0=ps[:, :256].rearrange("p (h d) -> p h d", h=4),
                    scalar=1.0 / SV,
                    in1=bv_rep.rearrange("p (h d) -> p h d", h=4),
                    op0=OP.mult, op1=OP.add)
                yield

        # ---- attention head h -> fp8 xnp half (scaled by SX) ----
        def gen_attention(h, xnp):
            g = h // 2
            p0 = 64 * (h % 2)
            o0 = S * (h % 2)
            for q0 in range(0, S, 1024):
                avs = []
                for half in range(2):
                    avs.append(pe_av.tile([68, 512], F32, tag="av", name="av"))
                for kt in range(KT):
                    e = pe_e.tile([128, 1024], F32, tag="e", name="e")
                    for half in range(2):
                        mm(e[:, 512 * half:512 * (half + 1)],
                           k_sb[g][p0:p0 + 64, 128 * kt:128 * (kt + 1)],
                           q_sb[g][p0:p0 + 64,
                                   q0 + 512 * half:q0 + 512 * (half + 1)],
                           start=True, stop=True)
                    at = att_pool.tile([128, 1024], BF16, tag="att", name="att")
                    act(at, e, AF.Exp, bias=mb_sb[:, kt:kt + 1],
                        scale=1.0 / (SQ * SK))
                    for half in range(2):
                        mm(avs[half], v_v[:, kt, h, :],
                           at[:, 512 * half:512 * (half + 1)],
                           start=(kt == 0), stop=(kt == KT - 1),
                           skip_group_check=True)
                    yield
                # tail: rep = SX/den via recip of the 4 identical den rows;
                # broadcast row 0 to 64 partitions on Pool; xn = x' * rep.
                for half in range(2):
                    rep4 = rep_pool.tile([4, 512], BF16, tag="rep4", name="rep4")
                    with nc.allow_low_precision(reason="softmax recip"):
                        dve.reciprocal(rep4, avs[half][64:68, :])
                    rep = rep_pool.tile([64, 512], BF16, tag="rep", name="rep")
                    gps.partition_broadcast(rep, rep4[0:1, :], channels=64)
                    dve.tensor_tensor(
                        out=xnp[0:64, o0 + q0 + 512 * half:o0 + q0 + 512 * (half + 1)],
                        in0=avs[half][0:64, :], in1=rep, op=OP.mult)
                    yield
            gps.tensor_copy(out=xnp[64:128, o0:o0 + S - 1],
                            in_=xnp[0:64, o0 + 1:o0 + S])
            yield

        # ---- layernorm on 4x[128, W] f32 z-tiles ----
        def gen_layernorm(z_tiles, g_sb, writers, W):
            # s1/s2 must sit in separate PSUM banks: a start=True matmul marks
            # its whole 2KB zero-region pending-zero, wiping any sibling
            # accumulation group sharing the bank.
            s1 = ppool.tile([128, W], F32, tag="ps", name="s1")
            s2 = ppool.tile([128, W], F32, tag="ps", name="s2")
            for ct in range(4):
                zb = tmp_pool.tile([128, W], BF16, tag="zb", name="zb")
                gps.tensor_copy(out=zb, in_=z_tiles[ct])
                sq = tmp_pool.tile([128, W], BF16, tag="sq", name="sq")
                dve.tensor_tensor(out=sq, in0=zb, in1=zb, op=OP.mult)
                mm(s1, ones_bf, zb, start=(ct == 0), stop=(ct == 3),
                   skip_group_check=True)
                mm(s2, ones_bf, sq, start=(ct == 0), stop=(ct == 3),
                   skip_group_check=True)
                yield
            bm = tmp_pool.tile([128, W], F32, tag="bm", name="bm")
            br = tmp_pool.tile([128, W], F32, tag="br", name="br")
            m2 = tmp_pool.tile([128, W], BF16, tag="m2", name="m2", bufs=1)
            dve.tensor_scalar_mul(bm, s1, 1.0 / HID)
            dve.tensor_tensor(out=m2, in0=bm, in1=bm, op=OP.mult)
            dve.scalar_tensor_tensor(out=br, in0=s2,
                                     scalar=1.0 / HID, in1=m2,
                                     op0=OP.mult, op1=OP.subtract)
            act(br, br, AF.Ln, bias=eps_t)
            act(br, br, AF.Exp, scale=-0.5)
            yield
            for ct in range(4):
                sub = tmp_pool.tile([128, W], F32, tag="sub", name="sub")
                gps.tensor_tensor(out=sub, in0=z_tiles[ct], in1=bm,
                                  op=OP.subtract)
                t2 = tmp_pool.tile([128, W], F32, tag="t2", name="t2")
                dve.scalar_tensor_tensor(out=t2, in0=sub,
                                         scalar=g_sb[:, ct:ct + 1], in1=br,
                                         op0=OP.mult, op1=OP.mult)
                writers(ct, t2)
                yield

        # ---- chain A for one 256-token block: Wo(DR) + res, LN1 ----
        src1_f = [[con.tile([128, 256], F32, tag=f"s1f{i}_{j}",
                            name=f"s1f{i}_{j}") for j in range(4)]
                  for i in range(4)]
        src1_8 = ctile([128, 4, R], FP8, "src1_8")

        def gen_chain_a(blk, xnp):
            c0 = 256 * blk
            hh = blk % 2
            xw = xnp.rearrange("p (hh m j) -> p j hh m", hh=2, j=8)
            z1 = [z_pool.tile([128, 256], F32, tag=f"z{mt}", name=f"z{mt}")
                  for mt in range(4)]
            for mt in range(4):
                for half in range(2):
                    ps = ppool.tile([64, 512], F32, tag="ps", name="pswo")
                    mcol = 64 * (2 * mt + half)
                    for t in range(2):
                        ifm = bass.AP(
                            tensor=xw.tensor,
                            offset=xw.offset + (4 * t) * xw.ap[1][0]
                            + hh * xw.ap[2][0],
                            ap=[xw.ap[0], [2 * xw.ap[1][0], 2], xw.ap[3]])
                        mm(ps[:, :256], wo8[:, t, :, mcol:mcol + 64], ifm,
                           start=(t == 0), stop=(t == 1), perf_mode=DR)
                    dve.scalar_tensor_tensor(
                        out=z1[mt][64 * half:64 * half + 64, :],
                        in0=ps[:, :256],
                        scalar=1.0 / (SX * SW),
                        in1=src_res[mt][64 * half:64 * half + 64, c0:c0 + 256],
                        op0=OP.mult, op1=OP.add)
                    yield

            def w1(ct, t2):
                gps.tensor_scalar_add(src1_f[ct][blk], t2,
                                      bt1f_sb[:, ct:ct + 1])
                gps.tensor_scalar_add(src1_8[:, ct, c0:c0 + 256], t2,
                                      bt1_sb[:, ct:ct + 1])

            yield from gen_layernorm(z1, g1_sb, w1, 256)

        # ---- FFN1 for a 512-token pair (blocks 2p, 2p+1) ----
        h18s = {}

        def gen_ffn1(p):
            c0 = 512 * p
            h18 = h1_pool.tile([128, 16, 512], FP8, tag="h1", name="h1", bufs=2)
            h18s[p] = h18
            for mt in range(16):
                for half in range(2):
                    ps = ppool.tile([64, 512], F32, tag="ps", name="psf1")
                    mcol = 64 * (2 * mt + half)
                    for t in range(2):
                        mm(ps, w18[:, t, :, mcol:mcol + 64],
                           src1_8[:, 2 * t:2 * t + 2, c0:c0 + 512],
                           start=(t == 0), stop=(t == 1), perf_mode=DR)
                    if half == 0:
                        act(h18[0:64, mt, :], ps, AF.Relu,
                            bias=b1_sb[0:64, mt:mt + 1], scale=1.0)
                    else:
                        dve.tensor_scalar(
                            out=h18[64:128, mt, :], in0=ps,
                            scalar1=b1_sb[64:128, mt:mt + 1], scalar2=0.0,
                            op0=OP.add, op1=OP.max)
                    yield

        # ---- chain B for one 256-token block: FFN2(DR) + res, LN2, out ----
        z2s = {}

        def gen_chain_b_ffn(blk):
            c0 = 256 * blk
            h18 = h18s[blk // 2]
            r0 = 256 * (blk % 2)
            z2 = [z_pool.tile([128, 256], F32, tag=f"z{ot}", name=f"z{ot}")
                  for ot in range(4)]
            z2s[blk] = z2
            for ot in range(4):
                for half in range(2):
                    ps = ppool.tile([64, 512], F32, tag="ps", name="psf2")
                    mcol = 64 * (2 * ot + half)
                    for t in range(8):
                        mm(ps[:, :256], w28[:, t, :, mcol:mcol + 64],
                           h18[:, 2 * t:2 * t + 2, r0:r0 + 256],
                           start=(t == 0), stop=(t == 7), perf_mode=DR)
                    dve.scalar_tensor_tensor(
                        out=z2[ot][64 * half:64 * half + 64, :],
                        in0=ps[:, :256],
                        scalar=1.0 / (SW * SW),
                        in1=src1_f[ot][blk][64 * half:64 * half + 64, :],
                        op0=OP.mult, op1=OP.add)
                    yield

        def gen_chain_b_ln(blk):
            c0 = 256 * blk
            z2 = z2s[blk]

            def w2(ct, t2):
                o = o_pool.tile([128, 256], F32, tag="out", name="out", bufs=2)
                gps.tensor_scalar_add(o, t2, bt2_sb[:, ct:ct + 1])
                dma(out_d[128 * ct:128 * (ct + 1), c0:c0 + 256], o)

            yield from gen_layernorm(z2, g2_sb, w2, 256)

        def gen_chain_b(blk):
            yield from gen_chain_b_ffn(blk)
            yield from gen_chain_b_ln(blk)

        def gen_seq(*gens):
            for g in gens:
                yield from g

        # ---- schedule ----
        # V' ones cols hold 1/SX so den rows accumulate den/SX and
        # rep4 = recip(den/SX) = SX/den.  (memset inside gen_vproj runs first.)
        xnp0 = xn_pool.tile([128, 2 * S], FP8, tag="xn", name="xn")
        xnp1 = xn_pool.tile([128, 2 * S], FP8, tag="xn", name="xn")

        _interleave(gen_proj(wk8, bk_sb, srcu8, sup, k_sb, 0),
                    gen_proj(wq8, bq_sb, src8, S, q_sb, 0),
                    gen_vproj())
        load_chain_weights()
        _interleave(gen_attention(0, xnp0),
                    gen_seq(gen_proj(wk8, bk_sb, srcu8, sup, k_sb, 1),
                            gen_proj(wq8, bq_sb, src8, S, q_sb, 1)))
        _interleave(gen_attention(1, xnp0), gen_chain_a(0, xnp0))
        _interleave(gen_attention(2, xnp1),
                    gen_seq(gen_chain_a(1, xnp0), gen_ffn1(0)))
        _interleave(gen_attention(3, xnp1),
                    gen_seq(gen_chain_b(0), gen_chain_a(2, xnp1),
                            gen_chain_b(1)))
        _interleave(gen_seq(gen_chain_a(3, xnp1), gen_ffn1(1),
                            gen_chain_b(2), gen_chain_b(3)))

    nc.compile()
    return nc


def _prep_core(c, src, idxs, sup, w):
    b, hg = c // 2, c % 2
    heads = list(range(HPC * hg, HPC * hg + HPC))
    st = np.ascontiguousarray(src[b].T)                       # [512, 2048] f32
    idx = idxs[b]
    su = len(idx)
    srcu = np.zeros((HID, sup), np.float32)
    srcu[:, :su] = st[:, idx]
    wqe = np.concatenate([w["Wm"] @ w["Wq"][64 * h:64 * (h + 1), :] for h in heads])
    bqe = np.concatenate([w["Wm"] @ w["bq"][64 * h:64 * (h + 1)] + w["bm"]
                          for h in heads])
    wks = np.concatenate([w["Wk"][64 * h:64 * (h + 1), :] for h in heads])
    bks = np.concatenate([w["bk"][64 * h:64 * (h + 1)] for h in heads])
    wvs = np.concatenate([w["Wv"][64 * h:64 * (h + 1), :] for h in heads])
    bvs = np.concatenate([w["bv"][64 * h:64 * (h + 1)] for h in heads])
    mb = np.full(sup, NEG_BIG, np.float32)
    mb[:su] = 0.0
    f32 = np.float32
    src_res = np.ascontiguousarray(st[:, R * hg:R * (hg + 1)]) \
        + w["bo"][:, None].astype(f32)
    return {
        "src8": st.astype(fp8np),
        "src_res": src_res.astype(f32),
        "srcu8": srcu.astype(fp8np),
        "wq": _pack_dr(np.ascontiguousarray(wqe.T) * SQ).astype(fp8np),
        "wk": _pack_dr(np.ascontiguousarray(wks.T) * SK).astype(fp8np),
        "wv": (np.ascontiguousarray(wvs.T) * SV).astype(fp8np),
        "wo": _pack_dr(np.ascontiguousarray(w["Wo"].T) * SW).astype(fp8np),
        "w1": _pack_dr(np.ascontiguousarray(w["W1"].T) * SW).astype(fp8np),
        "w2": _pack_dr(np.ascontiguousarray(w["W2"].T) * SW).astype(fp8np),
        "bq": (bqe * SQ).reshape(2, 128).astype(f32),
        "bk": (bks * SK).reshape(2, 128).astype(f32),
        "bv": bvs.astype(f32),
        "b1": (w["b1"] * SW).reshape(16, 128).astype(f32),
        "bt1": w["ln1_b"].reshape(4, 128).astype(f32),
        "bt1f": (w["ln1_b"] + w["b2"]).reshape(4, 128).astype(f32),
        "g1": w["ln1_g"].reshape(4, 128).astype(f32),
        "g2": w["ln2_g"].reshape(4, 128).astype(f32),
        "bt2": w["ln2_b"].reshape(4, 128).astype(f32),
        "mb": mb.reshape(sup // 128, 128),
    }


def kernel(**inputs):
    global last_results
    w = {k: np.asarray(v, np.float32) for k, v in inputs.items()
         if k not in ("src", "src_mask")}
    src = np.asarray(inputs["src"], np.float32)
    mask = np.asarray(inputs["src_mask"]).reshape(B, S)
    idxs = [np.nonzero(mask[b] != 0)[0] for b in range(B)]
    sup = max(128, ((max(len(i) for i in idxs) + 127) // 128) * 128)

    if sup not in _built_cache:
        _built_cache[sup] = build_bass(sup)
    nc = _built_cache[sup]

    in_maps = [_prep_core(c, src, idxs, sup, w) for c in range(N_CORES)]
    res = bass_utils.run_bass_kernel_spmd(nc, in_maps, core_ids=list(range(N_CORES)),
                                          **run_kwargs)
    last_results = res
    out = np.empty((B, S, HID), np.float32)
    for c in range(N_CORES):
        b, hg = c // 2, c % 2
        out[b, R * hg:R * (hg + 1), :] = res.results[c]["out_t"].T
    return out


# revision 40
# speedup vs baseline: 1.0147x; 1.0147x over previous
"""Trainium2 Bass kernel for nn_EncoderLayer (multiplicative-attention encoder layer).

Sharding: 8 cores; core c handles batch b=c//2, head-group hg=c%2 (4 of 8 heads).
The reference's head-major reshape bug maps head h exactly to output rows
[256h, 256h+256), so each core owns 1024 complete output rows -> no collectives.

v3: - big GEMMs (QKV proj, Wo, FFN1, FFN2) in fp8e4 DoubleRow perf mode
      (0.5 PE cycles/out-col, K=256/pass) with host-prepacked weights and
      power-of-2 pre-scales folded into writer ops / the exp scale.
    - software-pipelined emission: chain work for head h's 256 output tokens
      is interleaved (generator round-robin) with head h+1's attention, so
      Act (exp-bound) and PE (GEMM-bound) run concurrently.
    - softmax tail: recip straight off PSUM den rows, Pool partition_broadcast
      replaces the PE broadcast matmul, xn multiply reads PSUM directly.

Per-token chain independence: LN1/FFN/LN2 normalize over features, so the
chain runs on 256-token blocks (one attention head's scrambled rows each).
FFN1 runs per 512-token pair to halve writer-instruction overhead.
"""

import numpy as np
import ml_dtypes

import concourse.bass as bass
import concourse.tile as tile
import concourse.bacc as bacc
from concourse import mybir
from concourse import bass_utils
from concourse import hw_specs as _hw_specs

_real_gat = _hw_specs.get_activation_tables


def _gat_pinned(arch):
    tabs = _real_gat(arch)
    return {name: (fns if name == "natural_log_exp_and_others" else set())
            for name, fns in tabs.items()}


bacc.get_activation_tables = _gat_pinned

B, S, HID, H, PF, D = 4, 2048, 512, 8, 2048, 64
N_CORES = 8
HPC = H // 2          # heads per core (4)
R = HPC * 256         # output rows per core (1024)
F32 = mybir.dt.float32
BF16 = mybir.dt.bfloat16
FP8 = mybir.dt.float8e4
AF = mybir.ActivationFunctionType
OP = mybir.AluOpType
DR = mybir.MatmulPerfMode.DoubleRow
NEG_BIG = -87.0
LN_EPS = 1e-5
SQ = 64.0             # wq scale
SK = 32.0             # wk scale
SV = 32.0             # wv scale
SX = 64.0             # xn scale
SW = 32.0             # wo/w1/w2 scale
fp8np = ml_dtypes.float8_e4m3

_built_cache = {}
last_results = None
run_kwargs = {}


def _bcast_ap(ap_1d, parts):
    return bass.AP(tensor=ap_1d.tensor, offset=ap_1d.offset,
                   ap=[[0, parts], *ap_1d.ap])


def _pack_dr(wT):
    """[K, M] (K mult of 256) -> DR-packed [K//256 * 128, 2 * M] host layout."""
    K, M = wT.shape
    return np.ascontiguousarray(
        wT.reshape(K // 256, 2, 128, M).transpose(0, 2, 1, 3)
    ).reshape(K // 2, 2 * M)


import os as _os
_RATIO = int(_os.environ.get("KRATIO", "1"))


def _interleave(*gens, ratio=None):
    # first generator gets `ratio` bursts per single burst of the others
    r = ratio if ratio is not None else _RATIO
    active = [iter(g) for g in gens]
    while active:
        for i, g in enumerate(list(active)):
            n = r if (i == 0 and len(active) > 1) else 1
            for _ in range(n):
                try:
                    next(g)
                except StopIteration:
                    if g in active:
                        active.remove(g)
                    break


def build_bass(sup):
    """Per-core module. sup = padded unmasked key count (mult of 128)."""
    KT = sup // 128
    nc = bacc.Bacc("TRN2", target_bir_lowering=False, debug=False,
                   num_devices=N_CORES)

    def inp(name, shape, dt=F32):
        return nc.dram_tensor(name, shape, dt, kind="ExternalInput").ap()

    src8_d = inp("src8", [HID, S], FP8)
    src_res_d = inp("src_res", [HID, R])         # fp32 src.T slice + bo
    srcu8_d = inp("srcu8", [HID, sup], FP8)
    wq_d = inp("wq", [2 * 128, 2 * 256], FP8)    # DR-packed SQ*(Wm@Wq).T
    wk_d = inp("wk", [2 * 128, 2 * 256], FP8)
    wv_d = inp("wv", [HID, 256], FP8)
    wo_d = inp("wo", [2 * 128, 2 * 512], FP8)
    w1_d = inp("w1", [2 * 128, 2 * PF], FP8)
    w2_d = inp("w2", [8 * 128, 2 * 512], FP8)
    bq_d = inp("bq", [2, 128])
    bk_d = inp("bk", [2, 128])
    bv_d = inp("bv", [256])
    b1_d = inp("b1", [16, 128])                  # SW*b1
    bt1_d = inp("bt1", [4, 128])                 # ln1_b
    bt1f_d = inp("bt1f", [4, 128])               # ln1_b + b2
    g1_d = inp("g1", [4, 128])
    g2_d = inp("g2", [4, 128])
    bt2_d = inp("bt2", [4, 128])
    mb_d = inp("mb", [KT, 128])
    out_d = nc.dram_tensor("out_t", [HID, R], F32, kind="ExternalOutput").ap()

    from contextlib import ExitStack
    with tile.TileContext(nc) as tc, ExitStack() as ctx:
        con = ctx.enter_context(tc.tile_pool(name="con", bufs=1))
        ppool = ctx.enter_context(tc.tile_pool(name="ps", bufs=2, space="PSUM"))
        pe_e = ctx.enter_context(tc.tile_pool(name="pe", bufs=2, space="PSUM"))
        pe_av = ctx.enter_context(tc.tile_pool(name="pav", bufs=2, space="PSUM"))
        att_pool = ctx.enter_context(tc.tile_pool(name="att", bufs=4))
        xn_pool = ctx.enter_context(tc.tile_pool(name="xn", bufs=2))
        rep_pool = ctx.enter_context(tc.tile_pool(name="rep", bufs=3))
        h1_pool = ctx.enter_context(tc.tile_pool(name="h1", bufs=2))
        tmp_pool = ctx.enter_context(tc.tile_pool(name="tmp", bufs=3))
        z_pool = ctx.enter_context(tc.tile_pool(name="z", bufs=2))
        o_pool = ctx.enter_context(tc.tile_pool(name="o", bufs=4))

        mm = nc.tensor.matmul
        act = nc.scalar.activation
        dve = nc.vector
        gps = nc.gpsimd

        def dma(out, in_):
            nc.sync.dma_start(out=out, in_=in_)

        def ctile(shape, dt, tag):
            return con.tile(shape, dt, tag=tag, name=tag)

        # ---- constants / weights ----
        srcu8 = ctile([128, 4, sup], FP8, "srcu8")
        wq8 = ctile([128, 2, 2, 256], FP8, "wq8")
        wk8 = ctile([128, 2, 2, 256], FP8, "wk8")
        wv8 = ctile([128, 4, 256], FP8, "wv8")
        dma(wk8, wk_d.rearrange("(t p) (i m) -> p t i m", t=2, i=2))
        dma(srcu8, srcu8_d.rearrange("(c p) n -> p c n", p=128))
        dma(wv8, wv_d.rearrange("(c p) m -> p c m", p=128))
        src8 = ctile([128, 4, S], FP8, "src8")
        dma(src8, src8_d.rearrange("(c p) n -> p c n", p=128))
        dma(wq8, wq_d.rearrange("(t p) (i m) -> p t i m", t=2, i=2))
        src_res = [ctile([128, R], F32, f"srcres{i}") for i in range(4)]
        wo8 = ctile([128, 2, 2, 512], FP8, "wo8")
        w18 = ctile([128, 2, 2, PF], FP8, "w18")
        w28 = ctile([128, 8, 2, 512], FP8, "w28")

        def load_chain_weights():
            dma(wo8, wo_d.rearrange("(t p) (i m) -> p t i m", t=2, i=2))
            for i in range(4):
                dma(src_res[i], src_res_d[128 * i:128 * (i + 1), :])
            dma(w18, w1_d.rearrange("(t p) (i m) -> p t i m", t=2, i=2))
            dma(w28, w2_d.rearrange("(t p) (i m) -> p t i m", t=8, i=2))

        def vec_in(dram, n, tag):
            t = ctile([128, n], F32, tag)
            dma(t, dram.rearrange("m p -> p m"))
            return t

        bq_sb = vec_in(bq_d, 2, "bq")
        bk_sb = vec_in(bk_d, 2, "bk")
        b1_sb = vec_in(b1_d, 16, "b1")
        g1_sb = vec_in(g1_d, 4, "g1")
        bt1_sb = vec_in(bt1_d, 4, "bt1")
        bt1f_sb = vec_in(bt1f_d, 4, "bt1f")
        g2_sb = vec_in(g2_d, 4, "g2")
        bt2_sb = vec_in(bt2_d, 4, "bt2")
        mb_sb = vec_in(mb_d, KT, "mb")
        bv_rep = ctile([128, 256], F32, "bvrep")
        dma(bv_rep, _bcast_ap(bv_d, 128))

        ones_bf = ctile([128, 128], BF16, "onesbf")
        dve.memset(ones_bf, 1.0)
        eps_t = ctile([128, 1], F32, "eps")
        dve.memset(eps_t, LN_EPS)

        q_sb = [ctile([128, S], BF16, f"q{m}") for m in range(2)]
        k_sb = [ctile([128, sup], BF16, f"k{m}") for m in range(2)]

        def gen_proj(w8, bias_sb, src_t, n_total, out_tiles, mt):
            n0 = 0
            while n0 < n_total:
                nq = min(512, n_total - n0)
                for half in range(2):
                    ps = ppool.tile([64, 512], F32, tag="ps", name="psp")
                    mcol = 64 * (2 * mt + half)
                    for t in range(2):
                        mm(ps[:, :nq],
                           w8[:, t, :, mcol:mcol + 64],
                           src_t[:, 2 * t:2 * t + 2, n0:n0 + nq],
                           start=(t == 0), stop=(t == 1), perf_mode=DR)
                    dve.tensor_scalar_add(
                        out_tiles[mt][64 * half:64 * half + 64, n0:n0 + nq],
                        ps[:, :nq],
                        bias_sb[64 * half:64 * half + 64, mt:mt + 1])
                    yield
                n0 += nq

        # ---- V natural [keys, 4*68] bf16 with ones cols (fp8 matmul) ----
        v_sb = ctile([128, KT * 4 * 68], BF16, "v")
        v_v = v_sb.rearrange("p (kt h e) -> p kt h e", kt=KT, h=4)

        def gen_vproj():
            # ones cols hold 1/SX so den rows accumulate den/SX and
            # rep4 = recip(den/SX) = SX/den (xn lands mid-range for fp8)
            dve.memset(v_v[:, :, :, 64:68], 1.0 / SX)
            for kt in range(KT):
                ps = ppool.tile([128, 512], F32, tag="ps", name="psv")
                for ct in range(4):
                    mm(ps[:, :256], srcu8[:, ct, 128 * kt:128 * (kt + 1)],
                       wv8[:, ct, :], start=(ct == 0), stop=(ct == 3))
                dve.scalar_tensor_tensor(
                    out=v_v[:, kt, :, 0:64],
                    in0=ps[:, :256].rearrange("p (h d) -> p h d", h=4),
                    scalar=1.0 / SV,
                    in1=bv_rep.rearrange("p (h d) -> p h d", h=4),
                    op0=OP.mult, op1=OP.add)
                yield

        # ---- attention head h -> fp8 xnp half (scaled by SX) ----
        def gen_attention(h, xnp):
            g = h // 2
            p0 = 64 * (h % 2)
            o0 = S * (h % 2)
            for q0 in range(0, S, 1024):
                avs = []
                for half in range(2):
                    avs.append(pe_av.tile([68, 512], F32, tag="av", name="av"))
                for kt in range(KT):
                    e = pe_e.tile([128, 1024], F32, tag="e", name="e")
                    with tc.high_priority():
                        for half in range(2):
                            mm(e[:, 512 * half:512 * (half + 1)],
                               k_sb[g][p0:p0 + 64, 128 * kt:128 * (kt + 1)],
                               q_sb[g][p0:p0 + 64,
                                       q0 + 512 * half:q0 + 512 * (half + 1)],
                               start=True, stop=True)
                    at = att_pool.tile([128, 1024], BF16, tag="att", name="att")
                    act(at, e, AF.Exp, bias=mb_sb[:, kt:kt + 1],
                        scale=1.0 / (SQ * SK))
                    for half in range(2):
                        mm(avs[half], v_v[:, kt, h, :],
                           at[:, 512 * half:512 * (half + 1)],
                           start=(kt == 0), stop=(kt == KT - 1),
                           skip_group_check=True)
                    yield
                # tail: rep = SX/den via recip of the 4 identical den rows;
                # broadcast row 0 to 64 partitions on Pool; xn = x' * rep.
                for half in range(2):
                    rep4 = rep_pool.tile([4, 512], BF16, tag="rep4", name="rep4")
                    with tc.high_priority(), \
                         nc.allow_low_precision(reason="softmax recip"):
                        dve.reciprocal(rep4, avs[half][64:68, :])
                    rep = rep_pool.tile([64, 512], BF16, tag="rep", name="rep")
                    gps.partition_broadcast(rep, rep4[0:1, :], channels=64)
                    dve.tensor_tensor(
                        out=xnp[0:64, o0 + q0 + 512 * half:o0 + q0 + 512 * (half + 1)],
                        in0=avs[half][0:64, :], in1=rep, op=OP.mult)
                    yield
            gps.tensor_copy(out=xnp[64:128, o0:o0 + S - 1],
                            in_=xnp[0:64, o0 + 1:o0 + S])
            yield

        # ---- layernorm on 4x[128, W] f32 z-tiles ----
        def gen_layernorm(z_tiles, g_sb, writers, W):
            # s1/s2 must sit in separate PSUM banks: a start=True matmul marks
            # its whole 2KB zero-region pending-zero, wiping any sibling
            # accumulation group sharing the bank.
            s1 = ppool.tile([128, W], F32, tag="ps", name="s1")
            s2 = ppool.tile([128, W], F32, tag="ps", name="s2")
            for ct in range(4):
                zb = tmp_pool.tile([128, W], BF16, tag="zb", name="zb")
                gps.tensor_copy(out=zb, in_=z_tiles[ct])
                sq = tmp_pool.tile([128, W], BF16, tag="sq", name="sq")
                dve.tensor_tensor(out=sq, in0=zb, in1=zb, op=OP.mult)
                mm(s1, ones_bf, zb, start=(ct == 0), stop=(ct == 3),
                   skip_group_check=True)
                mm(s2, ones_bf, sq, start=(ct == 0), stop=(ct == 3),
                   skip_group_check=True)
                yield
            bm = tmp_pool.tile([128, W], F32, tag="bm", name="bm")
            br = tmp_pool.tile([128, W], F32, tag="br", name="br")
            m2 = tmp_pool.tile([128, W], BF16, tag="m2", name="m2", bufs=1)
            dve.tensor_scalar_mul(bm, s1, 1.0 / HID)
            dve.tensor_tensor(out=m2, in0=bm, in1=bm, op=OP.mult)
            dve.scalar_tensor_tensor(out=br, in0=s2,
                                     scalar=1.0 / HID, in1=m2,
                                     op0=OP.mult, op1=OP.subtract)
            with tc.high_priority():
                act(br, br, AF.Ln, bias=eps_t)
                act(br, br, AF.Exp, scale=-0.5)
            yield
            for ct in range(4):
                sub = tmp_pool.tile([128, W], F32, tag="sub", name="sub")
                gps.tensor_tensor(out=sub, in0=z_tiles[ct], in1=bm,
                                  op=OP.subtract)
                t2 = tmp_pool.tile([128, W], F32, tag="t2", name="t2")
                dve.scalar_tensor_tensor(out=t2, in0=sub,
                                         scalar=g_sb[:, ct:ct + 1], in1=br,
                                         op0=OP.mult, op1=OP.mult)
                writers(ct, t2)
                yield

        # ---- chain A for one 256-token block: Wo(DR) + res, LN1 ----
        src1_f = [[con.tile([128, 256], F32, tag=f"s1f{i}_{j}",
                            name=f"s1f{i}_{j}") for j in range(4)]
                  for i in range(4)]
        src1_8 = ctile([128, 4, R], FP8, "src1_8")

        def gen_chain_a(blk, xnp):
            c0 = 256 * blk
            hh = blk % 2
            xw = xnp.rearrange("p (hh m j) -> p j hh m", hh=2, j=8)
            z1 = [z_pool.tile([128, 256], F32, tag=f"z{mt}", name=f"z{mt}")
                  for mt in range(4)]
            for mt in range(4):
                for half in range(2):
                    ps = ppool.tile([64, 512], F32, tag="ps", name="pswo")
                    mcol = 64 * (2 * mt + half)
                    for t in range(2):
                        ifm = bass.AP(
                            tensor=xw.tensor,
                            offset=xw.offset + (4 * t) * xw.ap[1][0]
                            + hh * xw.ap[2][0],
                            ap=[xw.ap[0], [2 * xw.ap[1][0], 2], xw.ap[3]])
                        mm(ps[:, :256], wo8[:, t, :, mcol:mcol + 64], ifm,
                           start=(t == 0), stop=(t == 1), perf_mode=DR)
                    dve.scalar_tensor_tensor(
                        out=z1[mt][64 * half:64 * half + 64, :],
                        in0=ps[:, :256],
                        scalar=1.0 / (SX * SW),
                        in1=src_res[mt][64 * half:64 * half + 64, c0:c0 + 256],
                        op0=OP.mult, op1=OP.add)
                    yield

            def w1(ct, t2):
                gps.tensor_scalar_add(src1_f[ct][blk], t2,
                                      bt1f_sb[:, ct:ct + 1])
                gps.tensor_scalar_add(src1_8[:, ct, c0:c0 + 256], t2,
                                      bt1_sb[:, ct:ct + 1])

            yield from gen_layernorm(z1, g1_sb, w1, 256)

        # ---- FFN1 for a 512-token pair (blocks 2p, 2p+1) ----
        h18s = {}

        def gen_ffn1(p):
            c0 = 512 * p
            h18 = h1_pool.tile([128, 16, 512], FP8, tag="h1", name="h1", bufs=2)
            h18s[p] = h18
            for mt in range(16):
                for half in range(2):
                    ps = ppool.tile([64, 512], F32, tag="ps", name="psf1")
                    mcol = 64 * (2 * mt + half)
                    for t in range(2):
                        mm(ps, w18[:, t, :, mcol:mcol + 64],
                           src1_8[:, 2 * t:2 * t + 2, c0:c0 + 512],
                           start=(t == 0), stop=(t == 1), perf_mode=DR)
                    if half == 0:
                        act(h18[0:64, mt, :], ps, AF.Relu,
                            bias=b1_sb[0:64, mt:mt + 1], scale=1.0)
                    else:
                        dve.tensor_scalar(
                            out=h18[64:128, mt, :], in0=ps,
                            scalar1=b1_sb[64:128, mt:mt + 1], scalar2=0.0,
                            op0=OP.add, op1=OP.max)
                    yield

        # ---- chain B for one 256-token block: FFN2(DR) + res, LN2, out ----
        z2s = {}

        def gen_chain_b_ffn(blk):
            c0 = 256 * blk
            h18 = h18s[blk // 2]
            r0 = 256 * (blk % 2)
            z2 = [z_pool.tile([128, 256], F32, tag=f"z{ot}", name=f"z{ot}")
                  for ot in range(4)]
            z2s[blk] = z2
            for ot in range(4):
                for half in range(2):
                    ps = ppool.tile([64, 512], F32, tag="ps", name="psf2")
                    mcol = 64 * (2 * ot + half)
                    for t in range(8):
                        mm(ps[:, :256], w28[:, t, :, mcol:mcol + 64],
                           h18[:, 2 * t:2 * t + 2, r0:r0 + 256],
                           start=(t == 0), stop=(t == 7), perf_mode=DR)
                    dve.scalar_tensor_tensor(
                        out=z2[ot][64 * half:64 * half + 64, :],
                        in0=ps[:, :256],
                        scalar=1.0 / (SW * SW),
                        in1=src1_f[ot][blk][64 * half:64 * half + 64, :],
                        op0=OP.mult, op1=OP.add)
                    yield

        def gen_chain_b_ln(blk):
            c0 = 256 * blk
            z2 = z2s[blk]

            def w2(ct, t2):
                o = o_pool.tile([128, 256], F32, tag="out", name="out", bufs=2)
                gps.tensor_scalar_add(o, t2, bt2_sb[:, ct:ct + 1])
                dma(out_d[128 * ct:128 * (ct + 1), c0:c0 + 256], o)

            yield from gen_layernorm(z2, g2_sb, w2, 256)

        def gen_chain_b(blk):
            yield from gen_chain_b_ffn(blk)
            yield from gen_chain_b_ln(blk)

        def gen_seq(*gens):
            for g in gens:
                yield from g

        # ---- schedule ----
        # V' ones cols hold 1/SX so den rows accumulate den/SX and
        # rep4 = recip(den/SX) = SX/den.  (memset inside gen_vproj runs first.)
        xnp0 = xn_pool.tile([128, 2 * S], FP8, tag="xn", name="xn")
        xnp1 = xn_pool.tile([128, 2 * S], FP8, tag="xn", name="xn")

        _interleave(gen_proj(wk8, bk_sb, srcu8, sup, k_sb, 0),
                    gen_proj(wq8, bq_sb, src8, S, q_sb, 0),
                    gen_vproj())
        load_chain_weights()
        _interleave(gen_attention(0, xnp0),
                    gen_seq(gen_proj(wk8, bk_sb, srcu8, sup, k_sb, 1),
                            gen_proj(wq8, bq_sb, src8, S, q_sb, 1)))
        _interleave(gen_attention(1, xnp0), gen_chain_a(0, xnp0))
        _interleave(gen_attention(2, xnp1),
                    gen_seq(gen_chain_a(1, xnp0), gen_ffn1(0)))
        _interleave(gen_attention(3, xnp1),
                    gen_seq(gen_chain_b(0), gen_chain_a(2, xnp1),
                            gen_chain_b(1)))
        _interleave(gen_seq(gen_chain_a(3, xnp1), gen_ffn1(1),
                            gen_chain_b(2), gen_chain_b(3)))

    nc.compile()
    return nc


def _prep_core(c, src, idxs, sup, w):
    b, hg = c // 2, c % 2
    heads = list(range(HPC * hg, HPC * hg + HPC))
    st = np.ascontiguousarray(src[b].T)                       # [512, 2048] f32
    idx = idxs[b]
    su = len(idx)
    srcu = np.zeros((HID, sup), np.float32)
    srcu[:, :su] = st[:, idx]
    wqe = np.concatenate([w["Wm"] @ w["Wq"][64 * h:64 * (h + 1), :] for h in heads])
    bqe = np.concatenate([w["Wm"] @ w["bq"][64 * h:64 * (h + 1)] + w["bm"]
                          for h in heads])
    wks = np.concatenate([w["Wk"][64 * h:64 * (h + 1), :] for h in heads])
    bks = np.concatenate([w["bk"][64 * h:64 * (h + 1)] for h in heads])
    wvs = np.concatenate([w["Wv"][64 * h:64 * (h + 1), :] for h in heads])
    bvs = np.concatenate([w["bv"][64 * h:64 * (h + 1)] for h in heads])
    mb = np.full(sup, NEG_BIG, np.float32)
    mb[:su] = 0.0
    f32 = np.float32
    src_res = np.ascontiguousarray(st[:, R * hg:R * (hg + 1)]) \
        + w["bo"][:, None].astype(f32)
    return {
        "src8": st.astype(fp8np),
        "src_res": src_res.astype(f32),
        "srcu8": srcu.astype(fp8np),
        "wq": _pack_dr(np.ascontiguousarray(wqe.T) * SQ).astype(fp8np),
        "wk": _pack_dr(np.ascontiguousarray(wks.T) * SK).astype(fp8np),
        "wv": (np.ascontiguousarray(wvs.T) * SV).astype(fp8np),
        "wo": _pack_dr(np.ascontiguousarray(w["Wo"].T) * SW).astype(fp8np),
        "w1": _pack_dr(np.ascontiguousarray(w["W1"].T) * SW).astype(fp8np),
        "w2": _pack_dr(np.ascontiguousarray(w["W2"].T) * SW).astype(fp8np),
        "bq": (bqe * SQ).reshape(2, 128).astype(f32),
        "bk": (bks * SK).reshape(2, 128).astype(f32),
        "bv": bvs.astype(f32),
        "b1": (w["b1"] * SW).reshape(16, 128).astype(f32),
        "bt1": w["ln1_b"].reshape(4, 128).astype(f32),
        "bt1f": (w["ln1_b"] + w["b2"]).reshape(4, 128).astype(f32),
        "g1": w["ln1_g"].reshape(4, 128).astype(f32),
        "g2": w["ln2_g"].reshape(4, 128).astype(f32),
        "bt2": w["ln2_b"].reshape(4, 128).astype(f32),
        "mb": mb.reshape(sup // 128, 128),
    }


def kernel(**inputs):
    global last_results
    w = {k: np.asarray(v, np.float32) for k, v in inputs.items()
         if k not in ("src", "src_mask")}
    src = np.asarray(inputs["src"], np.float32)
    mask = np.asarray(inputs["src_mask"]).reshape(B, S)
    idxs = [np.nonzero(mask[b] != 0)[0] for b in range(B)]
    sup = max(128, ((max(len(i) for i in idxs) + 127) // 128) * 128)

    if sup not in _built_cache:
        _built_cache[sup] = build_bass(sup)
    nc = _built_cache[sup]

    in_maps = [_prep_core(c, src, idxs, sup, w) for c in range(N_CORES)]
    res = bass_utils.run_bass_kernel_spmd(nc, in_maps, core_ids=list(range(N_CORES)),
                                          **run_kwargs)
    last_results = res
    out = np.empty((B, S, HID), np.float32)
    for c in range(N_CORES):
        b, hg = c // 2, c % 2
        out[b, R * hg:R * (hg + 1), :] = res.results[c]["out_t"].T
    return out


# revision 48
# speedup vs baseline: 1.0358x; 1.0208x over previous
"""Trainium2 Bass kernel for nn_EncoderLayer (multiplicative-attention encoder layer).

Sharding: 8 cores; core c handles batch b=c//2, head-group hg=c%2 (4 of 8 heads).
The reference's head-major reshape bug maps head h exactly to output rows
[256h, 256h+256), so each core owns 1024 complete output rows -> no collectives.

v3: - big GEMMs (QKV proj, Wo, FFN1, FFN2) in fp8e4 DoubleRow perf mode
      (0.5 PE cycles/out-col, K=256/pass) with host-prepacked weights and
      power-of-2 pre-scales folded into writer ops / the exp scale.
    - software-pipelined emission: chain work for head h's 256 output tokens
      is interleaved (generator round-robin) with head h+1's attention, so
      Act (exp-bound) and PE (GEMM-bound) run concurrently.
    - softmax tail: recip straight off PSUM den rows, Pool partition_broadcast
      replaces the PE broadcast matmul, xn multiply reads PSUM directly.

Per-token chain independence: LN1/FFN/LN2 normalize over features, so the
chain runs on 256-token blocks (one attention head's scrambled rows each).
FFN1 runs per 512-token pair to halve writer-instruction overhead.
"""

import numpy as np
import ml_dtypes

import concourse.bass as bass
import concourse.tile as tile
import concourse.bacc as bacc
from concourse import mybir
from concourse import bass_utils
from concourse import hw_specs as _hw_specs

_real_gat = _hw_specs.get_activation_tables


def _gat_pinned(arch):
    tabs = _real_gat(arch)
    return {name: (fns if name == "natural_log_exp_and_others" else set())
            for name, fns in tabs.items()}


bacc.get_activation_tables = _gat_pinned

B, S, HID, H, PF, D = 4, 2048, 512, 8, 2048, 64
N_CORES = 8
HPC = H // 2          # heads per core (4)
R = HPC * 256         # output rows per core (1024)
F32 = mybir.dt.float32
BF16 = mybir.dt.bfloat16
FP8 = mybir.dt.float8e4
AF = mybir.ActivationFunctionType
OP = mybir.AluOpType
DR = mybir.MatmulPerfMode.DoubleRow
NEG_BIG = -87.0
LN_EPS = 1e-5
SQ = 64.0             # wq scale
SK = 32.0             # wk scale
SV = 32.0             # wv scale
SX = 64.0             # xn scale
SW = 32.0             # wo/w1/w2 scale
fp8np = ml_dtypes.float8_e4m3

_built_cache = {}
last_results = None
run_kwargs = {}


def _bcast_ap(ap_1d, parts):
    return bass.AP(tensor=ap_1d.tensor, offset=ap_1d.offset,
                   ap=[[0, parts], *ap_1d.ap])


def _pack_dr(wT):
    """[K, M] (K mult of 256) -> DR-packed [K//256 * 128, 2 * M] host layout."""
    K, M = wT.shape
    return np.ascontiguousarray(
        wT.reshape(K // 256, 2, 128, M).transpose(0, 2, 1, 3)
    ).reshape(K // 2, 2 * M)


import os as _os
_RATIO = int(_os.environ.get("KRATIO", "1"))


def _interleave(*gens, ratio=None):
    # first generator gets `ratio` bursts per single burst of the others
    r = ratio if ratio is not None else _RATIO
    active = [iter(g) for g in gens]
    while active:
        for i, g in enumerate(list(active)):
            n = r if (i == 0 and len(active) > 1) else 1
            for _ in range(n):
                try:
                    next(g)
                except StopIteration:
                    if g in active:
                        active.remove(g)
                    break


def build_bass(sup):
    """Per-core module. sup = padded unmasked key count (mult of 128)."""
    KT = sup // 128
    nc = bacc.Bacc("TRN2", target_bir_lowering=False, debug=False,
                   num_devices=N_CORES)

    def inp(name, shape, dt=F32):
        return nc.dram_tensor(name, shape, dt, kind="ExternalInput").ap()

    src8_d = inp("src8", [HID, S], FP8)
    src_res_d = inp("src_res", [HID, R])         # fp32 src.T slice + bo
    srcu8_d = inp("srcu8", [HID, sup], FP8)
    wq_d = inp("wq", [2 * 128, 2 * 256], FP8)    # DR-packed SQ*(Wm@Wq).T
    wk_d = inp("wk", [2 * 128, 2 * 256], FP8)
    wv_d = inp("wv", [HID, 256], FP8)
    wo_d = inp("wo", [2 * 128, 2 * 512], FP8)
    w1_d = inp("w1", [2 * 128, 2 * PF], FP8)
    w2_d = inp("w2", [8 * 128, 2 * 512], FP8)
    bq_d = inp("bq", [2, 128])
    bk_d = inp("bk", [2, 128])
    bv_d = inp("bv", [256])
    b1_d = inp("b1", [16, 128])                  # SW*b1
    bt1_d = inp("bt1", [4, 128])                 # ln1_b
    bt1f_d = inp("bt1f", [4, 128])               # ln1_b + b2
    g1_d = inp("g1", [4, 128])
    g2_d = inp("g2", [4, 128])
    bt2_d = inp("bt2", [4, 128])
    mb_d = inp("mb", [KT, 128])
    out_d = nc.dram_tensor("out_t", [HID, R], F32, kind="ExternalOutput").ap()

    from contextlib import ExitStack
    with tile.TileContext(nc) as tc, ExitStack() as ctx:
        con = ctx.enter_context(tc.tile_pool(name="con", bufs=1))
        ppool = ctx.enter_context(tc.tile_pool(name="ps", bufs=2, space="PSUM"))
        pe_e = ctx.enter_context(tc.tile_pool(name="pe", bufs=2, space="PSUM"))
        pe_av = ctx.enter_context(tc.tile_pool(name="pav", bufs=2, space="PSUM"))
        att_pool = ctx.enter_context(tc.tile_pool(name="att", bufs=4))
        xn_pool = ctx.enter_context(tc.tile_pool(name="xn", bufs=2))
        rep_pool = ctx.enter_context(tc.tile_pool(name="rep", bufs=3))
        h1_pool = ctx.enter_context(tc.tile_pool(name="h1", bufs=2))
        tmp_pool = ctx.enter_context(tc.tile_pool(name="tmp", bufs=3))
        z_pool = ctx.enter_context(tc.tile_pool(name="z", bufs=2))
        o_pool = ctx.enter_context(tc.tile_pool(name="o", bufs=4))

        mm = nc.tensor.matmul
        act = nc.scalar.activation
        dve = nc.vector
        gps = nc.gpsimd

        def dma(out, in_):
            nc.sync.dma_start(out=out, in_=in_)

        def ctile(shape, dt, tag):
            return con.tile(shape, dt, tag=tag, name=tag)

        # ---- constants / weights ----
        srcu8 = ctile([128, 4, sup], FP8, "srcu8")
        wq8 = ctile([128, 2, 2, 256], FP8, "wq8")
        wk8 = ctile([128, 2, 2, 256], FP8, "wk8")
        wv8 = ctile([128, 4, 256], FP8, "wv8")
        dma(wk8, wk_d.rearrange("(t p) (i m) -> p t i m", t=2, i=2))
        dma(srcu8, srcu8_d.rearrange("(c p) n -> p c n", p=128))
        dma(wv8, wv_d.rearrange("(c p) m -> p c m", p=128))
        src8 = ctile([128, 4, S], FP8, "src8")
        dma(src8, src8_d.rearrange("(c p) n -> p c n", p=128))
        dma(wq8, wq_d.rearrange("(t p) (i m) -> p t i m", t=2, i=2))
        src_res = [ctile([128, R], F32, f"srcres{i}") for i in range(4)]
        wo8 = ctile([128, 2, 2, 512], FP8, "wo8")
        w18 = ctile([128, 2, 2, PF], FP8, "w18")
        w28 = ctile([128, 8, 2, 512], FP8, "w28")

        def load_chain_weights():
            dma(wo8, wo_d.rearrange("(t p) (i m) -> p t i m", t=2, i=2))
            for i in range(4):
                dma(src_res[i], src_res_d[128 * i:128 * (i + 1), :])
            dma(w18, w1_d.rearrange("(t p) (i m) -> p t i m", t=2, i=2))
            dma(w28, w2_d.rearrange("(t p) (i m) -> p t i m", t=8, i=2))

        def vec_in(dram, n, tag):
            t = ctile([128, n], F32, tag)
            dma(t, dram.rearrange("m p -> p m"))
            return t

        bq_sb = vec_in(bq_d, 2, "bq")
        bk_sb = vec_in(bk_d, 2, "bk")
        b1_sb = vec_in(b1_d, 16, "b1")
        g1_sb = vec_in(g1_d, 4, "g1")
        bt1_sb = vec_in(bt1_d, 4, "bt1")
        bt1f_sb = vec_in(bt1f_d, 4, "bt1f")
        g2_sb = vec_in(g2_d, 4, "g2")
        bt2_sb = vec_in(bt2_d, 4, "bt2")
        mb_sb = vec_in(mb_d, KT, "mb")
        bv_rep = ctile([128, 256], F32, "bvrep")
        dma(bv_rep, _bcast_ap(bv_d, 128))

        ones_bf = ctile([128, 128], BF16, "onesbf")
        dve.memset(ones_bf, 1.0)
        eps_t = ctile([128, 1], F32, "eps")
        dve.memset(eps_t, LN_EPS)

        q_sb = [ctile([128, S], BF16, f"q{m}") for m in range(2)]
        k_sb = [ctile([128, sup], BF16, f"k{m}") for m in range(2)]

        def gen_proj(w8, bias_sb, src_t, n_total, out_tiles, mt):
            n0 = 0
            while n0 < n_total:
                nq = min(512, n_total - n0)
                for half in range(2):
                    ps = ppool.tile([64, 512], F32, tag="ps", name="psp")
                    mcol = 64 * (2 * mt + half)
                    for t in range(2):
                        mm(ps[:, :nq],
                           w8[:, t, :, mcol:mcol + 64],
                           src_t[:, 2 * t:2 * t + 2, n0:n0 + nq],
                           start=(t == 0), stop=(t == 1), perf_mode=DR)
                    dve.tensor_scalar_add(
                        out_tiles[mt][64 * half:64 * half + 64, n0:n0 + nq],
                        ps[:, :nq],
                        bias_sb[64 * half:64 * half + 64, mt:mt + 1])
                    yield
                n0 += nq

        # ---- V natural [keys, 4*128] bf16: 64 V cols + 64 ones cols ----
        # (AV matmul cost is N-proportional, M-free: 64 ones cols give 64
        # identical den rows so recip writes [64, W] directly -- no broadcast)
        v_sb = ctile([128, KT * 4 * 128], BF16, "v")
        v_v = v_sb.rearrange("p (kt h e) -> p kt h e", kt=KT, h=4)

        def gen_vproj():
            # ones cols hold 1/SX so den rows accumulate den/SX and
            # rep = recip(den/SX) = SX/den (xn lands mid-range for fp8)
            dve.memset(v_v[:, :, :, 64:128], 1.0 / SX)
            for kt in range(KT):
                ps = ppool.tile([128, 512], F32, tag="ps", name="psv")
                for ct in range(4):
                    mm(ps[:, :256], srcu8[:, ct, 128 * kt:128 * (kt + 1)],
                       wv8[:, ct, :], start=(ct == 0), stop=(ct == 3))
                dve.scalar_tensor_tensor(
                    out=v_v[:, kt, :, 0:64],
                    in0=ps[:, :256].rearrange("p (h d) -> p h d", h=4),
                    scalar=1.0 / SV,
                    in1=bv_rep.rearrange("p (h d) -> p h d", h=4),
                    op0=OP.mult, op1=OP.add)
                yield

        # ---- attention head h -> fp8 xnp half (scaled by SX) ----
        def gen_attention(h, xnp):
            g = h // 2
            p0 = 64 * (h % 2)
            o0 = S * (h % 2)
            for q0 in range(0, S, 1024):
                avs = []
                for half in range(2):
                    avs.append(pe_av.tile([128, 512], F32, tag="av", name="av"))
                for kt in range(KT):
                    e = pe_e.tile([128, 1024], F32, tag="e", name="e")
                    with tc.high_priority():
                        for half in range(2):
                            mm(e[:, 512 * half:512 * (half + 1)],
                               k_sb[g][p0:p0 + 64, 128 * kt:128 * (kt + 1)],
                               q_sb[g][p0:p0 + 64,
                                       q0 + 512 * half:q0 + 512 * (half + 1)],
                               start=True, stop=True)
                    at = att_pool.tile([128, 1024], BF16, tag="att", name="att")
                    act(at, e, AF.Exp, bias=mb_sb[:, kt:kt + 1],
                        scale=1.0 / (SQ * SK))
                    for half in range(2):
                        mm(avs[half], v_v[:, kt, h, :],
                           at[:, 512 * half:512 * (half + 1)],
                           start=(kt == 0), stop=(kt == KT - 1),
                           skip_group_check=True)
                    yield
                # tail: rep = SX/den via recip of the 4 identical den rows;
                # broadcast row 0 to 64 partitions on Pool; xn = x' * rep.
                for half in range(2):
                    rep = rep_pool.tile([64, 512], BF16, tag="rep", name="rep")
                    with tc.high_priority(), \
                         nc.allow_low_precision(reason="softmax recip"):
                        dve.reciprocal(rep, avs[half][64:128, :])
                    with tc.high_priority():
                        dve.tensor_tensor(
                            out=xnp[0:64, o0 + q0 + 512 * half:o0 + q0 + 512 * (half + 1)],
                            in0=avs[half][0:64, :], in1=rep, op=OP.mult)
                    yield
            with tc.high_priority():
                gps.tensor_copy(out=xnp[64:128, o0:o0 + S // 2],
                                in_=xnp[0:64, o0 + 1:o0 + S // 2 + 1])
                gps.tensor_copy(out=xnp[64:128, o0 + S // 2:o0 + S - 1],
                                in_=xnp[0:64, o0 + S // 2 + 1:o0 + S])
            yield

        # ---- layernorm on 4x[128, W] f32 z-tiles ----
        def gen_layernorm(z_tiles, g_sb, writers, W, stats_pool=None):
            # s1/s2 must sit in separate PSUM banks: a start=True matmul marks
            # its whole 2KB zero-region pending-zero, wiping any sibling
            # accumulation group sharing the bank.  stats_pool lets tail LNs
            # borrow the idle attention av-tag banks so concurrent chain
            # lanes don't cycle-deadlock on the shared chain psum tag.
            sp = stats_pool
            if sp is None:
                s1 = ppool.tile([128, W], F32, tag="ps", name="s1")
                s2 = ppool.tile([128, W], F32, tag="ps", name="s2")
            else:
                s1 = sp.tile([128, W], F32, tag="av", name="s1")
                s2 = sp.tile([128, W], F32, tag="av", name="s2")
            for ct in range(4):
                zb = tmp_pool.tile([128, W], BF16, tag="zb", name="zb")
                gps.tensor_copy(out=zb, in_=z_tiles[ct])
                sq = tmp_pool.tile([128, W], BF16, tag="sq", name="sq")
                dve.tensor_tensor(out=sq, in0=zb, in1=zb, op=OP.mult)
                mm(s1, ones_bf, zb, start=(ct == 0), stop=(ct == 3),
                   skip_group_check=True)
                mm(s2, ones_bf, sq, start=(ct == 0), stop=(ct == 3),
                   skip_group_check=True)
                yield
            bm = tmp_pool.tile([128, W], F32, tag="bm", name="bm")
            br = tmp_pool.tile([128, W], F32, tag="br", name="br")
            m2 = tmp_pool.tile([128, W], BF16, tag="m2", name="m2", bufs=1)
            dve.tensor_scalar_mul(bm, s1, 1.0 / HID)
            dve.tensor_tensor(out=m2, in0=bm, in1=bm, op=OP.mult)
            dve.scalar_tensor_tensor(out=br, in0=s2,
                                     scalar=1.0 / HID, in1=m2,
                                     op0=OP.mult, op1=OP.subtract)
            with tc.high_priority():
                act(br, br, AF.Ln, bias=eps_t)
                act(br, br, AF.Exp, scale=-0.5)
            yield
            for ct in range(4):
                sub = tmp_pool.tile([128, W], F32, tag="sub", name="sub")
                gps.tensor_tensor(out=sub, in0=z_tiles[ct], in1=bm,
                                  op=OP.subtract)
                t2 = tmp_pool.tile([128, W], F32, tag="t2", name="t2")
                dve.scalar_tensor_tensor(out=t2, in0=sub,
                                         scalar=g_sb[:, ct:ct + 1], in1=br,
                                         op0=OP.mult, op1=OP.mult)
                writers(ct, t2)
                yield

        # ---- chain A for one 256-token block: Wo(DR) + res, LN1 ----
        src1_f = [[con.tile([128, 256], F32, tag=f"s1f{i}_{j}",
                            name=f"s1f{i}_{j}") for j in range(4)]
                  for i in range(4)]
        src1_8 = ctile([128, 4, R], FP8, "src1_8")

        def gen_chain_a(blk, xnp):
            c0 = 256 * blk
            hh = blk % 2
            xw = xnp.rearrange("p (hh m j) -> p j hh m", hh=2, j=8)
            z1 = [z_pool.tile([128, 256], F32, tag=f"z{mt}", name=f"z{mt}")
                  for mt in range(4)]
            for mt in range(4):
                for half in range(2):
                    ps = ppool.tile([64, 512], F32, tag="ps", name="pswo")
                    mcol = 64 * (2 * mt + half)
                    for t in range(2):
                        ifm = bass.AP(
                            tensor=xw.tensor,
                            offset=xw.offset + (4 * t) * xw.ap[1][0]
                            + hh * xw.ap[2][0],
                            ap=[xw.ap[0], [2 * xw.ap[1][0], 2], xw.ap[3]])
                        mm(ps[:, :256], wo8[:, t, :, mcol:mcol + 64], ifm,
                           start=(t == 0), stop=(t == 1), perf_mode=DR)
                    dve.scalar_tensor_tensor(
                        out=z1[mt][64 * half:64 * half + 64, :],
                        in0=ps[:, :256],
                        scalar=1.0 / (SX * SW),
                        in1=src_res[mt][64 * half:64 * half + 64, c0:c0 + 256],
                        op0=OP.mult, op1=OP.add)
                    yield

            def w1(ct, t2):
                gps.tensor_scalar_add(src1_f[ct][blk], t2,
                                      bt1f_sb[:, ct:ct + 1])
                gps.tensor_scalar_add(src1_8[:, ct, c0:c0 + 256], t2,
                                      bt1_sb[:, ct:ct + 1])

            yield from gen_layernorm(z1, g1_sb, w1, 256)

        # ---- FFN1 for a 512-token pair (blocks 2p, 2p+1) ----
        h18s = {}

        def gen_ffn1(p):
            c0 = 512 * p
            h18 = h1_pool.tile([128, 16, 512], FP8, tag="h1", name="h1", bufs=2)
            h18s[p] = h18
            for mt in range(16):
                for half in range(2):
                    ps = ppool.tile([64, 512], F32, tag="ps", name="psf1")
                    mcol = 64 * (2 * mt + half)
                    for t in range(2):
                        mm(ps, w18[:, t, :, mcol:mcol + 64],
                           src1_8[:, 2 * t:2 * t + 2, c0:c0 + 512],
                           start=(t == 0), stop=(t == 1), perf_mode=DR)
                    if half == 0:
                        act(h18[0:64, mt, :], ps, AF.Relu,
                            bias=b1_sb[0:64, mt:mt + 1], scale=1.0)
                    else:
                        dve.tensor_scalar(
                            out=h18[64:128, mt, :], in0=ps,
                            scalar1=b1_sb[64:128, mt:mt + 1], scalar2=0.0,
                            op0=OP.add, op1=OP.max)
                    yield

        # ---- chain B for one 256-token block: FFN2(DR) + res, LN2, out ----
        z2s = {}

        def gen_chain_b_ffn(blk):
            c0 = 256 * blk
            h18 = h18s[blk // 2]
            r0 = 256 * (blk % 2)
            z2 = [z_pool.tile([128, 256], F32, tag=f"z{ot}", name=f"z{ot}")
                  for ot in range(4)]
            z2s[blk] = z2
            for ot in range(4):
                for half in range(2):
                    ps = ppool.tile([64, 512], F32, tag="ps", name="psf2")
                    mcol = 64 * (2 * ot + half)
                    for t in range(8):
                        mm(ps[:, :256], w28[:, t, :, mcol:mcol + 64],
                           h18[:, 2 * t:2 * t + 2, r0:r0 + 256],
                           start=(t == 0), stop=(t == 7), perf_mode=DR)
                    dve.scalar_tensor_tensor(
                        out=z2[ot][64 * half:64 * half + 64, :],
                        in0=ps[:, :256],
                        scalar=1.0 / (SW * SW),
                        in1=src1_f[ot][blk][64 * half:64 * half + 64, :],
                        op0=OP.mult, op1=OP.add)
                    yield

        def gen_chain_b_ln(blk, stats_pool=None):
            c0 = 256 * blk
            z2 = z2s[blk]

            def w2(ct, t2):
                o = o_pool.tile([128, 256], F32, tag="out", name="out", bufs=2)
                gps.tensor_scalar_add(o, t2, bt2_sb[:, ct:ct + 1])
                dma(out_d[128 * ct:128 * (ct + 1), c0:c0 + 256], o)

            yield from gen_layernorm(z2, g2_sb, w2, 256, stats_pool=stats_pool)

        def gen_chain_b(blk, stats_pool=None):
            yield from gen_chain_b_ffn(blk)
            yield from gen_chain_b_ln(blk, stats_pool=stats_pool)

        def gen_seq(*gens):
            for g in gens:
                yield from g

        # ---- schedule ----
        # V' ones cols hold 1/SX so den rows accumulate den/SX and
        # rep4 = recip(den/SX) = SX/den.  (memset inside gen_vproj runs first.)
        xnp0 = xn_pool.tile([128, 2 * S], FP8, tag="xn", name="xn")
        xnp1 = xn_pool.tile([128, 2 * S], FP8, tag="xn", name="xn")

        _interleave(gen_proj(wk8, bk_sb, srcu8, sup, k_sb, 0),
                    gen_proj(wq8, bq_sb, src8, S, q_sb, 0),
                    gen_vproj())
        load_chain_weights()
        _interleave(gen_attention(0, xnp0),
                    gen_seq(gen_proj(wk8, bk_sb, srcu8, sup, k_sb, 1),
                            gen_proj(wq8, bq_sb, src8, S, q_sb, 1)))
        _interleave(gen_attention(1, xnp0), gen_chain_a(0, xnp0))
        _interleave(gen_attention(2, xnp1),
                    gen_seq(gen_chain_a(1, xnp0), gen_ffn1(0)))
        _interleave(gen_attention(3, xnp1),
                    gen_seq(gen_chain_b(0), gen_chain_a(2, xnp1),
                            gen_chain_b(1)))
        _interleave(gen_seq(gen_chain_a(3, xnp1), gen_ffn1(1),
                            gen_chain_b(2), gen_chain_b(3)))

    nc.compile()
    return nc


def _prep_core(c, src, idxs, sup, w):
    b, hg = c // 2, c % 2
    heads = list(range(HPC * hg, HPC * hg + HPC))
    st = np.ascontiguousarray(src[b].T)                       # [512, 2048] f32
    idx = idxs[b]
    su = len(idx)
    srcu = np.zeros((HID, sup), np.float32)
    srcu[:, :su] = st[:, idx]
    wqe = np.concatenate([w["Wm"] @ w["Wq"][64 * h:64 * (h + 1), :] for h in heads])
    bqe = np.concatenate([w["Wm"] @ w["bq"][64 * h:64 * (h + 1)] + w["bm"]
                          for h in heads])
    wks = np.concatenate([w["Wk"][64 * h:64 * (h + 1), :] for h in heads])
    bks = np.concatenate([w["bk"][64 * h:64 * (h + 1)] for h in heads])
    wvs = np.concatenate([w["Wv"][64 * h:64 * (h + 1), :] for h in heads])
    bvs = np.concatenate([w["bv"][64 * h:64 * (h + 1)] for h in heads])
    mb = np.full(sup, NEG_BIG, np.float32)
    mb[:su] = 0.0
    f32 = np.float32
    src_res = np.ascontiguousarray(st[:, R * hg:R * (hg + 1)]) \
        + w["bo"][:, None].astype(f32)
    return {
        "src8": st.astype(fp8np),
        "src_res": src_res.astype(f32),
        "srcu8": srcu.astype(fp8np),
        "wq": _pack_dr(np.ascontiguousarray(wqe.T) * SQ).astype(fp8np),
        "wk": _pack_dr(np.ascontiguousarray(wks.T) * SK).astype(fp8np),
        "wv": (np.ascontiguousarray(wvs.T) * SV).astype(fp8np),
        "wo": _pack_dr(np.ascontiguousarray(w["Wo"].T) * SW).astype(fp8np),
        "w1": _pack_dr(np.ascontiguousarray(w["W1"].T) * SW).astype(fp8np),
        "w2": _pack_dr(np.ascontiguousarray(w["W2"].T) * SW).astype(fp8np),
        "bq": (bqe * SQ).reshape(2, 128).astype(f32),
        "bk": (bks * SK).reshape(2, 128).astype(f32),
        "bv": bvs.astype(f32),
        "b1": (w["b1"] * SW).reshape(16, 128).astype(f32),
        "bt1": w["ln1_b"].reshape(4, 128).astype(f32),
        "bt1f": (w["ln1_b"] + w["b2"]).reshape(4, 128).astype(f32),
        "g1": w["ln1_g"].reshape(4, 128).astype(f32),
        "g2": w["ln2_g"].reshape(4, 128).astype(f32),
        "bt2": w["ln2_b"].reshape(4, 128).astype(f32),
        "mb": mb.reshape(sup // 128, 128),
    }


def kernel(**inputs):
    global last_results
    w = {k: np.asarray(v, np.float32) for k, v in inputs.items()
         if k not in ("src", "src_mask")}
    src = np.asarray(inputs["src"], np.float32)
    mask = np.asarray(inputs["src_mask"]).reshape(B, S)
    idxs = [np.nonzero(mask[b] != 0)[0] for b in range(B)]
    sup = max(128, ((max(len(i) for i in idxs) + 127) // 128) * 128)

    if sup not in _built_cache:
        _built_cache[sup] = build_bass(sup)
    nc = _built_cache[sup]

    in_maps = [_prep_core(c, src, idxs, sup, w) for c in range(N_CORES)]
    res = bass_utils.run_bass_kernel_spmd(nc, in_maps, core_ids=list(range(N_CORES)),
                                          **run_kwargs)
    last_results = res
    out = np.empty((B, S, HID), np.float32)
    for c in range(N_CORES):
        b, hg = c // 2, c % 2
        out[b, R * hg:R * (hg + 1), :] = res.results[c]["out_t"].T
    return out


# revision 55
# speedup vs baseline: 1.0432x; 1.0071x over previous
"""Trainium2 Bass kernel for nn_EncoderLayer (multiplicative-attention encoder layer).

Sharding: 8 cores; core c handles batch b=c//2, head-group hg=c%2 (4 of 8 heads).
The reference's head-major reshape bug maps head h exactly to output rows
[256h, 256h+256), so each core owns 1024 complete output rows -> no collectives.

v3: - big GEMMs (QKV proj, Wo, FFN1, FFN2) in fp8e4 DoubleRow perf mode
      (0.5 PE cycles/out-col, K=256/pass) with host-prepacked weights and
      power-of-2 pre-scales folded into writer ops / the exp scale.
    - software-pipelined emission: chain work for head h's 256 output tokens
      is interleaved (generator round-robin) with head h+1's attention, so
      Act (exp-bound) and PE (GEMM-bound) run concurrently.
    - softmax tail: recip straight off PSUM den rows, Pool partition_broadcast
      replaces the PE broadcast matmul, xn multiply reads PSUM directly.

Per-token chain independence: LN1/FFN/LN2 normalize over features, so the
chain runs on 256-token blocks (one attention head's scrambled rows each).
FFN1 runs per 512-token pair to halve writer-instruction overhead.
"""

import numpy as np
import ml_dtypes

import concourse.bass as bass
import concourse.tile as tile
import concourse.bacc as bacc
from concourse import mybir
from concourse import bass_utils
from concourse import hw_specs as _hw_specs

_real_gat = _hw_specs.get_activation_tables


def _gat_pinned(arch):
    tabs = _real_gat(arch)
    return {name: (fns if name == "natural_log_exp_and_others" else set())
            for name, fns in tabs.items()}


bacc.get_activation_tables = _gat_pinned

B, S, HID, H, PF, D = 4, 2048, 512, 8, 2048, 64
N_CORES = 8
HPC = H // 2          # heads per core (4)
R = HPC * 256         # output rows per core (1024)
F32 = mybir.dt.float32
BF16 = mybir.dt.bfloat16
FP8 = mybir.dt.float8e4
AF = mybir.ActivationFunctionType
OP = mybir.AluOpType
DR = mybir.MatmulPerfMode.DoubleRow
NEG_BIG = -87.0
LN_EPS = 1e-5
SQ = 64.0             # wq scale
SK = 32.0             # wk scale
SV = 32.0             # wv scale
SX = 64.0             # xn scale
SW = 32.0             # wo/w1/w2 scale
fp8np = ml_dtypes.float8_e4m3

_built_cache = {}
last_results = None
run_kwargs = {}


def _bcast_ap(ap_1d, parts):
    return bass.AP(tensor=ap_1d.tensor, offset=ap_1d.offset,
                   ap=[[0, parts], *ap_1d.ap])


def _pack_dr(wT):
    """[K, M] (K mult of 256) -> DR-packed [K//256 * 128, 2 * M] host layout."""
    K, M = wT.shape
    return np.ascontiguousarray(
        wT.reshape(K // 256, 2, 128, M).transpose(0, 2, 1, 3)
    ).reshape(K // 2, 2 * M)


import os as _os
_RATIO = int(_os.environ.get("KRATIO", "1"))


def _interleave(*gens, ratio=None):
    # first generator gets `ratio` bursts per single burst of the others
    r = ratio if ratio is not None else _RATIO
    active = [iter(g) for g in gens]
    while active:
        for i, g in enumerate(list(active)):
            n = r if (i == 0 and len(active) > 1) else 1
            for _ in range(n):
                try:
                    next(g)
                except StopIteration:
                    if g in active:
                        active.remove(g)
                    break


def build_bass(sup):
    """Per-core module. sup = padded unmasked key count (mult of 128)."""
    KT = sup // 128
    nc = bacc.Bacc("TRN2", target_bir_lowering=False, debug=False,
                   num_devices=N_CORES)

    def inp(name, shape, dt=F32):
        return nc.dram_tensor(name, shape, dt, kind="ExternalInput").ap()

    src8_d = inp("src8", [HID, S], FP8)
    src_res_d = inp("src_res", [HID, R])         # fp32 src.T slice + bo
    srcu8_d = inp("srcu8", [HID, sup], FP8)
    wq_d = inp("wq", [2 * 128, 2 * 256], FP8)    # DR-packed SQ*(Wm@Wq).T
    wk_d = inp("wk", [2 * 128, 2 * 256], FP8)
    wv_d = inp("wv", [HID, 256], FP8)
    wo_d = inp("wo", [2 * 128, 2 * 512], FP8)
    w1_d = inp("w1", [2 * 128, 2 * PF], FP8)
    w2_d = inp("w2", [8 * 128, 2 * 512], FP8)
    bq_d = inp("bq", [2, 128])
    bk_d = inp("bk", [2, 128])
    bv_d = inp("bv", [256])
    b1_d = inp("b1", [16, 128])                  # SW*b1
    bt1_d = inp("bt1", [4, 128])                 # ln1_b
    bt1f_d = inp("bt1f", [4, 128])               # ln1_b + b2
    g1_d = inp("g1", [4, 128])
    g2_d = inp("g2", [4, 128])
    bt2_d = inp("bt2", [4, 128])
    mb_d = inp("mb", [KT, 128])
    out_d = nc.dram_tensor("out_t", [HID, R], F32, kind="ExternalOutput").ap()

    from contextlib import ExitStack
    with tile.TileContext(nc) as tc, ExitStack() as ctx:
        con = ctx.enter_context(tc.tile_pool(name="con", bufs=1))
        ppool = ctx.enter_context(tc.tile_pool(name="ps", bufs=2, space="PSUM"))
        pe_e = ctx.enter_context(tc.tile_pool(name="pe", bufs=2, space="PSUM"))
        pe_av = ctx.enter_context(tc.tile_pool(name="pav", bufs=2, space="PSUM"))
        att_pool = ctx.enter_context(tc.tile_pool(name="att", bufs=4))
        xn_pool = ctx.enter_context(tc.tile_pool(name="xn", bufs=2))
        rep_pool = ctx.enter_context(tc.tile_pool(name="rep", bufs=3))
        h1_pool = ctx.enter_context(tc.tile_pool(name="h1", bufs=2))
        tmp_pool = ctx.enter_context(tc.tile_pool(name="tmp", bufs=3))
        z_pool = ctx.enter_context(tc.tile_pool(name="z", bufs=2))
        o_pool = ctx.enter_context(tc.tile_pool(name="o", bufs=4))

        mm = nc.tensor.matmul
        act = nc.scalar.activation
        dve = nc.vector
        gps = nc.gpsimd

        def dma(out, in_):
            nc.sync.dma_start(out=out, in_=in_)

        def ctile(shape, dt, tag):
            return con.tile(shape, dt, tag=tag, name=tag)

        # ---- constants / weights ----
        srcu8 = ctile([128, 4, sup], FP8, "srcu8")
        wq8 = ctile([128, 2, 2, 256], FP8, "wq8")
        wk8 = ctile([128, 2, 2, 256], FP8, "wk8")
        wv8 = ctile([128, 4, 256], FP8, "wv8")
        dma(wk8, wk_d.rearrange("(t p) (i m) -> p t i m", t=2, i=2))
        dma(srcu8, srcu8_d.rearrange("(c p) n -> p c n", p=128))
        dma(wv8, wv_d.rearrange("(c p) m -> p c m", p=128))
        src8 = ctile([128, 4, S], FP8, "src8")
        dma(src8, src8_d.rearrange("(c p) n -> p c n", p=128))
        dma(wq8, wq_d.rearrange("(t p) (i m) -> p t i m", t=2, i=2))
        src_res = [ctile([128, R], F32, f"srcres{i}") for i in range(4)]
        wo8 = ctile([128, 2, 2, 512], FP8, "wo8")
        w18 = ctile([128, 2, 2, PF], FP8, "w18")
        w28 = ctile([128, 8, 2, 512], FP8, "w28")

        def load_chain_weights():
            dma(wo8, wo_d.rearrange("(t p) (i m) -> p t i m", t=2, i=2))
            for i in range(4):
                dma(src_res[i], src_res_d[128 * i:128 * (i + 1), :])
            dma(w18, w1_d.rearrange("(t p) (i m) -> p t i m", t=2, i=2))
            dma(w28, w2_d.rearrange("(t p) (i m) -> p t i m", t=8, i=2))

        def vec_in(dram, n, tag):
            t = ctile([128, n], F32, tag)
            dma(t, dram.rearrange("m p -> p m"))
            return t

        bq_sb = vec_in(bq_d, 2, "bq")
        bk_sb = vec_in(bk_d, 2, "bk")
        b1_sb = vec_in(b1_d, 16, "b1")
        g1_sb = vec_in(g1_d, 4, "g1")
        bt1_sb = vec_in(bt1_d, 4, "bt1")
        bt1f_sb = vec_in(bt1f_d, 4, "bt1f")
        g2_sb = vec_in(g2_d, 4, "g2")
        bt2_sb = vec_in(bt2_d, 4, "bt2")
        mb_sb = vec_in(mb_d, KT, "mb")
        bv_rep = ctile([128, 256], F32, "bvrep")
        dma(bv_rep, _bcast_ap(bv_d, 128))

        ones_bf = ctile([128, 128], BF16, "onesbf")
        dve.memset(ones_bf, 1.0)
        eps_t = ctile([128, 1], F32, "eps")
        dve.memset(eps_t, LN_EPS)

        q_sb = [ctile([128, S], BF16, f"q{m}") for m in range(2)]
        k_sb = [ctile([128, sup], BF16, f"k{m}") for m in range(2)]

        def gen_proj(w8, bias_sb, src_t, n_total, out_tiles, mt):
            n0 = 0
            while n0 < n_total:
                nq = min(512, n_total - n0)
                for half in range(2):
                    ps = ppool.tile([64, 512], F32, tag="ps", name="psp")
                    mcol = 64 * (2 * mt + half)
                    for t in range(2):
                        mm(ps[:, :nq],
                           w8[:, t, :, mcol:mcol + 64],
                           src_t[:, 2 * t:2 * t + 2, n0:n0 + nq],
                           start=(t == 0), stop=(t == 1), perf_mode=DR)
                    dve.tensor_scalar_add(
                        out_tiles[mt][64 * half:64 * half + 64, n0:n0 + nq],
                        ps[:, :nq],
                        bias_sb[64 * half:64 * half + 64, mt:mt + 1])
                    yield
                n0 += nq

        # ---- V natural [keys, 4*128] bf16: 64 V cols + 64 ones cols ----
        # (AV matmul cost is N-proportional, M-free: 64 ones cols give 64
        # identical den rows so recip writes [64, W] directly -- no broadcast)
        v_sb = ctile([128, KT * 4 * 128], BF16, "v")
        v_v = v_sb.rearrange("p (kt h e) -> p kt h e", kt=KT, h=4)

        def gen_vproj():
            # ones cols hold 1/SX so den rows accumulate den/SX and
            # rep = recip(den/SX) = SX/den (xn lands mid-range for fp8)
            dve.memset(v_v[:, :, :, 64:128], 1.0 / SX)
            for kt in range(KT):
                ps = ppool.tile([128, 512], F32, tag="ps", name="psv")
                for ct in range(4):
                    mm(ps[:, :256], srcu8[:, ct, 128 * kt:128 * (kt + 1)],
                       wv8[:, ct, :], start=(ct == 0), stop=(ct == 3))
                dve.scalar_tensor_tensor(
                    out=v_v[:, kt, :, 0:64],
                    in0=ps[:, :256].rearrange("p (h d) -> p h d", h=4),
                    scalar=1.0 / SV,
                    in1=bv_rep.rearrange("p (h d) -> p h d", h=4),
                    op0=OP.mult, op1=OP.add)
                yield

        # ---- attention head h -> fp8 xnp half (scaled by SX) ----
        def gen_attention(h, xnp):
            g = h // 2
            p0 = 64 * (h % 2)
            o0 = S * (h % 2)
            for q0 in range(0, S, 1024):
                avs = []
                for half in range(2):
                    avs.append(pe_av.tile([128, 512], F32, tag="av", name="av"))
                for kt in range(KT):
                    e = pe_e.tile([128, 1024], F32, tag="e", name="e")
                    with tc.high_priority():
                        for half in range(2):
                            mm(e[:, 512 * half:512 * (half + 1)],
                               k_sb[g][p0:p0 + 64, 128 * kt:128 * (kt + 1)],
                               q_sb[g][p0:p0 + 64,
                                       q0 + 512 * half:q0 + 512 * (half + 1)],
                               start=True, stop=True)
                    at = att_pool.tile([128, 1024], BF16, tag="att", name="att")
                    act(at, e, AF.Exp, bias=mb_sb[:, kt:kt + 1],
                        scale=1.0 / (SQ * SK))
                    for half in range(2):
                        mm(avs[half], v_v[:, kt, h, :],
                           at[:, 512 * half:512 * (half + 1)],
                           start=(kt == 0), stop=(kt == KT - 1),
                           skip_group_check=True)
                    yield
                # tail: rep = SX/den via recip of the 4 identical den rows;
                # broadcast row 0 to 64 partitions on Pool; xn = x' * rep.
                for half in range(2):
                    rep = rep_pool.tile([64, 512], BF16, tag="rep", name="rep")
                    with tc.high_priority(), \
                         nc.allow_low_precision(reason="softmax recip"):
                        dve.reciprocal(rep, avs[half][64:128, :])
                    with tc.high_priority():
                        dve.tensor_tensor(
                            out=xnp[0:64, o0 + q0 + 512 * half:o0 + q0 + 512 * (half + 1)],
                            in0=avs[half][0:64, :], in1=rep, op=OP.mult)
                    yield
            with tc.high_priority():
                gps.tensor_copy(out=xnp[64:128, o0:o0 + S // 2],
                                in_=xnp[0:64, o0 + 1:o0 + S // 2 + 1])
                gps.tensor_copy(out=xnp[64:128, o0 + S // 2:o0 + S - 1],
                                in_=xnp[0:64, o0 + S // 2 + 1:o0 + S])
            yield

        # ---- layernorm on 4x[128, W] f32 z-tiles ----
        def gen_layernorm(z_tiles, g_sb, writers, W, stats_pool=None):
            # s1/s2 must sit in separate PSUM banks: a start=True matmul marks
            # its whole 2KB zero-region pending-zero, wiping any sibling
            # accumulation group sharing the bank.  stats_pool lets tail LNs
            # borrow the idle attention av-tag banks so concurrent chain
            # lanes don't cycle-deadlock on the shared chain psum tag.
            sp = stats_pool
            if sp is None:
                s1 = ppool.tile([128, W], F32, tag="ps", name="s1")
                s2 = ppool.tile([128, W], F32, tag="ps", name="s2")
            else:
                s1 = sp.tile([128, W], F32, tag="av", name="s1")
                s2 = sp.tile([128, W], F32, tag="av", name="s2")
            for ct in range(4):
                zb = tmp_pool.tile([128, W], BF16, tag="zb", name="zb")
                gps.tensor_copy(out=zb, in_=z_tiles[ct])
                sq = tmp_pool.tile([128, W], BF16, tag="sq", name="sq")
                dve.tensor_tensor(out=sq, in0=zb, in1=zb, op=OP.mult)
                mm(s1, ones_bf, zb, start=(ct == 0), stop=(ct == 3),
                   skip_group_check=True)
                mm(s2, ones_bf, sq, start=(ct == 0), stop=(ct == 3),
                   skip_group_check=True)
                yield
            bm = tmp_pool.tile([128, W], F32, tag="bm", name="bm")
            br = tmp_pool.tile([128, W], F32, tag="br", name="br")
            m2 = tmp_pool.tile([128, W], BF16, tag="m2", name="m2", bufs=1)
            with tc.high_priority():
                dve.tensor_scalar_mul(bm, s1, 1.0 / HID)
                dve.tensor_tensor(out=m2, in0=bm, in1=bm, op=OP.mult)
                dve.scalar_tensor_tensor(out=br, in0=s2,
                                         scalar=1.0 / HID, in1=m2,
                                         op0=OP.mult, op1=OP.subtract)
            with tc.high_priority():
                act(br, br, AF.Ln, bias=eps_t)
                act(br, br, AF.Exp, scale=-0.5)
            yield
            for ct in range(4):
                sub = tmp_pool.tile([128, W], F32, tag="sub", name="sub")
                gps.tensor_tensor(out=sub, in0=z_tiles[ct], in1=bm,
                                  op=OP.subtract)
                t2 = tmp_pool.tile([128, W], F32, tag="t2", name="t2")
                dve.scalar_tensor_tensor(out=t2, in0=sub,
                                         scalar=g_sb[:, ct:ct + 1], in1=br,
                                         op0=OP.mult, op1=OP.mult)
                writers(ct, t2)
                yield

        # ---- chain A for one 256-token block: Wo(DR) + res, LN1 ----
        src1_f = [[con.tile([128, 256], F32, tag=f"s1f{i}_{j}",
                            name=f"s1f{i}_{j}") for j in range(4)]
                  for i in range(4)]
        src1_8 = ctile([128, 4, R], FP8, "src1_8")

        def gen_chain_a(blk, xnp):
            c0 = 256 * blk
            hh = blk % 2
            xw = xnp.rearrange("p (hh m j) -> p j hh m", hh=2, j=8)
            z1 = [z_pool.tile([128, 256], F32, tag=f"z{mt}", name=f"z{mt}")
                  for mt in range(4)]
            for mt in range(4):
                for half in range(2):
                    ps = ppool.tile([64, 512], F32, tag="ps", name="pswo")
                    mcol = 64 * (2 * mt + half)
                    for t in range(2):
                        ifm = bass.AP(
                            tensor=xw.tensor,
                            offset=xw.offset + (4 * t) * xw.ap[1][0]
                            + hh * xw.ap[2][0],
                            ap=[xw.ap[0], [2 * xw.ap[1][0], 2], xw.ap[3]])
                        mm(ps[:, :256], wo8[:, t, :, mcol:mcol + 64], ifm,
                           start=(t == 0), stop=(t == 1), perf_mode=DR)
                    with tc.high_priority():
                        dve.scalar_tensor_tensor(
                            out=z1[mt][64 * half:64 * half + 64, :],
                            in0=ps[:, :256],
                            scalar=1.0 / (SX * SW),
                            in1=src_res[mt][64 * half:64 * half + 64, c0:c0 + 256],
                            op0=OP.mult, op1=OP.add)
                    yield

            def w1(ct, t2):
                gps.tensor_scalar_add(src1_f[ct][blk], t2,
                                      bt1f_sb[:, ct:ct + 1])
                gps.tensor_scalar_add(src1_8[:, ct, c0:c0 + 256], t2,
                                      bt1_sb[:, ct:ct + 1])

            yield from gen_layernorm(z1, g1_sb, w1, 256)

        # ---- FFN1 for a 512-token pair (blocks 2p, 2p+1) ----
        h18s = {}

        def gen_ffn1(p):
            c0 = 512 * p
            h18 = h1_pool.tile([128, 16, 512], FP8, tag="h1", name="h1", bufs=2)
            h18s[p] = h18
            for mt in range(16):
                for half in range(2):
                    ps = ppool.tile([64, 512], F32, tag="ps", name="psf1")
                    mcol = 64 * (2 * mt + half)
                    for t in range(2):
                        mm(ps, w18[:, t, :, mcol:mcol + 64],
                           src1_8[:, 2 * t:2 * t + 2, c0:c0 + 512],
                           start=(t == 0), stop=(t == 1), perf_mode=DR)
                    with tc.high_priority():
                        if half == 0:
                            act(h18[0:64, mt, :], ps, AF.Relu,
                                bias=b1_sb[0:64, mt:mt + 1], scale=1.0)
                        else:
                            dve.tensor_scalar(
                                out=h18[64:128, mt, :], in0=ps,
                                scalar1=b1_sb[64:128, mt:mt + 1], scalar2=0.0,
                                op0=OP.add, op1=OP.max)
                    yield

        # ---- chain B for one 256-token block: FFN2(DR) + res, LN2, out ----
        z2s = {}

        def gen_chain_b_ffn(blk):
            c0 = 256 * blk
            h18 = h18s[blk // 2]
            r0 = 256 * (blk % 2)
            z2 = [z_pool.tile([128, 256], F32, tag=f"z{ot}", name=f"z{ot}")
                  for ot in range(4)]
            z2s[blk] = z2
            for ot in range(4):
                for half in range(2):
                    ps = ppool.tile([64, 512], F32, tag="ps", name="psf2")
                    mcol = 64 * (2 * ot + half)
                    for t in range(8):
                        mm(ps[:, :256], w28[:, t, :, mcol:mcol + 64],
                           h18[:, 2 * t:2 * t + 2, r0:r0 + 256],
                           start=(t == 0), stop=(t == 7), perf_mode=DR)
                    with tc.high_priority():
                        dve.scalar_tensor_tensor(
                            out=z2[ot][64 * half:64 * half + 64, :],
                            in0=ps[:, :256],
                            scalar=1.0 / (SW * SW),
                            in1=src1_f[ot][blk][64 * half:64 * half + 64, :],
                            op0=OP.mult, op1=OP.add)
                    yield

        def gen_chain_b_ln(blk, stats_pool=None):
            c0 = 256 * blk
            z2 = z2s[blk]

            def w2(ct, t2):
                o = o_pool.tile([128, 256], F32, tag="out", name="out", bufs=2)
                gps.tensor_scalar_add(o, t2, bt2_sb[:, ct:ct + 1])
                dma(out_d[128 * ct:128 * (ct + 1), c0:c0 + 256], o)

            yield from gen_layernorm(z2, g2_sb, w2, 256, stats_pool=stats_pool)

        def gen_chain_b(blk, stats_pool=None):
            yield from gen_chain_b_ffn(blk)
            yield from gen_chain_b_ln(blk, stats_pool=stats_pool)

        def gen_seq(*gens):
            for g in gens:
                yield from g

        # ---- schedule ----
        # V' ones cols hold 1/SX so den rows accumulate den/SX and
        # rep4 = recip(den/SX) = SX/den.  (memset inside gen_vproj runs first.)
        xnp0 = xn_pool.tile([128, 2 * S], FP8, tag="xn", name="xn")
        xnp1 = xn_pool.tile([128, 2 * S], FP8, tag="xn", name="xn")

        _interleave(gen_proj(wk8, bk_sb, srcu8, sup, k_sb, 0),
                    gen_proj(wq8, bq_sb, src8, S, q_sb, 0),
                    gen_vproj())
        load_chain_weights()
        _interleave(gen_attention(0, xnp0),
                    gen_seq(gen_proj(wk8, bk_sb, srcu8, sup, k_sb, 1),
                            gen_proj(wq8, bq_sb, src8, S, q_sb, 1)))
        _interleave(gen_attention(1, xnp0), gen_chain_a(0, xnp0))
        _interleave(gen_attention(2, xnp1),
                    gen_seq(gen_chain_a(1, xnp0), gen_ffn1(0)))
        _interleave(gen_attention(3, xnp1),
                    gen_seq(gen_chain_b(0), gen_chain_a(2, xnp1),
                            gen_chain_b(1)))
        _interleave(gen_seq(gen_chain_a(3, xnp1), gen_ffn1(1),
                            gen_chain_b(2), gen_chain_b(3)))

    nc.compile()
    return nc


def _prep_core(c, src, idxs, sup, w):
    b, hg = c // 2, c % 2
    heads = list(range(HPC * hg, HPC * hg + HPC))
    st = np.ascontiguousarray(src[b].T)                       # [512, 2048] f32
    idx = idxs[b]
    su = len(idx)
    srcu = np.zeros((HID, sup), np.float32)
    srcu[:, :su] = st[:, idx]
    wqe = np.concatenate([w["Wm"] @ w["Wq"][64 * h:64 * (h + 1), :] for h in heads])
    bqe = np.concatenate([w["Wm"] @ w["bq"][64 * h:64 * (h + 1)] + w["bm"]
                          for h in heads])
    wks = np.concatenate([w["Wk"][64 * h:64 * (h + 1), :] for h in heads])
    bks = np.concatenate([w["bk"][64 * h:64 * (h + 1)] for h in heads])
    wvs = np.concatenate([w["Wv"][64 * h:64 * (h + 1), :] for h in heads])
    bvs = np.concatenate([w["bv"][64 * h:64 * (h + 1)] for h in heads])
    mb = np.full(sup, NEG_BIG, np.float32)
    mb[:su] = 0.0
    f32 = np.float32
    src_res = np.ascontiguousarray(st[:, R * hg:R * (hg + 1)]) \
        + w["bo"][:, None].astype(f32)
    return {
        "src8": st.astype(fp8np),
        "src_res": src_res.astype(f32),
        "srcu8": srcu.astype(fp8np),
        "wq": _pack_dr(np.ascontiguousarray(wqe.T) * SQ).astype(fp8np),
        "wk": _pack_dr(np.ascontiguousarray(wks.T) * SK).astype(fp8np),
        "wv": (np.ascontiguousarray(wvs.T) * SV).astype(fp8np),
        "wo": _pack_dr(np.ascontiguousarray(w["Wo"].T) * SW).astype(fp8np),
        "w1": _pack_dr(np.ascontiguousarray(w["W1"].T) * SW).astype(fp8np),
        "w2": _pack_dr(np.ascontiguousarray(w["W2"].T) * SW).astype(fp8np),
        "bq": (bqe * SQ).reshape(2, 128).astype(f32),
        "bk": (bks * SK).reshape(2, 128).astype(f32),
        "bv": bvs.astype(f32),
        "b1": (w["b1"] * SW).reshape(16, 128).astype(f32),
        "bt1": w["ln1_b"].reshape(4, 128).astype(f32),
        "bt1f": (w["ln1_b"] + w["b2"]).reshape(4, 128).astype(f32),
        "g1": w["ln1_g"].reshape(4, 128).astype(f32),
        "g2": w["ln2_g"].reshape(4, 128).astype(f32),
        "bt2": w["ln2_b"].reshape(4, 128).astype(f32),
        "mb": mb.reshape(sup // 128, 128),
    }


def kernel(**inputs):
    global last_results
    w = {k: np.asarray(v, np.float32) for k, v in inputs.items()
         if k not in ("src", "src_mask")}
    src = np.asarray(inputs["src"], np.float32)
    mask = np.asarray(inputs["src_mask"]).reshape(B, S)
    idxs = [np.nonzero(mask[b] != 0)[0] for b in range(B)]
    sup = max(128, ((max(len(i) for i in idxs) + 127) // 128) * 128)

    if sup not in _built_cache:
        _built_cache[sup] = build_bass(sup)
    nc = _built_cache[sup]

    in_maps = [_prep_core(c, src, idxs, sup, w) for c in range(N_CORES)]
    res = bass_utils.run_bass_kernel_spmd(nc, in_maps, core_ids=list(range(N_CORES)),
                                          **run_kwargs)
    last_results = res
    out = np.empty((B, S, HID), np.float32)
    for c in range(N_CORES):
        b, hg = c // 2, c % 2
        out[b, R * hg:R * (hg + 1), :] = res.results[c]["out_t"].T
    return out


# revision 61
# speedup vs baseline: 1.0435x; 1.0003x over previous
"""Trainium2 Bass kernel for nn_EncoderLayer (multiplicative-attention encoder layer).

Sharding: 8 cores; core c handles batch b=c//2, head-group hg=c%2 (4 of 8 heads).
The reference's head-major reshape bug maps head h exactly to output rows
[256h, 256h+256), so each core owns 1024 complete output rows -> no collectives.

v3: - big GEMMs (QKV proj, Wo, FFN1, FFN2) in fp8e4 DoubleRow perf mode
      (0.5 PE cycles/out-col, K=256/pass) with host-prepacked weights and
      power-of-2 pre-scales folded into writer ops / the exp scale.
    - software-pipelined emission: chain work for head h's 256 output tokens
      is interleaved (generator round-robin) with head h+1's attention, so
      Act (exp-bound) and PE (GEMM-bound) run concurrently.
    - softmax tail: recip straight off PSUM den rows, Pool partition_broadcast
      replaces the PE broadcast matmul, xn multiply reads PSUM directly.

Per-token chain independence: LN1/FFN/LN2 normalize over features, so the
chain runs on 256-token blocks (one attention head's scrambled rows each).
FFN1 runs per 512-token pair to halve writer-instruction overhead.
"""

import numpy as np
import ml_dtypes

import concourse.bass as bass
import concourse.tile as tile
import concourse.bacc as bacc
from concourse import mybir
from concourse import bass_utils
from concourse import hw_specs as _hw_specs

_real_gat = _hw_specs.get_activation_tables


def _gat_pinned(arch):
    tabs = _real_gat(arch)
    return {name: (fns if name == "natural_log_exp_and_others" else set())
            for name, fns in tabs.items()}


bacc.get_activation_tables = _gat_pinned

B, S, HID, H, PF, D = 4, 2048, 512, 8, 2048, 64
N_CORES = 8
HPC = H // 2          # heads per core (4)
R = HPC * 256         # output rows per core (1024)
F32 = mybir.dt.float32
BF16 = mybir.dt.bfloat16
FP8 = mybir.dt.float8e4
AF = mybir.ActivationFunctionType
OP = mybir.AluOpType
DR = mybir.MatmulPerfMode.DoubleRow
NEG_BIG = -87.0
LN_EPS = 1e-5
SQ = 64.0             # wq scale
SK = 32.0             # wk scale
SV = 32.0             # wv scale
SX = 64.0             # xn scale
SW = 32.0             # wo/w1/w2 scale
fp8np = ml_dtypes.float8_e4m3

_built_cache = {}
last_results = None
run_kwargs = {}


def _bcast_ap(ap_1d, parts):
    return bass.AP(tensor=ap_1d.tensor, offset=ap_1d.offset,
                   ap=[[0, parts], *ap_1d.ap])


def _pack_dr(wT):
    """[K, M] (K mult of 256) -> DR-packed [K//256 * 128, 2 * M] host layout."""
    K, M = wT.shape
    return np.ascontiguousarray(
        wT.reshape(K // 256, 2, 128, M).transpose(0, 2, 1, 3)
    ).reshape(K // 2, 2 * M)


import os as _os
_RATIO = int(_os.environ.get("KRATIO", "1"))


def _interleave(*gens, ratio=None):
    # first generator gets `ratio` bursts per single burst of the others
    r = ratio if ratio is not None else _RATIO
    active = [iter(g) for g in gens]
    while active:
        for i, g in enumerate(list(active)):
            n = r if (i == 0 and len(active) > 1) else 1
            for _ in range(n):
                try:
                    next(g)
                except StopIteration:
                    if g in active:
                        active.remove(g)
                    break


def build_bass(sup):
    """Per-core module. sup = padded unmasked key count (mult of 128)."""
    KT = sup // 128
    nc = bacc.Bacc("TRN2", target_bir_lowering=False, debug=False,
                   num_devices=N_CORES)

    def inp(name, shape, dt=F32):
        return nc.dram_tensor(name, shape, dt, kind="ExternalInput").ap()

    src8_d = inp("src8", [HID, S], FP8)
    src_res_d = inp("src_res", [HID, R])         # fp32 src.T slice + bo
    srcu8_d = inp("srcu8", [HID, sup], FP8)
    wq_d = inp("wq", [2 * 128, 2 * 256], FP8)    # DR-packed SQ*(Wm@Wq).T
    wk_d = inp("wk", [2 * 128, 2 * 256], FP8)
    wv_d = inp("wv", [HID, 256], FP8)
    wo_d = inp("wo", [2 * 128, 2 * 512], FP8)
    w1_d = inp("w1", [2 * 128, 2 * PF], FP8)
    w2_d = inp("w2", [8 * 128, 2 * 512], FP8)
    bq_d = inp("bq", [2, 128])
    bk_d = inp("bk", [2, 128])
    bv_d = inp("bv", [256])
    b1_d = inp("b1", [16, 128])                  # SW*b1
    bt1_d = inp("bt1", [4, 128])                 # ln1_b
    bt1f_d = inp("bt1f", [4, 128])               # ln1_b + b2
    g1_d = inp("g1", [4, 128])
    g2_d = inp("g2", [4, 128])
    bt2_d = inp("bt2", [4, 128])
    mb_d = inp("mb", [KT, 128])
    out_d = nc.dram_tensor("out_t", [HID, R], F32, kind="ExternalOutput").ap()

    from contextlib import ExitStack
    with tile.TileContext(nc) as tc, ExitStack() as ctx:
        con = ctx.enter_context(tc.tile_pool(name="con", bufs=1))
        ppool = ctx.enter_context(tc.tile_pool(name="ps", bufs=2, space="PSUM"))
        pe_e = ctx.enter_context(tc.tile_pool(name="pe", bufs=2, space="PSUM"))
        pe_av = ctx.enter_context(tc.tile_pool(name="pav", bufs=2, space="PSUM"))
        att_pool = ctx.enter_context(tc.tile_pool(name="att", bufs=4))
        xn_pool = ctx.enter_context(tc.tile_pool(name="xn", bufs=2))
        rep_pool = ctx.enter_context(tc.tile_pool(name="rep", bufs=3))
        h1_pool = ctx.enter_context(tc.tile_pool(name="h1", bufs=2))
        tmp_pool = ctx.enter_context(tc.tile_pool(name="tmp", bufs=3))
        z_pool = ctx.enter_context(tc.tile_pool(name="z", bufs=2))
        o_pool = ctx.enter_context(tc.tile_pool(name="o", bufs=4))

        mm = nc.tensor.matmul
        act = nc.scalar.activation
        dve = nc.vector
        gps = nc.gpsimd

        def dma(out, in_):
            nc.sync.dma_start(out=out, in_=in_)

        def ctile(shape, dt, tag):
            return con.tile(shape, dt, tag=tag, name=tag)

        # ---- constants / weights ----
        srcu8 = ctile([128, 4, sup], FP8, "srcu8")
        wq8 = ctile([128, 2, 2, 256], FP8, "wq8")
        wk8 = ctile([128, 2, 2, 256], FP8, "wk8")
        wv8 = ctile([128, 4, 256], FP8, "wv8")
        dma(wk8, wk_d.rearrange("(t p) (i m) -> p t i m", t=2, i=2))
        dma(srcu8, srcu8_d.rearrange("(c p) n -> p c n", p=128))
        dma(wv8, wv_d.rearrange("(c p) m -> p c m", p=128))
        src8 = ctile([128, 4, S], FP8, "src8")
        dma(src8, src8_d.rearrange("(c p) n -> p c n", p=128))
        dma(wq8, wq_d.rearrange("(t p) (i m) -> p t i m", t=2, i=2))
        src_res = [ctile([128, R], F32, f"srcres{i}") for i in range(4)]
        wo8 = ctile([128, 2, 2, 512], FP8, "wo8")
        w18 = ctile([128, 2, 2, PF], FP8, "w18")
        w28 = ctile([128, 8, 2, 512], FP8, "w28")

        def load_chain_weights():
            dma(wo8, wo_d.rearrange("(t p) (i m) -> p t i m", t=2, i=2))
            for i in range(4):
                dma(src_res[i], src_res_d[128 * i:128 * (i + 1), :])
            dma(w18, w1_d.rearrange("(t p) (i m) -> p t i m", t=2, i=2))
            dma(w28, w2_d.rearrange("(t p) (i m) -> p t i m", t=8, i=2))

        def vec_in(dram, n, tag):
            t = ctile([128, n], F32, tag)
            dma(t, dram.rearrange("m p -> p m"))
            return t

        bq_sb = vec_in(bq_d, 2, "bq")
        bk_sb = vec_in(bk_d, 2, "bk")
        b1_sb = vec_in(b1_d, 16, "b1")
        g1_sb = vec_in(g1_d, 4, "g1")
        bt1_sb = vec_in(bt1_d, 4, "bt1")
        bt1f_sb = vec_in(bt1f_d, 4, "bt1f")
        g2_sb = vec_in(g2_d, 4, "g2")
        bt2_sb = vec_in(bt2_d, 4, "bt2")
        mb_sb = vec_in(mb_d, KT, "mb")
        bv_rep = ctile([128, 256], F32, "bvrep")
        dma(bv_rep, _bcast_ap(bv_d, 128))

        ones_bf = ctile([128, 128], BF16, "onesbf")
        dve.memset(ones_bf, 1.0)
        eps_t = ctile([128, 1], F32, "eps")
        dve.memset(eps_t, LN_EPS)

        q_sb = [ctile([128, S], BF16, f"q{m}") for m in range(2)]
        k_sb = [ctile([128, sup], BF16, f"k{m}") for m in range(2)]

        def gen_proj(w8, bias_sb, src_t, n_total, out_tiles, mt):
            n0 = 0
            while n0 < n_total:
                nq = min(512, n_total - n0)
                for half in range(2):
                    ps = ppool.tile([64, 512], F32, tag="ps", name="psp")
                    mcol = 64 * (2 * mt + half)
                    for t in range(2):
                        mm(ps[:, :nq],
                           w8[:, t, :, mcol:mcol + 64],
                           src_t[:, 2 * t:2 * t + 2, n0:n0 + nq],
                           start=(t == 0), stop=(t == 1), perf_mode=DR)
                    dve.tensor_scalar_add(
                        out_tiles[mt][64 * half:64 * half + 64, n0:n0 + nq],
                        ps[:, :nq],
                        bias_sb[64 * half:64 * half + 64, mt:mt + 1])
                    yield
                n0 += nq

        # ---- V natural [keys, 4*128] bf16: 64 V cols + 64 ones cols ----
        # (AV matmul cost is N-proportional, M-free: 64 ones cols give 64
        # identical den rows so recip writes [64, W] directly -- no broadcast)
        v_sb = ctile([128, KT * 4 * 128], BF16, "v")
        v_v = v_sb.rearrange("p (kt h e) -> p kt h e", kt=KT, h=4)

        def gen_vproj():
            # ones cols hold 1/SX so den rows accumulate den/SX and
            # rep = recip(den/SX) = SX/den (xn lands mid-range for fp8)
            dve.memset(v_v[:, :, :, 64:128], 1.0 / SX)
            for kt in range(KT):
                ps = ppool.tile([128, 512], F32, tag="ps", name="psv")
                for ct in range(4):
                    mm(ps[:, :256], srcu8[:, ct, 128 * kt:128 * (kt + 1)],
                       wv8[:, ct, :], start=(ct == 0), stop=(ct == 3))
                dve.scalar_tensor_tensor(
                    out=v_v[:, kt, :, 0:64],
                    in0=ps[:, :256].rearrange("p (h d) -> p h d", h=4),
                    scalar=1.0 / SV,
                    in1=bv_rep.rearrange("p (h d) -> p h d", h=4),
                    op0=OP.mult, op1=OP.add)
                yield

        # ---- attention head h -> fp8 xnp half (scaled by SX) ----
        def gen_attention(h, xnp):
            g = h // 2
            p0 = 64 * (h % 2)
            o0 = S * (h % 2)
            for q0 in range(0, S, 1024):
                avs = []
                for half in range(2):
                    avs.append(pe_av.tile([128, 512], F32, tag="av", name="av"))
                for kt in range(KT):
                    e = pe_e.tile([128, 1024], F32, tag="e", name="e")
                    with tc.high_priority():
                        for half in range(2):
                            mm(e[:, 512 * half:512 * (half + 1)],
                               k_sb[g][p0:p0 + 64, 128 * kt:128 * (kt + 1)],
                               q_sb[g][p0:p0 + 64,
                                       q0 + 512 * half:q0 + 512 * (half + 1)],
                               start=True, stop=True)
                    at = att_pool.tile([128, 1024], BF16, tag="att", name="att")
                    act(at, e, AF.Exp, bias=mb_sb[:, kt:kt + 1],
                        scale=1.0 / (SQ * SK))
                    for half in range(2):
                        mm(avs[half], v_v[:, kt, h, :],
                           at[:, 512 * half:512 * (half + 1)],
                           start=(kt == 0), stop=(kt == KT - 1),
                           skip_group_check=True)
                    yield
                # tail: rep = SX/den via recip of the 4 identical den rows;
                # broadcast row 0 to 64 partitions on Pool; xn = x' * rep.
                for half in range(2):
                    rep = rep_pool.tile([64, 512], BF16, tag="rep", name="rep")
                    with tc.high_priority(), \
                         nc.allow_low_precision(reason="softmax recip"):
                        dve.reciprocal(rep, avs[half][64:128, :])
                    with tc.high_priority():
                        dve.tensor_tensor(
                            out=xnp[0:64, o0 + q0 + 512 * half:o0 + q0 + 512 * (half + 1)],
                            in0=avs[half][0:64, :], in1=rep, op=OP.mult)
                    yield
            with tc.high_priority():
                gps.tensor_copy(out=xnp[64:128, o0:o0 + S // 2],
                                in_=xnp[0:64, o0 + 1:o0 + S // 2 + 1])
                gps.tensor_copy(out=xnp[64:128, o0 + S // 2:o0 + S - 1],
                                in_=xnp[0:64, o0 + S // 2 + 1:o0 + S])
            yield

        # ---- layernorm on 4x[128, W] f32 z-tiles ----
        def gen_layernorm(z_tiles, g_sb, writers, W, stats_pool=None):
            # s1/s2 must sit in separate PSUM banks: a start=True matmul marks
            # its whole 2KB zero-region pending-zero, wiping any sibling
            # accumulation group sharing the bank.  stats_pool lets tail LNs
            # borrow the idle attention av-tag banks so concurrent chain
            # lanes don't cycle-deadlock on the shared chain psum tag.
            sp = stats_pool
            if sp is None:
                s1 = ppool.tile([128, W], F32, tag="ps", name="s1")
                s2 = ppool.tile([128, W], F32, tag="ps", name="s2")
            else:
                s1 = sp.tile([128, W], F32, tag="av", name="s1")
                s2 = sp.tile([128, W], F32, tag="av", name="s2")
            for ct in range(4):
                zb = tmp_pool.tile([128, W], BF16, tag="zb", name="zb")
                gps.tensor_copy(out=zb, in_=z_tiles[ct])
                sq = tmp_pool.tile([128, W], BF16, tag="sq", name="sq")
                dve.tensor_tensor(out=sq, in0=zb, in1=zb, op=OP.mult)
                mm(s1, ones_bf, zb, start=(ct == 0), stop=(ct == 3),
                   skip_group_check=True)
                mm(s2, ones_bf, sq, start=(ct == 0), stop=(ct == 3),
                   skip_group_check=True)
                yield
            bm = tmp_pool.tile([128, W], F32, tag="bm", name="bm")
            br = tmp_pool.tile([128, W], F32, tag="br", name="br")
            m2 = tmp_pool.tile([128, W], BF16, tag="m2", name="m2", bufs=1)
            with tc.high_priority():
                dve.tensor_scalar_mul(bm, s1, 1.0 / HID)
                dve.tensor_tensor(out=m2, in0=bm, in1=bm, op=OP.mult)
                dve.scalar_tensor_tensor(out=br, in0=s2,
                                         scalar=1.0 / HID, in1=m2,
                                         op0=OP.mult, op1=OP.subtract)
            with tc.high_priority():
                act(br, br, AF.Ln, bias=eps_t)
                act(br, br, AF.Exp, scale=-0.5)
            yield
            for ct in range(4):
                sub = tmp_pool.tile([128, W], F32, tag="sub", name="sub")
                gps.tensor_tensor(out=sub, in0=z_tiles[ct], in1=bm,
                                  op=OP.subtract)
                t2 = tmp_pool.tile([128, W], F32, tag="t2", name="t2")
                dve.scalar_tensor_tensor(out=t2, in0=sub,
                                         scalar=g_sb[:, ct:ct + 1], in1=br,
                                         op0=OP.mult, op1=OP.mult)
                writers(ct, t2)
                yield

        # ---- chain A for one 256-token block: Wo(DR) + res, LN1 ----
        src1_f = [[con.tile([128, 256], F32, tag=f"s1f{i}_{j}",
                            name=f"s1f{i}_{j}") for j in range(4)]
                  for i in range(4)]
        src1_8 = ctile([128, 4, R], FP8, "src1_8")

        def gen_chain_a(blk, xnp, stats_pool=None):
            c0 = 256 * blk
            hh = blk % 2
            xw = xnp.rearrange("p (hh m j) -> p j hh m", hh=2, j=8)
            z1 = [z_pool.tile([128, 256], F32, tag=f"z{mt}", name=f"z{mt}")
                  for mt in range(4)]
            for mt in range(4):
                for half in range(2):
                    ps = ppool.tile([64, 512], F32, tag="ps", name="pswo")
                    mcol = 64 * (2 * mt + half)
                    for t in range(2):
                        ifm = bass.AP(
                            tensor=xw.tensor,
                            offset=xw.offset + (4 * t) * xw.ap[1][0]
                            + hh * xw.ap[2][0],
                            ap=[xw.ap[0], [2 * xw.ap[1][0], 2], xw.ap[3]])
                        mm(ps[:, :256], wo8[:, t, :, mcol:mcol + 64], ifm,
                           start=(t == 0), stop=(t == 1), perf_mode=DR)
                    with tc.high_priority():
                        dve.scalar_tensor_tensor(
                            out=z1[mt][64 * half:64 * half + 64, :],
                            in0=ps[:, :256],
                            scalar=1.0 / (SX * SW),
                            in1=src_res[mt][64 * half:64 * half + 64, c0:c0 + 256],
                            op0=OP.mult, op1=OP.add)
                    yield

            def w1(ct, t2):
                gps.tensor_scalar_add(src1_f[ct][blk], t2,
                                      bt1f_sb[:, ct:ct + 1])
                gps.tensor_scalar_add(src1_8[:, ct, c0:c0 + 256], t2,
                                      bt1_sb[:, ct:ct + 1])

            yield from gen_layernorm(z1, g1_sb, w1, 256, stats_pool=stats_pool)

        # ---- FFN1 for a 512-token pair (blocks 2p, 2p+1) ----
        h18s = {}

        def gen_ffn1(p):
            c0 = 512 * p
            h18 = h1_pool.tile([128, 16, 512], FP8, tag="h1", name="h1", bufs=2)
            h18s[p] = h18
            for mt in range(16):
                for half in range(2):
                    ps = ppool.tile([64, 512], F32, tag="ps", name="psf1")
                    mcol = 64 * (2 * mt + half)
                    for t in range(2):
                        mm(ps, w18[:, t, :, mcol:mcol + 64],
                           src1_8[:, 2 * t:2 * t + 2, c0:c0 + 512],
                           start=(t == 0), stop=(t == 1), perf_mode=DR)
                    with tc.high_priority():
                        if half == 0:
                            act(h18[0:64, mt, :], ps, AF.Relu,
                                bias=b1_sb[0:64, mt:mt + 1], scale=1.0)
                        else:
                            dve.tensor_scalar(
                                out=h18[64:128, mt, :], in0=ps,
                                scalar1=b1_sb[64:128, mt:mt + 1], scalar2=0.0,
                                op0=OP.add, op1=OP.max)
                    yield

        # ---- chain B for one 256-token block: FFN2(DR) + res, LN2, out ----
        z2s = {}

        def gen_chain_b_ffn(blk):
            c0 = 256 * blk
            h18 = h18s[blk // 2]
            r0 = 256 * (blk % 2)
            z2 = [z_pool.tile([128, 256], F32, tag=f"z{ot}", name=f"z{ot}")
                  for ot in range(4)]
            z2s[blk] = z2
            for ot in range(4):
                for half in range(2):
                    ps = ppool.tile([64, 512], F32, tag="ps", name="psf2")
                    mcol = 64 * (2 * ot + half)
                    for t in range(8):
                        mm(ps[:, :256], w28[:, t, :, mcol:mcol + 64],
                           h18[:, 2 * t:2 * t + 2, r0:r0 + 256],
                           start=(t == 0), stop=(t == 7), perf_mode=DR)
                    with tc.high_priority():
                        dve.scalar_tensor_tensor(
                            out=z2[ot][64 * half:64 * half + 64, :],
                            in0=ps[:, :256],
                            scalar=1.0 / (SW * SW),
                            in1=src1_f[ot][blk][64 * half:64 * half + 64, :],
                            op0=OP.mult, op1=OP.add)
                    yield

        def gen_chain_b_ln(blk, stats_pool=None):
            c0 = 256 * blk
            z2 = z2s[blk]

            def w2(ct, t2):
                o = o_pool.tile([128, 256], F32, tag="out", name="out", bufs=2)
                gps.tensor_scalar_add(o, t2, bt2_sb[:, ct:ct + 1])
                dma(out_d[128 * ct:128 * (ct + 1), c0:c0 + 256], o)

            yield from gen_layernorm(z2, g2_sb, w2, 256, stats_pool=stats_pool)

        def gen_chain_b(blk, stats_pool=None):
            yield from gen_chain_b_ffn(blk)
            yield from gen_chain_b_ln(blk, stats_pool=stats_pool)

        def gen_seq(*gens):
            for g in gens:
                yield from g

        # ---- schedule ----
        # V' ones cols hold 1/SX so den rows accumulate den/SX and
        # rep4 = recip(den/SX) = SX/den.  (memset inside gen_vproj runs first.)
        xnp0 = xn_pool.tile([128, 2 * S], FP8, tag="xn", name="xn")
        xnp1 = xn_pool.tile([128, 2 * S], FP8, tag="xn", name="xn")

        _interleave(gen_proj(wk8, bk_sb, srcu8, sup, k_sb, 0),
                    gen_proj(wq8, bq_sb, src8, S, q_sb, 0),
                    gen_vproj())
        load_chain_weights()
        _interleave(gen_attention(0, xnp0),
                    gen_seq(gen_proj(wk8, bk_sb, srcu8, sup, k_sb, 1),
                            gen_proj(wq8, bq_sb, src8, S, q_sb, 1)))
        _interleave(gen_attention(1, xnp0), gen_chain_a(0, xnp0))
        _interleave(gen_attention(2, xnp1),
                    gen_seq(gen_chain_a(1, xnp0), gen_ffn1(0)))
        _interleave(gen_attention(3, xnp1),
                    gen_seq(gen_chain_b(0), gen_chain_a(2, xnp1),
                            gen_chain_b(1)))
        _interleave(gen_seq(gen_chain_a(3, xnp1), gen_ffn1(1),
                            gen_chain_b_ffn(2)))
        _interleave(gen_seq(gen_chain_b_ln(2, stats_pool=pe_av),
                            gen_chain_b_ln(3)),
                    gen_chain_b_ffn(3))

    nc.compile()
    return nc


def _prep_core(c, src, idxs, sup, w):
    b, hg = c // 2, c % 2
    heads = list(range(HPC * hg, HPC * hg + HPC))
    st = np.ascontiguousarray(src[b].T)                       # [512, 2048] f32
    idx = idxs[b]
    su = len(idx)
    srcu = np.zeros((HID, sup), np.float32)
    srcu[:, :su] = st[:, idx]
    wqe = np.concatenate([w["Wm"] @ w["Wq"][64 * h:64 * (h + 1), :] for h in heads])
    bqe = np.concatenate([w["Wm"] @ w["bq"][64 * h:64 * (h + 1)] + w["bm"]
                          for h in heads])
    wks = np.concatenate([w["Wk"][64 * h:64 * (h + 1), :] for h in heads])
    bks = np.concatenate([w["bk"][64 * h:64 * (h + 1)] for h in heads])
    wvs = np.concatenate([w["Wv"][64 * h:64 * (h + 1), :] for h in heads])
    bvs = np.concatenate([w["bv"][64 * h:64 * (h + 1)] for h in heads])
    mb = np.full(sup, NEG_BIG, np.float32)
    mb[:su] = 0.0
    f32 = np.float32
    src_res = np.ascontiguousarray(st[:, R * hg:R * (hg + 1)]) \
        + w["bo"][:, None].astype(f32)
    return {
        "src8": st.astype(fp8np),
        "src_res": src_res.astype(f32),
        "srcu8": srcu.astype(fp8np),
        "wq": _pack_dr(np.ascontiguousarray(wqe.T) * SQ).astype(fp8np),
        "wk": _pack_dr(np.ascontiguousarray(wks.T) * SK).astype(fp8np),
        "wv": (np.ascontiguousarray(wvs.T) * SV).astype(fp8np),
        "wo": _pack_dr(np.ascontiguousarray(w["Wo"].T) * SW).astype(fp8np),
        "w1": _pack_dr(np.ascontiguousarray(w["W1"].T) * SW).astype(fp8np),
        "w2": _pack_dr(np.ascontiguousarray(w["W2"].T) * SW).astype(fp8np),
        "bq": (bqe * SQ).reshape(2, 128).astype(f32),
        "bk": (bks * SK).reshape(2, 128).astype(f32),
        "bv": bvs.astype(f32),
        "b1": (w["b1"] * SW).reshape(16, 128).astype(f32),
        "bt1": w["ln1_b"].reshape(4, 128).astype(f32),
        "bt1f": (w["ln1_b"] + w["b2"]).reshape(4, 128).astype(f32),
        "g1": w["ln1_g"].reshape(4, 128).astype(f32),
        "g2": w["ln2_g"].reshape(4, 128).astype(f32),
        "bt2": w["ln2_b"].reshape(4, 128).astype(f32),
        "mb": mb.reshape(sup // 128, 128),
    }


def kernel(**inputs):
    global last_results
    w = {k: np.asarray(v, np.float32) for k, v in inputs.items()
         if k not in ("src", "src_mask")}
    src = np.asarray(inputs["src"], np.float32)
    mask = np.asarray(inputs["src_mask"]).reshape(B, S)
    idxs = [np.nonzero(mask[b] != 0)[0] for b in range(B)]
    sup = max(128, ((max(len(i) for i in idxs) + 127) // 128) * 128)

    if sup not in _built_cache:
        _built_cache[sup] = build_bass(sup)
    nc = _built_cache[sup]

    in_maps = [_prep_core(c, src, idxs, sup, w) for c in range(N_CORES)]
    res = bass_utils.run_bass_kernel_spmd(nc, in_maps, core_ids=list(range(N_CORES)),
                                          **run_kwargs)
    last_results = res
    out = np.empty((B, S, HID), np.float32)
    for c in range(N_CORES):
        b, hg = c // 2, c % 2
        out[b, R * hg:R * (hg + 1), :] = res.results[c]["out_t"].T
    return out


# revision 76
# speedup vs baseline: 1.1078x; 1.0616x over previous
"""Trainium2 Bass kernel for nn_EncoderLayer (multiplicative-attention encoder layer).

Sharding: 8 cores; core c handles batch b=c//2, head-group hg=c%2 (4 of 8 heads).
The reference's head-major reshape bug maps head h exactly to output rows
[256h, 256h+256), so each core owns 1024 complete output rows -> no collectives.

v3: - big GEMMs (QKV proj, Wo, FFN1, FFN2) in fp8e4 DoubleRow perf mode
      (0.5 PE cycles/out-col, K=256/pass) with host-prepacked weights and
      power-of-2 pre-scales folded into writer ops / the exp scale.
    - software-pipelined emission: chain work for head h's 256 output tokens
      is interleaved (generator round-robin) with head h+1's attention, so
      Act (exp-bound) and PE (GEMM-bound) run concurrently.
    - softmax tail: recip straight off PSUM den rows, Pool partition_broadcast
      replaces the PE broadcast matmul, xn multiply reads PSUM directly.

Per-token chain independence: LN1/FFN/LN2 normalize over features, so the
chain runs on 256-token blocks (one attention head's scrambled rows each).
FFN1 runs per 512-token pair to halve writer-instruction overhead.
"""

import numpy as np
import ml_dtypes

import concourse.bass as bass
import concourse.tile as tile
import concourse.bacc as bacc
from concourse import mybir
from concourse import bass_utils
from concourse import hw_specs as _hw_specs

_real_gat = _hw_specs.get_activation_tables


def _gat_pinned(arch):
    tabs = _real_gat(arch)
    return {name: (fns if name == "natural_log_exp_and_others" else set())
            for name, fns in tabs.items()}


bacc.get_activation_tables = _gat_pinned

B, S, HID, H, PF, D = 4, 2048, 512, 8, 2048, 64
N_CORES = 8
HPC = H // 2          # heads per core (4)
R = HPC * 256         # output rows per core (1024)
F32 = mybir.dt.float32
BF16 = mybir.dt.bfloat16
FP8 = mybir.dt.float8e4
AF = mybir.ActivationFunctionType
OP = mybir.AluOpType
DR = mybir.MatmulPerfMode.DoubleRow
NEG_BIG = -87.0
LN_EPS = 1e-5
SQ = 64.0             # wq scale
SK = 32.0             # wk scale
SV = 32.0             # wv scale
SX = 64.0             # xn scale
SW = 32.0             # wo/w1/w2 scale
fp8np = ml_dtypes.float8_e4m3

_built_cache = {}
last_results = None
run_kwargs = {}


def _bcast_ap(ap_1d, parts):
    return bass.AP(tensor=ap_1d.tensor, offset=ap_1d.offset,
                   ap=[[0, parts], *ap_1d.ap])


def _pack_dr(wT):
    """[K, M] (K mult of 256) -> DR-packed [K//256 * 128, 2 * M] host layout."""
    K, M = wT.shape
    return np.ascontiguousarray(
        wT.reshape(K // 256, 2, 128, M).transpose(0, 2, 1, 3)
    ).reshape(K // 2, 2 * M)


import os as _os
_RATIO = int(_os.environ.get("KRATIO", "1"))


def _interleave(*gens, ratio=None):
    # first generator gets `ratio` bursts per single burst of the others
    r = ratio if ratio is not None else _RATIO
    active = [iter(g) for g in gens]
    while active:
        for i, g in enumerate(list(active)):
            n = r if (i == 0 and len(active) > 1) else 1
            for _ in range(n):
                try:
                    next(g)
                except StopIteration:
                    if g in active:
                        active.remove(g)
                    break


def build_bass(sup):
    """Per-core module. sup = padded unmasked key count (mult of 128)."""
    KT = sup // 128
    nc = bacc.Bacc("TRN2", target_bir_lowering=False, debug=False,
                   num_devices=N_CORES)

    def inp(name, shape, dt=F32):
        return nc.dram_tensor(name, shape, dt, kind="ExternalInput").ap()

    src8_d = inp("src8", [HID, S], FP8)
    src_res_d = inp("src_res", [HID, R])         # fp32 src.T slice + bo
    srcu8_d = inp("srcu8", [HID, sup], FP8)
    wq_d = inp("wq", [2 * 128, 2 * 256], FP8)    # DR-packed SQ*(Wm@Wq).T
    wk_d = inp("wk", [2 * 128, 2 * 256], FP8)
    wv_d = inp("wv", [HID, 256], FP8)
    wo_d = inp("wo", [2 * 128, 2 * 512], FP8)
    w1_d = inp("w1", [2 * 128, 2 * PF], FP8)
    w2_d = inp("w2", [8 * 128, 2 * 512], FP8)
    bq_d = inp("bq", [2, 128])
    bk_d = inp("bk", [2, 128])
    bv_d = inp("bv", [256])
    b1_d = inp("b1", [16, 128])                  # SW*b1
    bt1_d = inp("bt1", [4, 128])                 # ln1_b
    bt1f_d = inp("bt1f", [4, 128])               # ln1_b + b2
    g1_d = inp("g1", [4, 128])
    g2_d = inp("g2", [4, 128])
    bt2_d = inp("bt2", [4, 128])
    mb_d = inp("mb", [KT, 128])
    out_d = nc.dram_tensor("out_t", [HID, R], F32, kind="ExternalOutput").ap()

    from contextlib import ExitStack
    with tile.TileContext(nc) as tc, ExitStack() as ctx:
        con = ctx.enter_context(tc.tile_pool(name="con", bufs=1))
        ppool = ctx.enter_context(tc.tile_pool(name="ps", bufs=2, space="PSUM"))
        pe_e = ctx.enter_context(tc.tile_pool(name="pe", bufs=2, space="PSUM"))
        pe_av = ctx.enter_context(tc.tile_pool(name="pav", bufs=2, space="PSUM"))
        att_pool = ctx.enter_context(tc.tile_pool(name="att", bufs=6))
        xn_pool = ctx.enter_context(tc.tile_pool(name="xn", bufs=2))
        rep_pool = ctx.enter_context(tc.tile_pool(name="rep", bufs=3))
        h1_pool = ctx.enter_context(tc.tile_pool(name="h1", bufs=2))
        tmp_pool = ctx.enter_context(tc.tile_pool(name="tmp", bufs=4))
        z_pool = ctx.enter_context(tc.tile_pool(name="z", bufs=2))
        o_pool = ctx.enter_context(tc.tile_pool(name="o", bufs=6))

        mm = nc.tensor.matmul
        act = nc.scalar.activation
        dve = nc.vector
        gps = nc.gpsimd

        def dma(out, in_):
            nc.sync.dma_start(out=out, in_=in_)

        def ctile(shape, dt, tag):
            return con.tile(shape, dt, tag=tag, name=tag)

        # ---- constants / weights ----
        srcu8 = ctile([128, 4, sup], FP8, "srcu8")
        wq8 = ctile([128, 2, 2, 256], FP8, "wq8")
        wk8 = ctile([128, 2, 2, 256], FP8, "wk8")
        wv8 = ctile([128, 4, 256], FP8, "wv8")
        dma(wk8, wk_d.rearrange("(t p) (i m) -> p t i m", t=2, i=2))
        dma(srcu8, srcu8_d.rearrange("(c p) n -> p c n", p=128))
        dma(wv8, wv_d.rearrange("(c p) m -> p c m", p=128))
        src8 = ctile([128, 4, S], FP8, "src8")
        dma(src8, src8_d.rearrange("(c p) n -> p c n", p=128))
        dma(wq8, wq_d.rearrange("(t p) (i m) -> p t i m", t=2, i=2))
        src_res = [ctile([128, R], F32, f"srcres{i}") for i in range(4)]
        wo8 = ctile([128, 2, 2, 512], FP8, "wo8")
        w18 = ctile([128, 2, 2, PF], FP8, "w18")
        w28 = ctile([128, 8, 2, 512], FP8, "w28")

        def load_chain_weights():
            dma(wo8, wo_d.rearrange("(t p) (i m) -> p t i m", t=2, i=2))
            for i in range(4):
                dma(src_res[i], src_res_d[128 * i:128 * (i + 1), :])
            dma(w18, w1_d.rearrange("(t p) (i m) -> p t i m", t=2, i=2))
            dma(w28, w2_d.rearrange("(t p) (i m) -> p t i m", t=8, i=2))

        def vec_in(dram, n, tag):
            t = ctile([128, n], F32, tag)
            dma(t, dram.rearrange("m p -> p m"))
            return t

        bq_sb = vec_in(bq_d, 2, "bq")
        bk_sb = vec_in(bk_d, 2, "bk")
        b1_sb = vec_in(b1_d, 16, "b1")
        g1_sb = vec_in(g1_d, 4, "g1")
        bt1_sb = vec_in(bt1_d, 4, "bt1")
        bt1f_sb = vec_in(bt1f_d, 4, "bt1f")
        g2_sb = vec_in(g2_d, 4, "g2")
        bt2_sb = vec_in(bt2_d, 4, "bt2")
        mb_sb = vec_in(mb_d, KT, "mb")
        bv_rep = ctile([128, 256], F32, "bvrep")
        dma(bv_rep, _bcast_ap(bv_d, 128))

        ones_bf = ctile([128, 128], BF16, "onesbf")
        dve.memset(ones_bf, 1.0)
        eps_t = ctile([128, 1], F32, "eps")
        dve.memset(eps_t, LN_EPS)

        q_sb = [ctile([128, S], BF16, f"q{m}") for m in range(2)]
        k_sb = [ctile([128, sup], BF16, f"k{m}") for m in range(2)]

        def gen_proj(w8, bias_sb, src_t, n_total, out_tiles, mt):
            n0 = 0
            while n0 < n_total:
                nq = min(512, n_total - n0)
                for half in range(2):
                    ps = ppool.tile([64, 512], F32, tag="ps", name="psp")
                    mcol = 64 * (2 * mt + half)
                    for t in range(2):
                        mm(ps[:, :nq],
                           w8[:, t, :, mcol:mcol + 64],
                           src_t[:, 2 * t:2 * t + 2, n0:n0 + nq],
                           start=(t == 0), stop=(t == 1), perf_mode=DR)
                    dve.tensor_scalar_add(
                        out_tiles[mt][64 * half:64 * half + 64, n0:n0 + nq],
                        ps[:, :nq],
                        bias_sb[64 * half:64 * half + 64, mt:mt + 1])
                    yield
                n0 += nq

        # ---- V natural [keys, 4*128] bf16: 64 V cols + 64 ones cols ----
        # (AV matmul cost is N-proportional, M-free: 64 ones cols give 64
        # identical den rows so recip writes [64, W] directly -- no broadcast)
        v_sb = ctile([128, KT * 4 * 128], BF16, "v")
        v_v = v_sb.rearrange("p (kt h e) -> p kt h e", kt=KT, h=4)

        def gen_vproj():
            # ones cols hold 1/SX so den rows accumulate den/SX and
            # rep = recip(den/SX) = SX/den (xn lands mid-range for fp8)
            dve.memset(v_v[:, :, :, 64:128], 1.0 / SX)
            for kt in range(KT):
                ps = ppool.tile([128, 512], F32, tag="ps", name="psv")
                for ct in range(4):
                    mm(ps[:, :256], srcu8[:, ct, 128 * kt:128 * (kt + 1)],
                       wv8[:, ct, :], start=(ct == 0), stop=(ct == 3))
                dve.scalar_tensor_tensor(
                    out=v_v[:, kt, :, 0:64],
                    in0=ps[:, :256].rearrange("p (h d) -> p h d", h=4),
                    scalar=1.0 / SV,
                    in1=bv_rep.rearrange("p (h d) -> p h d", h=4),
                    op0=OP.mult, op1=OP.add)
                yield

        # ---- attention head h -> fp8 xnp half (scaled by SX) ----
        def gen_attention(h, xnp):
            g = h // 2
            p0 = 64 * (h % 2)
            o0 = S * (h % 2)
            for q0 in range(0, S, 1024):
                avs = []
                for half in range(2):
                    avs.append(pe_av.tile([128, 512], F32, tag="av", name="av"))
                for kt in range(KT):
                    e = pe_e.tile([128, 1024], F32, tag="e", name="e")
                    with tc.high_priority():
                        for half in range(2):
                            mm(e[:, 512 * half:512 * (half + 1)],
                               k_sb[g][p0:p0 + 64, 128 * kt:128 * (kt + 1)],
                               q_sb[g][p0:p0 + 64,
                                       q0 + 512 * half:q0 + 512 * (half + 1)],
                               start=True, stop=True)
                    at = att_pool.tile([128, 1024], BF16, tag="att", name="att")
                    act(at, e, AF.Exp, bias=mb_sb[:, kt:kt + 1],
                        scale=1.0 / (SQ * SK))
                    for half in range(2):
                        mm(avs[half], v_v[:, kt, h, :],
                           at[:, 512 * half:512 * (half + 1)],
                           start=(kt == 0), stop=(kt == KT - 1),
                           skip_group_check=True)
                    yield
                # tail: rep = SX/den via recip of the 4 identical den rows;
                # broadcast row 0 to 64 partitions on Pool; xn = x' * rep.
                for half in range(2):
                    rep = rep_pool.tile([64, 512], BF16, tag="rep", name="rep")
                    with tc.high_priority(), \
                         nc.allow_low_precision(reason="softmax recip"):
                        dve.reciprocal(rep, avs[half][64:128, :])
                    with tc.high_priority():
                        dve.tensor_tensor(
                            out=xnp[0:64, o0 + q0 + 512 * half:o0 + q0 + 512 * (half + 1)],
                            in0=avs[half][0:64, :], in1=rep, op=OP.mult)
                    yield
            with tc.high_priority():
                gps.tensor_copy(out=xnp[64:128, o0:o0 + S // 2],
                                in_=xnp[0:64, o0 + 1:o0 + S // 2 + 1])
                gps.tensor_copy(out=xnp[64:128, o0 + S // 2:o0 + S - 1],
                                in_=xnp[0:64, o0 + S // 2 + 1:o0 + S])
            yield

        # ---- layernorm on 4x[128, W] f32 z-tiles ----
        def gen_layernorm(z_tiles, g_sb, writers, W, stats_pool=None):
            # s1/s2 must sit in separate PSUM banks: a start=True matmul marks
            # its whole 2KB zero-region pending-zero, wiping any sibling
            # accumulation group sharing the bank.  stats_pool lets tail LNs
            # borrow the idle attention av-tag banks so concurrent chain
            # lanes don't cycle-deadlock on the shared chain psum tag.
            sp = stats_pool
            if sp is None:
                s1 = ppool.tile([128, W], F32, tag="ps", name="s1")
                s2 = ppool.tile([128, W], F32, tag="ps", name="s2")
            else:
                s1 = sp.tile([128, W], F32, tag="av", name="s1")
                s2 = sp.tile([128, W], F32, tag="av", name="s2")
            for ct in range(4):
                zb = tmp_pool.tile([128, W], BF16, tag="zb", name="zb")
                gps.tensor_copy(out=zb, in_=z_tiles[ct])
                sq = tmp_pool.tile([128, W], BF16, tag="sq", name="sq")
                dve.tensor_tensor(out=sq, in0=zb, in1=zb, op=OP.mult)
                mm(s1, ones_bf, zb, start=(ct == 0), stop=(ct == 3),
                   skip_group_check=True)
                mm(s2, ones_bf, sq, start=(ct == 0), stop=(ct == 3),
                   skip_group_check=True)
                yield
            bm = tmp_pool.tile([128, W], F32, tag="bm", name="bm")
            br = tmp_pool.tile([128, W], F32, tag="br", name="br")
            m2 = tmp_pool.tile([128, W], BF16, tag="m2", name="m2", bufs=1)
            with tc.high_priority():
                dve.tensor_scalar_mul(bm, s1, 1.0 / HID)
                dve.tensor_tensor(out=m2, in0=bm, in1=bm, op=OP.mult)
                dve.scalar_tensor_tensor(out=br, in0=s2,
                                         scalar=1.0 / HID, in1=m2,
                                         op0=OP.mult, op1=OP.subtract)
            with tc.high_priority():
                act(br, br, AF.Ln, bias=eps_t)
                act(br, br, AF.Exp, scale=-0.5)
            yield
            for ct in range(4):
                sub = tmp_pool.tile([128, W], F32, tag="sub", name="sub")
                gps.tensor_tensor(out=sub, in0=z_tiles[ct], in1=bm,
                                  op=OP.subtract)
                t2 = tmp_pool.tile([128, W], F32, tag="t2", name="t2")
                dve.scalar_tensor_tensor(out=t2, in0=sub,
                                         scalar=g_sb[:, ct:ct + 1], in1=br,
                                         op0=OP.mult, op1=OP.mult)
                writers(ct, t2)
                yield

        # ---- chain A for one 256-token block: Wo(DR) + res, LN1 ----
        src1_f = [[con.tile([128, 256], F32, tag=f"s1f{i}_{j}",
                            name=f"s1f{i}_{j}") for j in range(4)]
                  for i in range(4)]
        src1_8 = ctile([128, 4, R], FP8, "src1_8")

        def gen_chain_a(blk, xnp, stats_pool=None):
            c0 = 256 * blk
            hh = blk % 2
            xw = xnp.rearrange("p (hh m j) -> p j hh m", hh=2, j=8)
            z1 = [z_pool.tile([128, 256], F32, tag=f"z{mt}", name=f"z{mt}")
                  for mt in range(4)]
            for mt in range(4):
                for half in range(2):
                    ps = ppool.tile([64, 512], F32, tag="ps", name="pswo")
                    mcol = 64 * (2 * mt + half)
                    for t in range(2):
                        ifm = bass.AP(
                            tensor=xw.tensor,
                            offset=xw.offset + (4 * t) * xw.ap[1][0]
                            + hh * xw.ap[2][0],
                            ap=[xw.ap[0], [2 * xw.ap[1][0], 2], xw.ap[3]])
                        mm(ps[:, :256], wo8[:, t, :, mcol:mcol + 64], ifm,
                           start=(t == 0), stop=(t == 1), perf_mode=DR)
                    with tc.high_priority():
                        dve.scalar_tensor_tensor(
                            out=z1[mt][64 * half:64 * half + 64, :],
                            in0=ps[:, :256],
                            scalar=1.0 / (SX * SW),
                            in1=src_res[mt][64 * half:64 * half + 64, c0:c0 + 256],
                            op0=OP.mult, op1=OP.add)
                    yield

            def w1(ct, t2):
                gps.tensor_scalar_add(src1_f[ct][blk], t2,
                                      bt1f_sb[:, ct:ct + 1])
                gps.tensor_scalar_add(src1_8[:, ct, c0:c0 + 256], t2,
                                      bt1_sb[:, ct:ct + 1])

            yield from gen_layernorm(z1, g1_sb, w1, 256, stats_pool=stats_pool)

        # ---- FFN1 for a 512-token pair (blocks 2p, 2p+1) ----
        h18s = {}

        def gen_ffn1(p):
            c0 = 512 * p
            h18 = h1_pool.tile([128, 16, 512], FP8, tag="h1", name="h1", bufs=2)
            h18s[p] = h18
            for mt in range(16):
                for half in range(2):
                    ps = ppool.tile([64, 512], F32, tag="ps", name="psf1")
                    mcol = 64 * (2 * mt + half)
                    for t in range(2):
                        mm(ps, w18[:, t, :, mcol:mcol + 64],
                           src1_8[:, 2 * t:2 * t + 2, c0:c0 + 512],
                           start=(t == 0), stop=(t == 1), perf_mode=DR)
                    with tc.high_priority():
                        if half == 0:
                            act(h18[0:64, mt, :], ps, AF.Relu,
                                bias=b1_sb[0:64, mt:mt + 1], scale=1.0)
                        else:
                            dve.tensor_scalar(
                                out=h18[64:128, mt, :], in0=ps,
                                scalar1=b1_sb[64:128, mt:mt + 1], scalar2=0.0,
                                op0=OP.add, op1=OP.max)
                    yield

        # ---- chain B for one 256-token block: FFN2(DR) + res, LN2, out ----
        z2s = {}

        def gen_chain_b_ffn(blk):
            c0 = 256 * blk
            h18 = h18s[blk // 2]
            r0 = 256 * (blk % 2)
            z2 = [z_pool.tile([128, 256], F32, tag=f"z{ot}", name=f"z{ot}")
                  for ot in range(4)]
            z2s[blk] = z2
            for ot in range(4):
                for half in range(2):
                    ps = ppool.tile([64, 512], F32, tag="ps", name="psf2")
                    mcol = 64 * (2 * ot + half)
                    for t in range(8):
                        mm(ps[:, :256], w28[:, t, :, mcol:mcol + 64],
                           h18[:, 2 * t:2 * t + 2, r0:r0 + 256],
                           start=(t == 0), stop=(t == 7), perf_mode=DR)
                    with tc.high_priority():
                        dve.scalar_tensor_tensor(
                            out=z2[ot][64 * half:64 * half + 64, :],
                            in0=ps[:, :256],
                            scalar=1.0 / (SW * SW),
                            in1=src1_f[ot][blk][64 * half:64 * half + 64, :],
                            op0=OP.mult, op1=OP.add)
                    yield

        def gen_chain_b_ln(blk, stats_pool=None):
            c0 = 256 * blk
            z2 = z2s[blk]

            def w2(ct, t2):
                o = o_pool.tile([128, 256], F32, tag="out", name="out", bufs=6)
                gps.tensor_scalar_add(o, t2, bt2_sb[:, ct:ct + 1])
                dma(out_d[128 * ct:128 * (ct + 1), c0:c0 + 256], o)

            yield from gen_layernorm(z2, g2_sb, w2, 256, stats_pool=stats_pool)

        def gen_chain_b(blk, stats_pool=None):
            yield from gen_chain_b_ffn(blk)
            yield from gen_chain_b_ln(blk, stats_pool=stats_pool)

        def gen_seq(*gens):
            for g in gens:
                yield from g

        # ---- schedule ----
        # V' ones cols hold 1/SX so den rows accumulate den/SX and
        # rep4 = recip(den/SX) = SX/den.  (memset inside gen_vproj runs first.)
        xnp0 = xn_pool.tile([128, 2 * S], FP8, tag="xn", name="xn")
        xnp1 = xn_pool.tile([128, 2 * S], FP8, tag="xn", name="xn")

        _interleave(gen_proj(wk8, bk_sb, srcu8, sup, k_sb, 0),
                    gen_proj(wq8, bq_sb, src8, S, q_sb, 0),
                    gen_vproj())
        load_chain_weights()
        _interleave(gen_attention(0, xnp0),
                    gen_seq(gen_proj(wk8, bk_sb, srcu8, sup, k_sb, 1),
                            gen_proj(wq8, bq_sb, src8, S, q_sb, 1)))
        _interleave(gen_attention(1, xnp0), gen_chain_a(0, xnp0))
        _interleave(gen_attention(2, xnp1),
                    gen_seq(gen_chain_a(1, xnp0), gen_ffn1(0)))
        _interleave(gen_attention(3, xnp1),
                    gen_seq(gen_chain_b(0), gen_chain_a(2, xnp1),
                            gen_chain_b(1)))
        _interleave(gen_seq(gen_chain_a(3, xnp1), gen_ffn1(1),
                            gen_chain_b_ffn(2)))
        _interleave(gen_seq(gen_chain_b_ln(2, stats_pool=pe_av),
                            gen_chain_b_ln(3)),
                    gen_chain_b_ffn(3))

    nc.compile()
    return nc


def _prep_core(c, src, idxs, sup, w):
    b, hg = c // 2, c % 2
    heads = list(range(HPC * hg, HPC * hg + HPC))
    st = np.ascontiguousarray(src[b].T)                       # [512, 2048] f32
    idx = idxs[b]
    su = len(idx)
    srcu = np.zeros((HID, sup), np.float32)
    srcu[:, :su] = st[:, idx]
    wqe = np.concatenate([w["Wm"] @ w["Wq"][64 * h:64 * (h + 1), :] for h in heads])
    bqe = np.concatenate([w["Wm"] @ w["bq"][64 * h:64 * (h + 1)] + w["bm"]
                          for h in heads])
    wks = np.concatenate([w["Wk"][64 * h:64 * (h + 1), :] for h in heads])
    bks = np.concatenate([w["bk"][64 * h:64 * (h + 1)] for h in heads])
    wvs = np.concatenate([w["Wv"][64 * h:64 * (h + 1), :] for h in heads])
    bvs = np.concatenate([w["bv"][64 * h:64 * (h + 1)] for h in heads])
    mb = np.full(sup, NEG_BIG, np.float32)
    mb[:su] = 0.0
    f32 = np.float32
    src_res = np.ascontiguousarray(st[:, R * hg:R * (hg + 1)]) \
        + w["bo"][:, None].astype(f32)
    return {
        "src8": st.astype(fp8np),
        "src_res": src_res.astype(f32),
        "srcu8": srcu.astype(fp8np),
        "wq": _pack_dr(np.ascontiguousarray(wqe.T) * SQ).astype(fp8np),
        "wk": _pack_dr(np.ascontiguousarray(wks.T) * SK).astype(fp8np),
        "wv": (np.ascontiguousarray(wvs.T) * SV).astype(fp8np),
        "wo": _pack_dr(np.ascontiguousarray(w["Wo"].T) * SW).astype(fp8np),
        "w1": _pack_dr(np.ascontiguousarray(w["W1"].T) * SW).astype(fp8np),
        "w2": _pack_dr(np.ascontiguousarray(w["W2"].T) * SW).astype(fp8np),
        "bq": (bqe * SQ).reshape(2, 128).astype(f32),
        "bk": (bks * SK).reshape(2, 128).astype(f32),
        "bv": bvs.astype(f32),
        "b1": (w["b1"] * SW).reshape(16, 128).astype(f32),
        "bt1": w["ln1_b"].reshape(4, 128).astype(f32),
        "bt1f": (w["ln1_b"] + w["b2"]).reshape(4, 128).astype(f32),
        "g1": w["ln1_g"].reshape(4, 128).astype(f32),
        "g2": w["ln2_g"].reshape(4, 128).astype(f32),
        "bt2": w["ln2_b"].reshape(4, 128).astype(f32),
        "mb": mb.reshape(sup // 128, 128),
    }


def kernel(**inputs):
    global last_results
    w = {k: np.asarray(v, np.float32) for k, v in inputs.items()
         if k not in ("src", "src_mask")}
    src = np.asarray(inputs["src"], np.float32)
    mask = np.asarray(inputs["src_mask"]).reshape(B, S)
    idxs = [np.nonzero(mask[b] != 0)[0] for b in range(B)]
    sup = max(128, ((max(len(i) for i in idxs) + 127) // 128) * 128)

    if sup not in _built_cache:
        _built_cache[sup] = build_bass(sup)
    nc = _built_cache[sup]

    in_maps = [_prep_core(c, src, idxs, sup, w) for c in range(N_CORES)]
    res = bass_utils.run_bass_kernel_spmd(nc, in_maps, core_ids=list(range(N_CORES)),
                                          **run_kwargs)
    last_results = res
    out = np.empty((B, S, HID), np.float32)
    for c in range(N_CORES):
        b, hg = c // 2, c % 2
        out[b, R * hg:R * (hg + 1), :] = res.results[c]["out_t"].T
    return out


# revision 78
# speedup vs baseline: 1.1079x; 1.0001x over previous
"""Trainium2 Bass kernel for nn_EncoderLayer (multiplicative-attention encoder layer).

Sharding: 8 cores; core c handles batch b=c//2, head-group hg=c%2 (4 of 8 heads).
The reference's head-major reshape bug maps head h exactly to output rows
[256h, 256h+256), so each core owns 1024 complete output rows -> no collectives.

v3: - big GEMMs (QKV proj, Wo, FFN1, FFN2) in fp8e4 DoubleRow perf mode
      (0.5 PE cycles/out-col, K=256/pass) with host-prepacked weights and
      power-of-2 pre-scales folded into writer ops / the exp scale.
    - software-pipelined emission: chain work for head h's 256 output tokens
      is interleaved (generator round-robin) with head h+1's attention, so
      Act (exp-bound) and PE (GEMM-bound) run concurrently.
    - softmax tail: recip straight off PSUM den rows, Pool partition_broadcast
      replaces the PE broadcast matmul, xn multiply reads PSUM directly.

Per-token chain independence: LN1/FFN/LN2 normalize over features, so the
chain runs on 256-token blocks (one attention head's scrambled rows each).
FFN1 runs per 512-token pair to halve writer-instruction overhead.
"""

import numpy as np
import ml_dtypes

import concourse.bass as bass
import concourse.tile as tile
import concourse.bacc as bacc
from concourse import mybir
from concourse import bass_utils
from concourse import hw_specs as _hw_specs

_real_gat = _hw_specs.get_activation_tables


def _gat_pinned(arch):
    tabs = _real_gat(arch)
    return {name: (fns if name == "natural_log_exp_and_others" else set())
            for name, fns in tabs.items()}


bacc.get_activation_tables = _gat_pinned

B, S, HID, H, PF, D = 4, 2048, 512, 8, 2048, 64
N_CORES = 8
HPC = H // 2          # heads per core (4)
R = HPC * 256         # output rows per core (1024)
F32 = mybir.dt.float32
BF16 = mybir.dt.bfloat16
FP8 = mybir.dt.float8e4
AF = mybir.ActivationFunctionType
OP = mybir.AluOpType
DR = mybir.MatmulPerfMode.DoubleRow
NEG_BIG = -87.0
LN_EPS = 1e-5
SQ = 64.0             # wq scale
SK = 32.0             # wk scale
SV = 32.0             # wv scale
SX = 64.0             # xn scale
SW = 32.0             # wo/w1/w2 scale
fp8np = ml_dtypes.float8_e4m3

_built_cache = {}
last_results = None
run_kwargs = {}


def _bcast_ap(ap_1d, parts):
    return bass.AP(tensor=ap_1d.tensor, offset=ap_1d.offset,
                   ap=[[0, parts], *ap_1d.ap])


def _pack_dr(wT):
    """[K, M] (K mult of 256) -> DR-packed [K//256 * 128, 2 * M] host layout."""
    K, M = wT.shape
    return np.ascontiguousarray(
        wT.reshape(K // 256, 2, 128, M).transpose(0, 2, 1, 3)
    ).reshape(K // 2, 2 * M)


import os as _os
_RATIO = int(_os.environ.get("KRATIO", "1"))


def _interleave(*gens, ratio=None):
    # first generator gets `ratio` bursts per single burst of the others
    r = ratio if ratio is not None else _RATIO
    active = [iter(g) for g in gens]
    while active:
        for i, g in enumerate(list(active)):
            n = r if (i == 0 and len(active) > 1) else 1
            for _ in range(n):
                try:
                    next(g)
                except StopIteration:
                    if g in active:
                        active.remove(g)
                    break


def build_bass(sup):
    """Per-core module. sup = padded unmasked key count (mult of 128)."""
    KT = sup // 128
    nc = bacc.Bacc("TRN2", target_bir_lowering=False, debug=False,
                   num_devices=N_CORES)

    def inp(name, shape, dt=F32):
        return nc.dram_tensor(name, shape, dt, kind="ExternalInput").ap()

    src8_d = inp("src8", [HID, S], FP8)
    src_res_d = inp("src_res", [HID, R])         # fp32 src.T slice + bo
    srcu8_d = inp("srcu8", [HID, sup], FP8)
    wq_d = inp("wq", [2 * 128, 2 * 256], FP8)    # DR-packed SQ*(Wm@Wq).T
    wk_d = inp("wk", [2 * 128, 2 * 256], FP8)
    wv_d = inp("wv", [HID, 256], FP8)
    wo_d = inp("wo", [2 * 128, 2 * 512], FP8)
    w1_d = inp("w1", [2 * 128, 2 * PF], FP8)
    w2_d = inp("w2", [8 * 128, 2 * 512], FP8)
    bq_d = inp("bq", [2, 128])
    bk_d = inp("bk", [2, 128])
    bv_d = inp("bv", [256])
    b1_d = inp("b1", [16, 128])                  # SW*b1
    bt1_d = inp("bt1", [4, 128])                 # ln1_b
    bt1f_d = inp("bt1f", [4, 128])               # ln1_b + b2
    g1_d = inp("g1", [4, 128])
    g2_d = inp("g2", [4, 128])
    bt2_d = inp("bt2", [4, 128])
    mb_d = inp("mb", [KT, 128])
    out_d = nc.dram_tensor("out_t", [HID, R], F32, kind="ExternalOutput").ap()

    from contextlib import ExitStack
    with tile.TileContext(nc) as tc, ExitStack() as ctx:
        con = ctx.enter_context(tc.tile_pool(name="con", bufs=1))
        ppool = ctx.enter_context(tc.tile_pool(name="ps", bufs=2, space="PSUM"))
        pe_e = ctx.enter_context(tc.tile_pool(name="pe", bufs=2, space="PSUM"))
        pe_av = ctx.enter_context(tc.tile_pool(name="pav", bufs=2, space="PSUM"))
        att_pool = ctx.enter_context(tc.tile_pool(name="att", bufs=6))
        xn_pool = ctx.enter_context(tc.tile_pool(name="xn", bufs=2))
        rep_pool = ctx.enter_context(tc.tile_pool(name="rep", bufs=4))
        h1_pool = ctx.enter_context(tc.tile_pool(name="h1", bufs=2))
        tmp_pool = ctx.enter_context(tc.tile_pool(name="tmp", bufs=5))
        z_pool = ctx.enter_context(tc.tile_pool(name="z", bufs=2))
        o_pool = ctx.enter_context(tc.tile_pool(name="o", bufs=6))

        mm = nc.tensor.matmul
        act = nc.scalar.activation
        dve = nc.vector
        gps = nc.gpsimd

        def dma(out, in_):
            nc.sync.dma_start(out=out, in_=in_)

        def ctile(shape, dt, tag):
            return con.tile(shape, dt, tag=tag, name=tag)

        # ---- constants / weights ----
        srcu8 = ctile([128, 4, sup], FP8, "srcu8")
        wq8 = ctile([128, 2, 2, 256], FP8, "wq8")
        wk8 = ctile([128, 2, 2, 256], FP8, "wk8")
        wv8 = ctile([128, 4, 256], FP8, "wv8")
        dma(wk8, wk_d.rearrange("(t p) (i m) -> p t i m", t=2, i=2))
        dma(srcu8, srcu8_d.rearrange("(c p) n -> p c n", p=128))
        dma(wv8, wv_d.rearrange("(c p) m -> p c m", p=128))
        src8 = ctile([128, 4, S], FP8, "src8")
        dma(src8, src8_d.rearrange("(c p) n -> p c n", p=128))
        dma(wq8, wq_d.rearrange("(t p) (i m) -> p t i m", t=2, i=2))
        src_res = [ctile([128, R], F32, f"srcres{i}") for i in range(4)]
        wo8 = ctile([128, 2, 2, 512], FP8, "wo8")
        w18 = ctile([128, 2, 2, PF], FP8, "w18")
        w28 = ctile([128, 8, 2, 512], FP8, "w28")

        def load_chain_weights():
            dma(wo8, wo_d.rearrange("(t p) (i m) -> p t i m", t=2, i=2))
            for i in range(4):
                dma(src_res[i], src_res_d[128 * i:128 * (i + 1), :])
            dma(w18, w1_d.rearrange("(t p) (i m) -> p t i m", t=2, i=2))
            dma(w28, w2_d.rearrange("(t p) (i m) -> p t i m", t=8, i=2))

        def vec_in(dram, n, tag):
            t = ctile([128, n], F32, tag)
            dma(t, dram.rearrange("m p -> p m"))
            return t

        bq_sb = vec_in(bq_d, 2, "bq")
        bk_sb = vec_in(bk_d, 2, "bk")
        b1_sb = vec_in(b1_d, 16, "b1")
        g1_sb = vec_in(g1_d, 4, "g1")
        bt1_sb = vec_in(bt1_d, 4, "bt1")
        bt1f_sb = vec_in(bt1f_d, 4, "bt1f")
        g2_sb = vec_in(g2_d, 4, "g2")
        bt2_sb = vec_in(bt2_d, 4, "bt2")
        mb_sb = vec_in(mb_d, KT, "mb")
        bv_rep = ctile([128, 256], F32, "bvrep")
        dma(bv_rep, _bcast_ap(bv_d, 128))

        ones_bf = ctile([128, 128], BF16, "onesbf")
        dve.memset(ones_bf, 1.0)
        eps_t = ctile([128, 1], F32, "eps")
        dve.memset(eps_t, LN_EPS)

        q_sb = [ctile([128, S], BF16, f"q{m}") for m in range(2)]
        k_sb = [ctile([128, sup], BF16, f"k{m}") for m in range(2)]

        def gen_proj(w8, bias_sb, src_t, n_total, out_tiles, mt):
            n0 = 0
            while n0 < n_total:
                nq = min(512, n_total - n0)
                for half in range(2):
                    ps = ppool.tile([64, 512], F32, tag="ps", name="psp")
                    mcol = 64 * (2 * mt + half)
                    for t in range(2):
                        mm(ps[:, :nq],
                           w8[:, t, :, mcol:mcol + 64],
                           src_t[:, 2 * t:2 * t + 2, n0:n0 + nq],
                           start=(t == 0), stop=(t == 1), perf_mode=DR)
                    dve.tensor_scalar_add(
                        out_tiles[mt][64 * half:64 * half + 64, n0:n0 + nq],
                        ps[:, :nq],
                        bias_sb[64 * half:64 * half + 64, mt:mt + 1])
                    yield
                n0 += nq

        # ---- V natural [keys, 4*128] bf16: 64 V cols + 64 ones cols ----
        # (AV matmul cost is N-proportional, M-free: 64 ones cols give 64
        # identical den rows so recip writes [64, W] directly -- no broadcast)
        v_sb = ctile([128, KT * 4 * 128], BF16, "v")
        v_v = v_sb.rearrange("p (kt h e) -> p kt h e", kt=KT, h=4)

        def gen_vproj():
            # ones cols hold 1/SX so den rows accumulate den/SX and
            # rep = recip(den/SX) = SX/den (xn lands mid-range for fp8)
            dve.memset(v_v[:, :, :, 64:128], 1.0 / SX)
            for kt in range(KT):
                ps = ppool.tile([128, 512], F32, tag="ps", name="psv")
                for ct in range(4):
                    mm(ps[:, :256], srcu8[:, ct, 128 * kt:128 * (kt + 1)],
                       wv8[:, ct, :], start=(ct == 0), stop=(ct == 3))
                dve.scalar_tensor_tensor(
                    out=v_v[:, kt, :, 0:64],
                    in0=ps[:, :256].rearrange("p (h d) -> p h d", h=4),
                    scalar=1.0 / SV,
                    in1=bv_rep.rearrange("p (h d) -> p h d", h=4),
                    op0=OP.mult, op1=OP.add)
                yield

        # ---- attention head h -> fp8 xnp half (scaled by SX) ----
        def gen_attention(h, xnp):
            g = h // 2
            p0 = 64 * (h % 2)
            o0 = S * (h % 2)
            for q0 in range(0, S, 1024):
                avs = []
                for half in range(2):
                    avs.append(pe_av.tile([128, 512], F32, tag="av", name="av"))
                for kt in range(KT):
                    e = pe_e.tile([128, 1024], F32, tag="e", name="e")
                    with tc.high_priority():
                        for half in range(2):
                            mm(e[:, 512 * half:512 * (half + 1)],
                               k_sb[g][p0:p0 + 64, 128 * kt:128 * (kt + 1)],
                               q_sb[g][p0:p0 + 64,
                                       q0 + 512 * half:q0 + 512 * (half + 1)],
                               start=True, stop=True)
                    at = att_pool.tile([128, 1024], BF16, tag="att", name="att")
                    act(at, e, AF.Exp, bias=mb_sb[:, kt:kt + 1],
                        scale=1.0 / (SQ * SK))
                    for half in range(2):
                        mm(avs[half], v_v[:, kt, h, :],
                           at[:, 512 * half:512 * (half + 1)],
                           start=(kt == 0), stop=(kt == KT - 1),
                           skip_group_check=True)
                    yield
                # tail: rep = SX/den via recip of the 4 identical den rows;
                # broadcast row 0 to 64 partitions on Pool; xn = x' * rep.
                for half in range(2):
                    rep = rep_pool.tile([64, 512], BF16, tag="rep", name="rep")
                    with tc.high_priority(), \
                         nc.allow_low_precision(reason="softmax recip"):
                        dve.reciprocal(rep, avs[half][64:128, :])
                    with tc.high_priority():
                        dve.tensor_tensor(
                            out=xnp[0:64, o0 + q0 + 512 * half:o0 + q0 + 512 * (half + 1)],
                            in0=avs[half][0:64, :], in1=rep, op=OP.mult)
                    yield
            with tc.high_priority():
                gps.tensor_copy(out=xnp[64:128, o0:o0 + S // 2],
                                in_=xnp[0:64, o0 + 1:o0 + S // 2 + 1])
                gps.tensor_copy(out=xnp[64:128, o0 + S // 2:o0 + S - 1],
                                in_=xnp[0:64, o0 + S // 2 + 1:o0 + S])
            yield

        # ---- layernorm on 4x[128, W] f32 z-tiles ----
        def gen_layernorm(z_tiles, g_sb, writers, W, stats_pool=None):
            # s1/s2 must sit in separate PSUM banks: a start=True matmul marks
            # its whole 2KB zero-region pending-zero, wiping any sibling
            # accumulation group sharing the bank.  stats_pool lets tail LNs
            # borrow the idle attention av-tag banks so concurrent chain
            # lanes don't cycle-deadlock on the shared chain psum tag.
            sp = stats_pool
            if sp is None:
                s1 = ppool.tile([128, W], F32, tag="ps", name="s1")
                s2 = ppool.tile([128, W], F32, tag="ps", name="s2")
            else:
                s1 = sp.tile([128, W], F32, tag="av", name="s1")
                s2 = sp.tile([128, W], F32, tag="av", name="s2")
            for ct in range(4):
                zb = tmp_pool.tile([128, W], BF16, tag="zb", name="zb")
                gps.tensor_copy(out=zb, in_=z_tiles[ct])
                sq = tmp_pool.tile([128, W], BF16, tag="sq", name="sq")
                dve.tensor_tensor(out=sq, in0=zb, in1=zb, op=OP.mult)
                mm(s1, ones_bf, zb, start=(ct == 0), stop=(ct == 3),
                   skip_group_check=True)
                mm(s2, ones_bf, sq, start=(ct == 0), stop=(ct == 3),
                   skip_group_check=True)
                yield
            bm = tmp_pool.tile([128, W], F32, tag="bm", name="bm")
            br = tmp_pool.tile([128, W], F32, tag="br", name="br")
            m2 = tmp_pool.tile([128, W], BF16, tag="m2", name="m2", bufs=1)
            with tc.high_priority():
                dve.tensor_scalar_mul(bm, s1, 1.0 / HID)
                dve.tensor_tensor(out=m2, in0=bm, in1=bm, op=OP.mult)
                dve.scalar_tensor_tensor(out=br, in0=s2,
                                         scalar=1.0 / HID, in1=m2,
                                         op0=OP.mult, op1=OP.subtract)
            with tc.high_priority():
                act(br, br, AF.Ln, bias=eps_t)
                act(br, br, AF.Exp, scale=-0.5)
            yield
            for ct in range(4):
                sub = tmp_pool.tile([128, W], F32, tag="sub", name="sub")
                gps.tensor_tensor(out=sub, in0=z_tiles[ct], in1=bm,
                                  op=OP.subtract)
                t2 = tmp_pool.tile([128, W], F32, tag="t2", name="t2")
                dve.scalar_tensor_tensor(out=t2, in0=sub,
                                         scalar=g_sb[:, ct:ct + 1], in1=br,
                                         op0=OP.mult, op1=OP.mult)
                writers(ct, t2)
                yield

        # ---- chain A for one 256-token block: Wo(DR) + res, LN1 ----
        src1_f = [[con.tile([128, 256], F32, tag=f"s1f{i}_{j}",
                            name=f"s1f{i}_{j}") for j in range(4)]
                  for i in range(4)]
        src1_8 = ctile([128, 4, R], FP8, "src1_8")

        def gen_chain_a(blk, xnp, stats_pool=None):
            c0 = 256 * blk
            hh = blk % 2
            xw = xnp.rearrange("p (hh m j) -> p j hh m", hh=2, j=8)
            z1 = [z_pool.tile([128, 256], F32, tag=f"z{mt}", name=f"z{mt}")
                  for mt in range(4)]
            for mt in range(4):
                for half in range(2):
                    ps = ppool.tile([64, 512], F32, tag="ps", name="pswo")
                    mcol = 64 * (2 * mt + half)
                    for t in range(2):
                        ifm = bass.AP(
                            tensor=xw.tensor,
                            offset=xw.offset + (4 * t) * xw.ap[1][0]
                            + hh * xw.ap[2][0],
                            ap=[xw.ap[0], [2 * xw.ap[1][0], 2], xw.ap[3]])
                        mm(ps[:, :256], wo8[:, t, :, mcol:mcol + 64], ifm,
                           start=(t == 0), stop=(t == 1), perf_mode=DR)
                    with tc.high_priority():
                        dve.scalar_tensor_tensor(
                            out=z1[mt][64 * half:64 * half + 64, :],
                            in0=ps[:, :256],
                            scalar=1.0 / (SX * SW),
                            in1=src_res[mt][64 * half:64 * half + 64, c0:c0 + 256],
                            op0=OP.mult, op1=OP.add)
                    yield

            def w1(ct, t2):
                gps.tensor_scalar_add(src1_f[ct][blk], t2,
                                      bt1f_sb[:, ct:ct + 1])
                gps.tensor_scalar_add(src1_8[:, ct, c0:c0 + 256], t2,
                                      bt1_sb[:, ct:ct + 1])

            yield from gen_layernorm(z1, g1_sb, w1, 256, stats_pool=stats_pool)

        # ---- FFN1 for a 512-token pair (blocks 2p, 2p+1) ----
        h18s = {}

        def gen_ffn1(p):
            c0 = 512 * p
            h18 = h1_pool.tile([128, 16, 512], FP8, tag="h1", name="h1", bufs=2)
            h18s[p] = h18
            for mt in range(16):
                for half in range(2):
                    ps = ppool.tile([64, 512], F32, tag="ps", name="psf1")
                    mcol = 64 * (2 * mt + half)
                    for t in range(2):
                        mm(ps, w18[:, t, :, mcol:mcol + 64],
                           src1_8[:, 2 * t:2 * t + 2, c0:c0 + 512],
                           start=(t == 0), stop=(t == 1), perf_mode=DR)
                    with tc.high_priority():
                        if half == 0:
                            act(h18[0:64, mt, :], ps, AF.Relu,
                                bias=b1_sb[0:64, mt:mt + 1], scale=1.0)
                        else:
                            dve.tensor_scalar(
                                out=h18[64:128, mt, :], in0=ps,
                                scalar1=b1_sb[64:128, mt:mt + 1], scalar2=0.0,
                                op0=OP.add, op1=OP.max)
                    yield

        # ---- chain B for one 256-token block: FFN2(DR) + res, LN2, out ----
        z2s = {}

        def gen_chain_b_ffn(blk):
            c0 = 256 * blk
            h18 = h18s[blk // 2]
            r0 = 256 * (blk % 2)
            z2 = [z_pool.tile([128, 256], F32, tag=f"z{ot}", name=f"z{ot}")
                  for ot in range(4)]
            z2s[blk] = z2
            for ot in range(4):
                for half in range(2):
                    ps = ppool.tile([64, 512], F32, tag="ps", name="psf2")
                    mcol = 64 * (2 * ot + half)
                    for t in range(8):
                        mm(ps[:, :256], w28[:, t, :, mcol:mcol + 64],
                           h18[:, 2 * t:2 * t + 2, r0:r0 + 256],
                           start=(t == 0), stop=(t == 7), perf_mode=DR)
                    with tc.high_priority():
                        dve.scalar_tensor_tensor(
                            out=z2[ot][64 * half:64 * half + 64, :],
                            in0=ps[:, :256],
                            scalar=1.0 / (SW * SW),
                            in1=src1_f[ot][blk][64 * half:64 * half + 64, :],
                            op0=OP.mult, op1=OP.add)
                    yield

        def gen_chain_b_ln(blk, stats_pool=None):
            c0 = 256 * blk
            z2 = z2s[blk]

            def w2(ct, t2):
                o = o_pool.tile([128, 256], F32, tag="out", name="out", bufs=6)
                gps.tensor_scalar_add(o, t2, bt2_sb[:, ct:ct + 1])
                dma(out_d[128 * ct:128 * (ct + 1), c0:c0 + 256], o)

            yield from gen_layernorm(z2, g2_sb, w2, 256, stats_pool=stats_pool)

        def gen_chain_b(blk, stats_pool=None):
            yield from gen_chain_b_ffn(blk)
            yield from gen_chain_b_ln(blk, stats_pool=stats_pool)

        def gen_seq(*gens):
            for g in gens:
                yield from g

        # ---- schedule ----
        # V' ones cols hold 1/SX so den rows accumulate den/SX and
        # rep4 = recip(den/SX) = SX/den.  (memset inside gen_vproj runs first.)
        xnp0 = xn_pool.tile([128, 2 * S], FP8, tag="xn", name="xn")
        xnp1 = xn_pool.tile([128, 2 * S], FP8, tag="xn", name="xn")

        _interleave(gen_proj(wk8, bk_sb, srcu8, sup, k_sb, 0),
                    gen_proj(wq8, bq_sb, src8, S, q_sb, 0),
                    gen_vproj())
        load_chain_weights()
        _interleave(gen_attention(0, xnp0),
                    gen_seq(gen_proj(wk8, bk_sb, srcu8, sup, k_sb, 1),
                            gen_proj(wq8, bq_sb, src8, S, q_sb, 1)))
        _interleave(gen_attention(1, xnp0), gen_chain_a(0, xnp0))
        _interleave(gen_attention(2, xnp1),
                    gen_seq(gen_chain_a(1, xnp0), gen_ffn1(0)))
        _interleave(gen_attention(3, xnp1),
                    gen_seq(gen_chain_b(0), gen_chain_a(2, xnp1),
                            gen_chain_b(1)))
        _interleave(gen_seq(gen_chain_a(3, xnp1), gen_ffn1(1),
                            gen_chain_b_ffn(2)))
        _interleave(gen_seq(gen_chain_b_ln(2, stats_pool=pe_av),
                            gen_chain_b_ln(3)),
                    gen_chain_b_ffn(3))

    nc.compile()
    return nc


def _prep_core(c, src, idxs, sup, w):
    b, hg = c // 2, c % 2
    heads = list(range(HPC * hg, HPC * hg + HPC))
    st = np.ascontiguousarray(src[b].T)                       # [512, 2048] f32
    idx = idxs[b]
    su = len(idx)
    srcu = np.zeros((HID, sup), np.float32)
    srcu[:, :su] = st[:, idx]
    wqe = np.concatenate([w["Wm"] @ w["Wq"][64 * h:64 * (h + 1), :] for h in heads])
    bqe = np.concatenate([w["Wm"] @ w["bq"][64 * h:64 * (h + 1)] + w["bm"]
                          for h in heads])
    wks = np.concatenate([w["Wk"][64 * h:64 * (h + 1), :] for h in heads])
    bks = np.concatenate([w["bk"][64 * h:64 * (h + 1)] for h in heads])
    wvs = np.concatenate([w["Wv"][64 * h:64 * (h + 1), :] for h in heads])
    bvs = np.concatenate([w["bv"][64 * h:64 * (h + 1)] for h in heads])
    mb = np.full(sup, NEG_BIG, np.float32)
    mb[:su] = 0.0
    f32 = np.float32
    src_res = np.ascontiguousarray(st[:, R * hg:R * (hg + 1)]) \
        + w["bo"][:, None].astype(f32)
    return {
        "src8": st.astype(fp8np),
        "src_res": src_res.astype(f32),
        "srcu8": srcu.astype(fp8np),
        "wq": _pack_dr(np.ascontiguousarray(wqe.T) * SQ).astype(fp8np),
        "wk": _pack_dr(np.ascontiguousarray(wks.T) * SK).astype(fp8np),
        "wv": (np.ascontiguousarray(wvs.T) * SV).astype(fp8np),
        "wo": _pack_dr(np.ascontiguousarray(w["Wo"].T) * SW).astype(fp8np),
        "w1": _pack_dr(np.ascontiguousarray(w["W1"].T) * SW).astype(fp8np),
        "w2": _pack_dr(np.ascontiguousarray(w["W2"].T) * SW).astype(fp8np),
        "bq": (bqe * SQ).reshape(2, 128).astype(f32),
        "bk": (bks * SK).reshape(2, 128).astype(f32),
        "bv": bvs.astype(f32),
        "b1": (w["b1"] * SW).reshape(16, 128).astype(f32),
        "bt1": w["ln1_b"].reshape(4, 128).astype(f32),
        "bt1f": (w["ln1_b"] + w["b2"]).reshape(4, 128).astype(f32),
        "g1": w["ln1_g"].reshape(4, 128).astype(f32),
        "g2": w["ln2_g"].reshape(4, 128).astype(f32),
        "bt2": w["ln2_b"].reshape(4, 128).astype(f32),
        "mb": mb.reshape(sup // 128, 128),
    }


def kernel(**inputs):
    global last_results
    w = {k: np.asarray(v, np.float32) for k, v in inputs.items()
         if k not in ("src", "src_mask")}
    src = np.asarray(inputs["src"], np.float32)
    mask = np.asarray(inputs["src_mask"]).reshape(B, S)
    idxs = [np.nonzero(mask[b] != 0)[0] for b in range(B)]
    sup = max(128, ((max(len(i) for i in idxs) + 127) // 128) * 128)

    if sup not in _built_cache:
        _built_cache[sup] = build_bass(sup)
    nc = _built_cache[sup]

    in_maps = [_prep_core(c, src, idxs, sup, w) for c in range(N_CORES)]
    res = bass_utils.run_bass_kernel_spmd(nc, in_maps, core_ids=list(range(N_CORES)),
                                          **run_kwargs)
    last_results = res
    out = np.empty((B, S, HID), np.float32)
    for c in range(N_CORES):
        b, hg = c // 2, c % 2
        out[b, R * hg:R * (hg + 1), :] = res.results[c]["out_t"].T
    return out


# revision 83
# speedup vs baseline: 1.1081x; 1.0001x over previous
"""Trainium2 Bass kernel for nn_EncoderLayer (multiplicative-attention encoder layer).

Sharding: 8 cores; core c handles batch b=c//2, head-group hg=c%2 (4 of 8 heads).
The reference's head-major reshape bug maps head h exactly to output rows
[256h, 256h+256), so each core owns 1024 complete output rows -> no collectives.

v3: - big GEMMs (QKV proj, Wo, FFN1, FFN2) in fp8e4 DoubleRow perf mode
      (0.5 PE cycles/out-col, K=256/pass) with host-prepacked weights and
      power-of-2 pre-scales folded into writer ops / the exp scale.
    - software-pipelined emission: chain work for head h's 256 output tokens
      is interleaved (generator round-robin) with head h+1's attention, so
      Act (exp-bound) and PE (GEMM-bound) run concurrently.
    - softmax tail: recip straight off PSUM den rows, Pool partition_broadcast
      replaces the PE broadcast matmul, xn multiply reads PSUM directly.

Per-token chain independence: LN1/FFN/LN2 normalize over features, so the
chain runs on 256-token blocks (one attention head's scrambled rows each).
FFN1 runs per 512-token pair to halve writer-instruction overhead.
"""

import numpy as np
import ml_dtypes

import concourse.bass as bass
import concourse.tile as tile
import concourse.bacc as bacc
from concourse import mybir
from concourse import bass_utils
from concourse import hw_specs as _hw_specs

_real_gat = _hw_specs.get_activation_tables


def _gat_pinned(arch):
    tabs = _real_gat(arch)
    return {name: (fns if name == "natural_log_exp_and_others" else set())
            for name, fns in tabs.items()}


bacc.get_activation_tables = _gat_pinned

B, S, HID, H, PF, D = 4, 2048, 512, 8, 2048, 64
N_CORES = 8
HPC = H // 2          # heads per core (4)
R = HPC * 256         # output rows per core (1024)
F32 = mybir.dt.float32
BF16 = mybir.dt.bfloat16
FP8 = mybir.dt.float8e4
AF = mybir.ActivationFunctionType
OP = mybir.AluOpType
DR = mybir.MatmulPerfMode.DoubleRow
NEG_BIG = -87.0
LN_EPS = 1e-5
SQ = 64.0             # wq scale
SK = 32.0             # wk scale
SV = 32.0             # wv scale
SX = 64.0             # xn scale
SW = 32.0             # wo/w1/w2 scale
fp8np = ml_dtypes.float8_e4m3

_built_cache = {}
last_results = None
run_kwargs = {}


def _bcast_ap(ap_1d, parts):
    return bass.AP(tensor=ap_1d.tensor, offset=ap_1d.offset,
                   ap=[[0, parts], *ap_1d.ap])


def _pack_dr(wT):
    """[K, M] (K mult of 256) -> DR-packed [K//256 * 128, 2 * M] host layout."""
    K, M = wT.shape
    return np.ascontiguousarray(
        wT.reshape(K // 256, 2, 128, M).transpose(0, 2, 1, 3)
    ).reshape(K // 2, 2 * M)


import os as _os
_RATIO = int(_os.environ.get("KRATIO", "1"))


def _interleave(*gens, ratio=None):
    # first generator gets `ratio` bursts per single burst of the others
    r = ratio if ratio is not None else _RATIO
    active = [iter(g) for g in gens]
    while active:
        for i, g in enumerate(list(active)):
            n = r if (i == 0 and len(active) > 1) else 1
            for _ in range(n):
                try:
                    next(g)
                except StopIteration:
                    if g in active:
                        active.remove(g)
                    break


def build_bass(sup):
    """Per-core module. sup = padded unmasked key count (mult of 128)."""
    KT = sup // 128
    nc = bacc.Bacc("TRN2", target_bir_lowering=False, debug=False,
                   num_devices=N_CORES)

    def inp(name, shape, dt=F32):
        return nc.dram_tensor(name, shape, dt, kind="ExternalInput").ap()

    src8_d = inp("src8", [HID, S], FP8)
    src_res_d = inp("src_res", [HID, R])         # fp32 src.T slice + bo
    srcu8_d = inp("srcu8", [HID, sup], FP8)
    wq_d = inp("wq", [2 * 128, 2 * 256], FP8)    # DR-packed SQ*(Wm@Wq).T
    wk_d = inp("wk", [2 * 128, 2 * 256], FP8)
    wv_d = inp("wv", [HID, 256], FP8)
    wo_d = inp("wo", [2 * 128, 2 * 512], FP8)
    w1_d = inp("w1", [2 * 128, 2 * PF], FP8)
    w2_d = inp("w2", [8 * 128, 2 * 512], FP8)
    bq_d = inp("bq", [2, 128])
    bk_d = inp("bk", [2, 128])
    bv_d = inp("bv", [256])
    b1_d = inp("b1", [16, 128])                  # SW*b1
    bt1_d = inp("bt1", [4, 128])                 # ln1_b
    bt1f_d = inp("bt1f", [4, 128])               # ln1_b + b2
    g1_d = inp("g1", [4, 128])
    g2_d = inp("g2", [4, 128])
    bt2_d = inp("bt2", [4, 128])
    mb_d = inp("mb", [KT, 128])
    out_d = nc.dram_tensor("out_t", [HID, R], F32, kind="ExternalOutput").ap()

    from contextlib import ExitStack
    with tile.TileContext(nc) as tc, ExitStack() as ctx:
        con = ctx.enter_context(tc.tile_pool(name="con", bufs=1))
        ppool = ctx.enter_context(tc.tile_pool(name="ps", bufs=2, space="PSUM"))
        pe_e = ctx.enter_context(tc.tile_pool(name="pe", bufs=2, space="PSUM"))
        pe_av = ctx.enter_context(tc.tile_pool(name="pav", bufs=2, space="PSUM"))
        att_pool = ctx.enter_context(tc.tile_pool(name="att", bufs=12))
        xn_pool = ctx.enter_context(tc.tile_pool(name="xn", bufs=2))
        rep_pool = ctx.enter_context(tc.tile_pool(name="rep", bufs=4))
        h1_pool = ctx.enter_context(tc.tile_pool(name="h1", bufs=2))
        tmp_pool = ctx.enter_context(tc.tile_pool(name="tmp", bufs=5))
        z_pool = ctx.enter_context(tc.tile_pool(name="z", bufs=2))
        o_pool = ctx.enter_context(tc.tile_pool(name="o", bufs=6))

        mm = nc.tensor.matmul
        act = nc.scalar.activation
        dve = nc.vector
        gps = nc.gpsimd

        def dma(out, in_):
            nc.sync.dma_start(out=out, in_=in_)

        def ctile(shape, dt, tag):
            return con.tile(shape, dt, tag=tag, name=tag)

        # ---- constants / weights ----
        srcu8 = ctile([128, 4, sup], FP8, "srcu8")
        wq8 = ctile([128, 2, 2, 256], FP8, "wq8")
        wk8 = ctile([128, 2, 2, 256], FP8, "wk8")
        wv8 = ctile([128, 4, 256], FP8, "wv8")
        dma(wk8, wk_d.rearrange("(t p) (i m) -> p t i m", t=2, i=2))
        dma(srcu8, srcu8_d.rearrange("(c p) n -> p c n", p=128))
        dma(wv8, wv_d.rearrange("(c p) m -> p c m", p=128))
        src8 = ctile([128, 4, S], FP8, "src8")
        dma(src8, src8_d.rearrange("(c p) n -> p c n", p=128))
        dma(wq8, wq_d.rearrange("(t p) (i m) -> p t i m", t=2, i=2))
        src_res = [ctile([128, R], F32, f"srcres{i}") for i in range(4)]
        wo8 = ctile([128, 2, 2, 512], FP8, "wo8")
        w18 = ctile([128, 2, 2, PF], FP8, "w18")
        w28 = ctile([128, 8, 2, 512], FP8, "w28")

        def load_chain_weights():
            dma(wo8, wo_d.rearrange("(t p) (i m) -> p t i m", t=2, i=2))
            for i in range(4):
                dma(src_res[i], src_res_d[128 * i:128 * (i + 1), :])
            dma(w18, w1_d.rearrange("(t p) (i m) -> p t i m", t=2, i=2))
            dma(w28, w2_d.rearrange("(t p) (i m) -> p t i m", t=8, i=2))

        def vec_in(dram, n, tag):
            t = ctile([128, n], F32, tag)
            dma(t, dram.rearrange("m p -> p m"))
            return t

        bq_sb = vec_in(bq_d, 2, "bq")
        bk_sb = vec_in(bk_d, 2, "bk")
        b1_sb = vec_in(b1_d, 16, "b1")
        g1_sb = vec_in(g1_d, 4, "g1")
        bt1_sb = vec_in(bt1_d, 4, "bt1")
        bt1f_sb = vec_in(bt1f_d, 4, "bt1f")
        g2_sb = vec_in(g2_d, 4, "g2")
        bt2_sb = vec_in(bt2_d, 4, "bt2")
        mb_sb = vec_in(mb_d, KT, "mb")
        bv_rep = ctile([128, 256], F32, "bvrep")
        dma(bv_rep, _bcast_ap(bv_d, 128))

        ones_bf = ctile([128, 128], BF16, "onesbf")
        dve.memset(ones_bf, 1.0)
        eps_t = ctile([128, 1], F32, "eps")
        dve.memset(eps_t, LN_EPS)

        q_sb = [ctile([128, S], BF16, f"q{m}") for m in range(2)]
        k_sb = [ctile([128, sup], BF16, f"k{m}") for m in range(2)]

        def gen_proj(w8, bias_sb, src_t, n_total, out_tiles, mt):
            n0 = 0
            while n0 < n_total:
                nq = min(512, n_total - n0)
                for half in range(2):
                    ps = ppool.tile([64, 512], F32, tag="ps", name="psp")
                    mcol = 64 * (2 * mt + half)
                    for t in range(2):
                        mm(ps[:, :nq],
                           w8[:, t, :, mcol:mcol + 64],
                           src_t[:, 2 * t:2 * t + 2, n0:n0 + nq],
                           start=(t == 0), stop=(t == 1), perf_mode=DR)
                    dve.tensor_scalar_add(
                        out_tiles[mt][64 * half:64 * half + 64, n0:n0 + nq],
                        ps[:, :nq],
                        bias_sb[64 * half:64 * half + 64, mt:mt + 1])
                    yield
                n0 += nq

        # ---- V natural [keys, 4*128] bf16: 64 V cols + 64 ones cols ----
        # (AV matmul cost is N-proportional, M-free: 64 ones cols give 64
        # identical den rows so recip writes [64, W] directly -- no broadcast)
        v_sb = ctile([128, KT * 4 * 128], BF16, "v")
        v_v = v_sb.rearrange("p (kt h e) -> p kt h e", kt=KT, h=4)

        def gen_vproj():
            # ones cols hold 1/SX so den rows accumulate den/SX and
            # rep = recip(den/SX) = SX/den (xn lands mid-range for fp8)
            dve.memset(v_v[:, :, :, 64:128], 1.0 / SX)
            for kt in range(KT):
                ps = ppool.tile([128, 512], F32, tag="ps", name="psv")
                for ct in range(4):
                    mm(ps[:, :256], srcu8[:, ct, 128 * kt:128 * (kt + 1)],
                       wv8[:, ct, :], start=(ct == 0), stop=(ct == 3))
                dve.scalar_tensor_tensor(
                    out=v_v[:, kt, :, 0:64],
                    in0=ps[:, :256].rearrange("p (h d) -> p h d", h=4),
                    scalar=1.0 / SV,
                    in1=bv_rep.rearrange("p (h d) -> p h d", h=4),
                    op0=OP.mult, op1=OP.add)
                yield

        # ---- attention head h -> fp8 xnp half (scaled by SX) ----
        def gen_attention(h, xnp):
            g = h // 2
            p0 = 64 * (h % 2)
            o0 = S * (h % 2)
            for q0 in range(0, S, 1024):
                avs = []
                for half in range(2):
                    avs.append(pe_av.tile([128, 512], F32, tag="av", name="av"))
                for kt in range(KT):
                    e = pe_e.tile([128, 1024], F32, tag="e", name="e")
                    with tc.high_priority():
                        for half in range(2):
                            mm(e[:, 512 * half:512 * (half + 1)],
                               k_sb[g][p0:p0 + 64, 128 * kt:128 * (kt + 1)],
                               q_sb[g][p0:p0 + 64,
                                       q0 + 512 * half:q0 + 512 * (half + 1)],
                               start=True, stop=True)
                    at = att_pool.tile([128, 1024], BF16, tag="att", name="att")
                    act(at, e, AF.Exp, bias=mb_sb[:, kt:kt + 1],
                        scale=1.0 / (SQ * SK))
                    for half in range(2):
                        mm(avs[half], v_v[:, kt, h, :],
                           at[:, 512 * half:512 * (half + 1)],
                           start=(kt == 0), stop=(kt == KT - 1),
                           skip_group_check=True)
                    yield
                # tail: rep = SX/den via recip of the 4 identical den rows;
                # broadcast row 0 to 64 partitions on Pool; xn = x' * rep.
                for half in range(2):
                    rep = rep_pool.tile([64, 512], BF16, tag="rep", name="rep")
                    with tc.high_priority(), \
                         nc.allow_low_precision(reason="softmax recip"):
                        dve.reciprocal(rep, avs[half][64:128, :])
                    with tc.high_priority():
                        dve.tensor_tensor(
                            out=xnp[0:64, o0 + q0 + 512 * half:o0 + q0 + 512 * (half + 1)],
                            in0=avs[half][0:64, :], in1=rep, op=OP.mult)
                    yield
            with tc.high_priority():
                gps.tensor_copy(out=xnp[64:128, o0:o0 + S // 2],
                                in_=xnp[0:64, o0 + 1:o0 + S // 2 + 1])
                gps.tensor_copy(out=xnp[64:128, o0 + S // 2:o0 + S - 1],
                                in_=xnp[0:64, o0 + S // 2 + 1:o0 + S])
            yield

        # ---- layernorm on 4x[128, W] f32 z-tiles ----
        def gen_layernorm(z_tiles, g_sb, writers, W, stats_pool=None):
            # s1/s2 must sit in separate PSUM banks: a start=True matmul marks
            # its whole 2KB zero-region pending-zero, wiping any sibling
            # accumulation group sharing the bank.  stats_pool lets tail LNs
            # borrow the idle attention av-tag banks so concurrent chain
            # lanes don't cycle-deadlock on the shared chain psum tag.
            sp = stats_pool
            if sp is None:
                s1 = ppool.tile([128, W], F32, tag="ps", name="s1")
                s2 = ppool.tile([128, W], F32, tag="ps", name="s2")
            else:
                s1 = sp.tile([128, W], F32, tag="av", name="s1")
                s2 = sp.tile([128, W], F32, tag="av", name="s2")
            for ct in range(4):
                zb = tmp_pool.tile([128, W], BF16, tag="zb", name="zb")
                gps.tensor_copy(out=zb, in_=z_tiles[ct])
                sq = tmp_pool.tile([128, W], BF16, tag="sq", name="sq")
                dve.tensor_tensor(out=sq, in0=zb, in1=zb, op=OP.mult)
                mm(s1, ones_bf, zb, start=(ct == 0), stop=(ct == 3),
                   skip_group_check=True)
                mm(s2, ones_bf, sq, start=(ct == 0), stop=(ct == 3),
                   skip_group_check=True)
                yield
            bm = tmp_pool.tile([128, W], F32, tag="bm", name="bm")
            br = tmp_pool.tile([128, W], F32, tag="br", name="br")
            m2 = tmp_pool.tile([128, W], BF16, tag="m2", name="m2", bufs=1)
            with tc.high_priority():
                dve.tensor_scalar_mul(bm, s1, 1.0 / HID)
                dve.tensor_tensor(out=m2, in0=bm, in1=bm, op=OP.mult)
                dve.scalar_tensor_tensor(out=br, in0=s2,
                                         scalar=1.0 / HID, in1=m2,
                                         op0=OP.mult, op1=OP.subtract)
            with tc.high_priority():
                act(br, br, AF.Ln, bias=eps_t)
                act(br, br, AF.Exp, scale=-0.5)
            yield
            for ct in range(4):
                sub = tmp_pool.tile([128, W], F32, tag="sub", name="sub")
                gps.tensor_tensor(out=sub, in0=z_tiles[ct], in1=bm,
                                  op=OP.subtract)
                t2 = tmp_pool.tile([128, W], F32, tag="t2", name="t2")
                dve.scalar_tensor_tensor(out=t2, in0=sub,
                                         scalar=g_sb[:, ct:ct + 1], in1=br,
                                         op0=OP.mult, op1=OP.mult)
                writers(ct, t2)
                yield

        # ---- chain A for one 256-token block: Wo(DR) + res, LN1 ----
        src1_f = [[con.tile([128, 256], F32, tag=f"s1f{i}_{j}",
                            name=f"s1f{i}_{j}") for j in range(4)]
                  for i in range(4)]
        src1_8 = ctile([128, 4, R], FP8, "src1_8")

        def gen_chain_a(blk, xnp, stats_pool=None):
            c0 = 256 * blk
            hh = blk % 2
            xw = xnp.rearrange("p (hh m j) -> p j hh m", hh=2, j=8)
            z1 = [z_pool.tile([128, 256], F32, tag=f"z{mt}", name=f"z{mt}")
                  for mt in range(4)]
            for mt in range(4):
                for half in range(2):
                    ps = ppool.tile([64, 512], F32, tag="ps", name="pswo")
                    mcol = 64 * (2 * mt + half)
                    for t in range(2):
                        ifm = bass.AP(
                            tensor=xw.tensor,
                            offset=xw.offset + (4 * t) * xw.ap[1][0]
                            + hh * xw.ap[2][0],
                            ap=[xw.ap[0], [2 * xw.ap[1][0], 2], xw.ap[3]])
                        mm(ps[:, :256], wo8[:, t, :, mcol:mcol + 64], ifm,
                           start=(t == 0), stop=(t == 1), perf_mode=DR)
                    with tc.high_priority():
                        dve.scalar_tensor_tensor(
                            out=z1[mt][64 * half:64 * half + 64, :],
                            in0=ps[:, :256],
                            scalar=1.0 / (SX * SW),
                            in1=src_res[mt][64 * half:64 * half + 64, c0:c0 + 256],
                            op0=OP.mult, op1=OP.add)
                    yield

            def w1(ct, t2):
                gps.tensor_scalar_add(src1_f[ct][blk], t2,
                                      bt1f_sb[:, ct:ct + 1])
                gps.tensor_scalar_add(src1_8[:, ct, c0:c0 + 256], t2,
                                      bt1_sb[:, ct:ct + 1])

            yield from gen_layernorm(z1, g1_sb, w1, 256, stats_pool=stats_pool)

        # ---- FFN1 for a 512-token pair (blocks 2p, 2p+1) ----
        h18s = {}

        def gen_ffn1(p):
            c0 = 512 * p
            h18 = h1_pool.tile([128, 16, 512], FP8, tag="h1", name="h1", bufs=2)
            h18s[p] = h18
            for mt in range(16):
                for half in range(2):
                    ps = ppool.tile([64, 512], F32, tag="ps", name="psf1")
                    mcol = 64 * (2 * mt + half)
                    for t in range(2):
                        mm(ps, w18[:, t, :, mcol:mcol + 64],
                           src1_8[:, 2 * t:2 * t + 2, c0:c0 + 512],
                           start=(t == 0), stop=(t == 1), perf_mode=DR)
                    with tc.high_priority():
                        if half == 0:
                            act(h18[0:64, mt, :], ps, AF.Relu,
                                bias=b1_sb[0:64, mt:mt + 1], scale=1.0)
                        else:
                            dve.tensor_scalar(
                                out=h18[64:128, mt, :], in0=ps,
                                scalar1=b1_sb[64:128, mt:mt + 1], scalar2=0.0,
                                op0=OP.add, op1=OP.max)
                    yield

        # ---- chain B for one 256-token block: FFN2(DR) + res, LN2, out ----
        z2s = {}

        def gen_chain_b_ffn(blk):
            c0 = 256 * blk
            h18 = h18s[blk // 2]
            r0 = 256 * (blk % 2)
            z2 = [z_pool.tile([128, 256], F32, tag=f"z{ot}", name=f"z{ot}")
                  for ot in range(4)]
            z2s[blk] = z2
            for ot in range(4):
                for half in range(2):
                    ps = ppool.tile([64, 512], F32, tag="ps", name="psf2")
                    mcol = 64 * (2 * ot + half)
                    for t in range(8):
                        mm(ps[:, :256], w28[:, t, :, mcol:mcol + 64],
                           h18[:, 2 * t:2 * t + 2, r0:r0 + 256],
                           start=(t == 0), stop=(t == 7), perf_mode=DR)
                    with tc.high_priority():
                        dve.scalar_tensor_tensor(
                            out=z2[ot][64 * half:64 * half + 64, :],
                            in0=ps[:, :256],
                            scalar=1.0 / (SW * SW),
                            in1=src1_f[ot][blk][64 * half:64 * half + 64, :],
                            op0=OP.mult, op1=OP.add)
                    yield

        def gen_chain_b_ln(blk, stats_pool=None):
            c0 = 256 * blk
            z2 = z2s[blk]

            def w2(ct, t2):
                o = o_pool.tile([128, 256], F32, tag="out", name="out", bufs=6)
                gps.tensor_scalar_add(o, t2, bt2_sb[:, ct:ct + 1])
                dma(out_d[128 * ct:128 * (ct + 1), c0:c0 + 256], o)

            yield from gen_layernorm(z2, g2_sb, w2, 256, stats_pool=stats_pool)

        def gen_chain_b(blk, stats_pool=None):
            yield from gen_chain_b_ffn(blk)
            yield from gen_chain_b_ln(blk, stats_pool=stats_pool)

        def gen_seq(*gens):
            for g in gens:
                yield from g

        # ---- schedule ----
        # V' ones cols hold 1/SX so den rows accumulate den/SX and
        # rep4 = recip(den/SX) = SX/den.  (memset inside gen_vproj runs first.)
        xnp0 = xn_pool.tile([128, 2 * S], FP8, tag="xn", name="xn")
        xnp1 = xn_pool.tile([128, 2 * S], FP8, tag="xn", name="xn")

        _interleave(gen_proj(wk8, bk_sb, srcu8, sup, k_sb, 0),
                    gen_proj(wq8, bq_sb, src8, S, q_sb, 0),
                    gen_vproj())
        load_chain_weights()
        _interleave(gen_attention(0, xnp0),
                    gen_seq(gen_proj(wk8, bk_sb, srcu8, sup, k_sb, 1),
                            gen_proj(wq8, bq_sb, src8, S, q_sb, 1)))
        _interleave(gen_attention(1, xnp0), gen_chain_a(0, xnp0))
        _interleave(gen_attention(2, xnp1),
                    gen_seq(gen_chain_a(1, xnp0), gen_ffn1(0)))
        _interleave(gen_attention(3, xnp1),
                    gen_seq(gen_chain_b(0), gen_chain_a(2, xnp1),
                            gen_chain_b(1)))
        _interleave(gen_seq(gen_chain_a(3, xnp1), gen_ffn1(1),
                            gen_chain_b_ffn(2)))
        _interleave(gen_seq(gen_chain_b_ln(2, stats_pool=pe_av),
                            gen_chain_b_ln(3)),
                    gen_chain_b_ffn(3))

    nc.compile()
    return nc


def _prep_core(c, src, idxs, sup, w):
    b, hg = c // 2, c % 2
    heads = list(range(HPC * hg, HPC * hg + HPC))
    st = np.ascontiguousarray(src[b].T)                       # [512, 2048] f32
    idx = idxs[b]
    su = len(idx)
    srcu = np.zeros((HID, sup), np.float32)
    srcu[:, :su] = st[:, idx]
    wqe = np.concatenate([w["Wm"] @ w["Wq"][64 * h:64 * (h + 1), :] for h in heads])
    bqe = np.concatenate([w["Wm"] @ w["bq"][64 * h:64 * (h + 1)] + w["bm"]
                          for h in heads])
    wks = np.concatenate([w["Wk"][64 * h:64 * (h + 1), :] for h in heads])
    bks = np.concatenate([w["bk"][64 * h:64 * (h + 1)] for h in heads])
    wvs = np.concatenate([w["Wv"][64 * h:64 * (h + 1), :] for h in heads])
    bvs = np.concatenate([w["bv"][64 * h:64 * (h + 1)] for h in heads])
    mb = np.full(sup, NEG_BIG, np.float32)
    mb[:su] = 0.0
    f32 = np.float32
    src_res = np.ascontiguousarray(st[:, R * hg:R * (hg + 1)]) \
        + w["bo"][:, None].astype(f32)
    return {
        "src8": st.astype(fp8np),
        "src_res": src_res.astype(f32),
        "srcu8": srcu.astype(fp8np),
        "wq": _pack_dr(np.ascontiguousarray(wqe.T) * SQ).astype(fp8np),
        "wk": _pack_dr(np.ascontiguousarray(wks.T) * SK).astype(fp8np),
        "wv": (np.ascontiguousarray(wvs.T) * SV).astype(fp8np),
        "wo": _pack_dr(np.ascontiguousarray(w["Wo"].T) * SW).astype(fp8np),
        "w1": _pack_dr(np.ascontiguousarray(w["W1"].T) * SW).astype(fp8np),
        "w2": _pack_dr(np.ascontiguousarray(w["W2"].T) * SW).astype(fp8np),
        "bq": (bqe * SQ).reshape(2, 128).astype(f32),
        "bk": (bks * SK).reshape(2, 128).astype(f32),
        "bv": bvs.astype(f32),
        "b1": (w["b1"] * SW).reshape(16, 128).astype(f32),
        "bt1": w["ln1_b"].reshape(4, 128).astype(f32),
        "bt1f": (w["ln1_b"] + w["b2"]).reshape(4, 128).astype(f32),
        "g1": w["ln1_g"].reshape(4, 128).astype(f32),
        "g2": w["ln2_g"].reshape(4, 128).astype(f32),
        "bt2": w["ln2_b"].reshape(4, 128).astype(f32),
        "mb": mb.reshape(sup // 128, 128),
    }


def kernel(**inputs):
    global last_results
    w = {k: np.asarray(v, np.float32) for k, v in inputs.items()
         if k not in ("src", "src_mask")}
    src = np.asarray(inputs["src"], np.float32)
    mask = np.asarray(inputs["src_mask"]).reshape(B, S)
    idxs = [np.nonzero(mask[b] != 0)[0] for b in range(B)]
    sup = max(128, ((max(len(i) for i in idxs) + 127) // 128) * 128)

    if sup not in _built_cache:
        _built_cache[sup] = build_bass(sup)
    nc = _built_cache[sup]

    in_maps = [_prep_core(c, src, idxs, sup, w) for c in range(N_CORES)]
    res = bass_utils.run_bass_kernel_spmd(nc, in_maps, core_ids=list(range(N_CORES)),
                                          **run_kwargs)
    last_results = res
    out = np.empty((B, S, HID), np.float32)
    for c in range(N_CORES):
        b, hg = c // 2, c % 2
        out[b, R * hg:R * (hg + 1), :] = res.results[c]["out_t"].T
    return out


# revision 86
# speedup vs baseline: 1.1163x; 1.0075x over previous
"""Trainium2 Bass kernel for nn_EncoderLayer (multiplicative-attention encoder layer).

Sharding: 8 cores; core c handles batch b=c//2, head-group hg=c%2 (4 of 8 heads).
The reference's head-major reshape bug maps head h exactly to output rows
[256h, 256h+256), so each core owns 1024 complete output rows -> no collectives.

v3: - big GEMMs (QKV proj, Wo, FFN1, FFN2) in fp8e4 DoubleRow perf mode
      (0.5 PE cycles/out-col, K=256/pass) with host-prepacked weights and
      power-of-2 pre-scales folded into writer ops / the exp scale.
    - software-pipelined emission: chain work for head h's 256 output tokens
      is interleaved (generator round-robin) with head h+1's attention, so
      Act (exp-bound) and PE (GEMM-bound) run concurrently.
    - softmax tail: recip straight off PSUM den rows, Pool partition_broadcast
      replaces the PE broadcast matmul, xn multiply reads PSUM directly.

Per-token chain independence: LN1/FFN/LN2 normalize over features, so the
chain runs on 256-token blocks (one attention head's scrambled rows each).
FFN1 runs per 512-token pair to halve writer-instruction overhead.
"""

import numpy as np
import ml_dtypes

import concourse.bass as bass
import concourse.tile as tile
import concourse.bacc as bacc
from concourse import mybir
from concourse import bass_utils
from concourse import hw_specs as _hw_specs

_real_gat = _hw_specs.get_activation_tables


def _gat_pinned(arch):
    tabs = _real_gat(arch)
    return {name: (fns if name == "natural_log_exp_and_others" else set())
            for name, fns in tabs.items()}


bacc.get_activation_tables = _gat_pinned

B, S, HID, H, PF, D = 4, 2048, 512, 8, 2048, 64
N_CORES = 8
HPC = H // 2          # heads per core (4)
R = HPC * 256         # output rows per core (1024)
F32 = mybir.dt.float32
BF16 = mybir.dt.bfloat16
FP8 = mybir.dt.float8e4
AF = mybir.ActivationFunctionType
OP = mybir.AluOpType
DR = mybir.MatmulPerfMode.DoubleRow
NEG_BIG = -87.0
LN_EPS = 1e-5
SQ = 64.0             # wq scale
SK = 32.0             # wk scale
SV = 32.0             # wv scale
SX = 64.0             # xn scale
SW = 32.0             # wo/w1/w2 scale
fp8np = ml_dtypes.float8_e4m3

_built_cache = {}
last_results = None
run_kwargs = {}


def _bcast_ap(ap_1d, parts):
    return bass.AP(tensor=ap_1d.tensor, offset=ap_1d.offset,
                   ap=[[0, parts], *ap_1d.ap])


def _pack_dr(wT):
    """[K, M] (K mult of 256) -> DR-packed [K//256 * 128, 2 * M] host layout."""
    K, M = wT.shape
    return np.ascontiguousarray(
        wT.reshape(K // 256, 2, 128, M).transpose(0, 2, 1, 3)
    ).reshape(K // 2, 2 * M)


import os as _os
_RATIO = int(_os.environ.get("KRATIO", "1"))


def _interleave(*gens, ratio=None):
    # first generator gets `ratio` bursts per single burst of the others
    r = ratio if ratio is not None else _RATIO
    active = [iter(g) for g in gens]
    while active:
        for i, g in enumerate(list(active)):
            n = r if (i == 0 and len(active) > 1) else 1
            for _ in range(n):
                try:
                    next(g)
                except StopIteration:
                    if g in active:
                        active.remove(g)
                    break


def build_bass(sup):
    """Per-core module. sup = padded unmasked key count (mult of 128)."""
    KT = sup // 128
    nc = bacc.Bacc("TRN2", target_bir_lowering=False, debug=False,
                   num_devices=N_CORES)

    def inp(name, shape, dt=F32):
        return nc.dram_tensor(name, shape, dt, kind="ExternalInput").ap()

    src8_d = inp("src8", [HID, S], FP8)
    src_res_d = inp("src_res", [HID, R])         # fp32 src.T slice + bo
    srcu8_d = inp("srcu8", [HID, sup], FP8)
    wq_d = inp("wq", [2 * 128, 2 * 256], FP8)    # DR-packed SQ*(Wm@Wq).T
    wk_d = inp("wk", [2 * 128, 2 * 256], FP8)
    wv_d = inp("wv", [HID, 256], FP8)
    wo_d = inp("wo", [2 * 128, 2 * 512], FP8)
    w1_d = inp("w1", [2 * 128, 2 * PF], FP8)
    w2_d = inp("w2", [8 * 128, 2 * 512], FP8)
    bq_d = inp("bq", [2, 128])
    bk_d = inp("bk", [2, 128])
    bv_d = inp("bv", [256])
    b1_d = inp("b1", [16, 128])                  # SW*b1
    bt1_d = inp("bt1", [4, 128])                 # ln1_b
    bt1f_d = inp("bt1f", [4, 128])               # ln1_b + b2
    g1_d = inp("g1", [4, 128])
    g2_d = inp("g2", [4, 128])
    bt2_d = inp("bt2", [4, 128])
    mb_d = inp("mb", [KT, 128])
    out_d = nc.dram_tensor("out_t", [HID, R], F32, kind="ExternalOutput").ap()

    from contextlib import ExitStack
    with tile.TileContext(nc) as tc, ExitStack() as ctx:
        con = ctx.enter_context(tc.tile_pool(name="con", bufs=1))
        ppool = ctx.enter_context(tc.tile_pool(name="ps", bufs=2, space="PSUM"))
        pe_e = ctx.enter_context(tc.tile_pool(name="pe", bufs=2, space="PSUM"))
        pe_av = ctx.enter_context(tc.tile_pool(name="pav", bufs=2, space="PSUM"))
        att_pool = ctx.enter_context(tc.tile_pool(name="att", bufs=12))
        xn_pool = ctx.enter_context(tc.tile_pool(name="xn", bufs=2))
        rep_pool = ctx.enter_context(tc.tile_pool(name="rep", bufs=4))
        h1_pool = ctx.enter_context(tc.tile_pool(name="h1", bufs=2))
        tmp_pool = ctx.enter_context(tc.tile_pool(name="tmp", bufs=5))
        z_pool = ctx.enter_context(tc.tile_pool(name="z", bufs=2))
        o_pool = ctx.enter_context(tc.tile_pool(name="o", bufs=6))

        mm = nc.tensor.matmul
        act = nc.scalar.activation
        dve = nc.vector
        gps = nc.gpsimd

        def dma(out, in_):
            nc.sync.dma_start(out=out, in_=in_)

        def ctile(shape, dt, tag):
            return con.tile(shape, dt, tag=tag, name=tag)

        # ---- constants / weights ----
        srcu8 = ctile([128, 4, sup], FP8, "srcu8")
        wq8 = ctile([128, 2, 2, 256], FP8, "wq8")
        wk8 = ctile([128, 2, 2, 256], FP8, "wk8")
        wv8 = ctile([128, 4, 256], FP8, "wv8")
        dma(wk8, wk_d.rearrange("(t p) (i m) -> p t i m", t=2, i=2))
        dma(wq8, wq_d.rearrange("(t p) (i m) -> p t i m", t=2, i=2))
        dma(srcu8, srcu8_d.rearrange("(c p) n -> p c n", p=128))
        src8 = ctile([128, 4, S], FP8, "src8")
        dma(src8, src8_d.rearrange("(c p) n -> p c n", p=128))
        dma(wv8, wv_d.rearrange("(c p) m -> p c m", p=128))
        src_res = [ctile([128, R], F32, f"srcres{i}") for i in range(4)]
        wo8 = ctile([128, 2, 2, 512], FP8, "wo8")
        w18 = ctile([128, 2, 2, PF], FP8, "w18")
        w28 = ctile([128, 8, 2, 512], FP8, "w28")

        def load_chain_weights():
            dma(wo8, wo_d.rearrange("(t p) (i m) -> p t i m", t=2, i=2))
            for i in range(4):
                dma(src_res[i], src_res_d[128 * i:128 * (i + 1), :])
            dma(w18, w1_d.rearrange("(t p) (i m) -> p t i m", t=2, i=2))
            dma(w28, w2_d.rearrange("(t p) (i m) -> p t i m", t=8, i=2))

        def vec_in(dram, n, tag):
            t = ctile([128, n], F32, tag)
            dma(t, dram.rearrange("m p -> p m"))
            return t

        bq_sb = vec_in(bq_d, 2, "bq")
        bk_sb = vec_in(bk_d, 2, "bk")
        b1_sb = vec_in(b1_d, 16, "b1")
        g1_sb = vec_in(g1_d, 4, "g1")
        bt1_sb = vec_in(bt1_d, 4, "bt1")
        bt1f_sb = vec_in(bt1f_d, 4, "bt1f")
        g2_sb = vec_in(g2_d, 4, "g2")
        bt2_sb = vec_in(bt2_d, 4, "bt2")
        mb_sb = vec_in(mb_d, KT, "mb")
        bv_rep = ctile([128, 256], F32, "bvrep")
        dma(bv_rep, _bcast_ap(bv_d, 128))

        ones_bf = ctile([128, 128], BF16, "onesbf")
        dve.memset(ones_bf, 1.0)
        eps_t = ctile([128, 1], F32, "eps")
        dve.memset(eps_t, LN_EPS)

        q_sb = [ctile([128, S], BF16, f"q{m}") for m in range(2)]
        k_sb = [ctile([128, sup], BF16, f"k{m}") for m in range(2)]

        def gen_proj(w8, bias_sb, src_t, n_total, out_tiles, mt):
            n0 = 0
            while n0 < n_total:
                nq = min(512, n_total - n0)
                for half in range(2):
                    ps = ppool.tile([64, 512], F32, tag="ps", name="psp")
                    mcol = 64 * (2 * mt + half)
                    for t in range(2):
                        mm(ps[:, :nq],
                           w8[:, t, :, mcol:mcol + 64],
                           src_t[:, 2 * t:2 * t + 2, n0:n0 + nq],
                           start=(t == 0), stop=(t == 1), perf_mode=DR)
                    dve.tensor_scalar_add(
                        out_tiles[mt][64 * half:64 * half + 64, n0:n0 + nq],
                        ps[:, :nq],
                        bias_sb[64 * half:64 * half + 64, mt:mt + 1])
                    yield
                n0 += nq

        # ---- V natural [keys, 4*128] bf16: 64 V cols + 64 ones cols ----
        # (AV matmul cost is N-proportional, M-free: 64 ones cols give 64
        # identical den rows so recip writes [64, W] directly -- no broadcast)
        v_sb = ctile([128, KT * 4 * 128], BF16, "v")
        v_v = v_sb.rearrange("p (kt h e) -> p kt h e", kt=KT, h=4)

        def gen_vproj():
            # ones cols hold 1/SX so den rows accumulate den/SX and
            # rep = recip(den/SX) = SX/den (xn lands mid-range for fp8)
            dve.memset(v_v[:, :, :, 64:128], 1.0 / SX)
            for kt in range(KT):
                ps = ppool.tile([128, 512], F32, tag="ps", name="psv")
                for ct in range(4):
                    mm(ps[:, :256], srcu8[:, ct, 128 * kt:128 * (kt + 1)],
                       wv8[:, ct, :], start=(ct == 0), stop=(ct == 3))
                dve.scalar_tensor_tensor(
                    out=v_v[:, kt, :, 0:64],
                    in0=ps[:, :256].rearrange("p (h d) -> p h d", h=4),
                    scalar=1.0 / SV,
                    in1=bv_rep.rearrange("p (h d) -> p h d", h=4),
                    op0=OP.mult, op1=OP.add)
                yield

        # ---- attention head h -> fp8 xnp half (scaled by SX) ----
        def gen_attention(h, xnp):
            g = h // 2
            p0 = 64 * (h % 2)
            o0 = S * (h % 2)
            for q0 in range(0, S, 1024):
                avs = []
                for half in range(2):
                    avs.append(pe_av.tile([128, 512], F32, tag="av", name="av"))
                for kt in range(KT):
                    e = pe_e.tile([128, 1024], F32, tag="e", name="e")
                    with tc.high_priority():
                        for half in range(2):
                            mm(e[:, 512 * half:512 * (half + 1)],
                               k_sb[g][p0:p0 + 64, 128 * kt:128 * (kt + 1)],
                               q_sb[g][p0:p0 + 64,
                                       q0 + 512 * half:q0 + 512 * (half + 1)],
                               start=True, stop=True)
                    at = att_pool.tile([128, 1024], BF16, tag="att", name="att")
                    act(at, e, AF.Exp, bias=mb_sb[:, kt:kt + 1],
                        scale=1.0 / (SQ * SK))
                    for half in range(2):
                        mm(avs[half], v_v[:, kt, h, :],
                           at[:, 512 * half:512 * (half + 1)],
                           start=(kt == 0), stop=(kt == KT - 1),
                           skip_group_check=True)
                    yield
                # tail: rep = SX/den via recip of the 4 identical den rows;
                # broadcast row 0 to 64 partitions on Pool; xn = x' * rep.
                for half in range(2):
                    rep = rep_pool.tile([64, 512], BF16, tag="rep", name="rep")
                    with tc.high_priority(), \
                         nc.allow_low_precision(reason="softmax recip"):
                        dve.reciprocal(rep, avs[half][64:128, :])
                    with tc.high_priority():
                        dve.tensor_tensor(
                            out=xnp[0:64, o0 + q0 + 512 * half:o0 + q0 + 512 * (half + 1)],
                            in0=avs[half][0:64, :], in1=rep, op=OP.mult)
                    yield
            with tc.high_priority():
                gps.tensor_copy(out=xnp[64:128, o0:o0 + S // 2],
                                in_=xnp[0:64, o0 + 1:o0 + S // 2 + 1])
                gps.tensor_copy(out=xnp[64:128, o0 + S // 2:o0 + S - 1],
                                in_=xnp[0:64, o0 + S // 2 + 1:o0 + S])
            yield

        # ---- layernorm on 4x[128, W] f32 z-tiles ----
        def gen_layernorm(z_tiles, g_sb, writers, W, stats_pool=None):
            # s1/s2 must sit in separate PSUM banks: a start=True matmul marks
            # its whole 2KB zero-region pending-zero, wiping any sibling
            # accumulation group sharing the bank.  stats_pool lets tail LNs
            # borrow the idle attention av-tag banks so concurrent chain
            # lanes don't cycle-deadlock on the shared chain psum tag.
            sp = stats_pool
            if sp is None:
                s1 = ppool.tile([128, W], F32, tag="ps", name="s1")
                s2 = ppool.tile([128, W], F32, tag="ps", name="s2")
            else:
                s1 = sp.tile([128, W], F32, tag="av", name="s1")
                s2 = sp.tile([128, W], F32, tag="av", name="s2")
            for ct in range(4):
                zb = tmp_pool.tile([128, W], BF16, tag="zb", name="zb")
                gps.tensor_copy(out=zb, in_=z_tiles[ct])
                sq = tmp_pool.tile([128, W], BF16, tag="sq", name="sq")
                dve.tensor_tensor(out=sq, in0=zb, in1=zb, op=OP.mult)
                mm(s1, ones_bf, zb, start=(ct == 0), stop=(ct == 3),
                   skip_group_check=True)
                mm(s2, ones_bf, sq, start=(ct == 0), stop=(ct == 3),
                   skip_group_check=True)
                yield
            bm = tmp_pool.tile([128, W], F32, tag="bm", name="bm")
            br = tmp_pool.tile([128, W], F32, tag="br", name="br")
            m2 = tmp_pool.tile([128, W], BF16, tag="m2", name="m2", bufs=1)
            with tc.high_priority():
                dve.tensor_scalar_mul(bm, s1, 1.0 / HID)
                dve.tensor_tensor(out=m2, in0=bm, in1=bm, op=OP.mult)
                dve.scalar_tensor_tensor(out=br, in0=s2,
                                         scalar=1.0 / HID, in1=m2,
                                         op0=OP.mult, op1=OP.subtract)
            with tc.high_priority():
                act(br, br, AF.Ln, bias=eps_t)
                act(br, br, AF.Exp, scale=-0.5)
            yield
            for ct in range(4):
                sub = tmp_pool.tile([128, W], F32, tag="sub", name="sub")
                gps.tensor_tensor(out=sub, in0=z_tiles[ct], in1=bm,
                                  op=OP.subtract)
                t2 = tmp_pool.tile([128, W], F32, tag="t2", name="t2")
                dve.scalar_tensor_tensor(out=t2, in0=sub,
                                         scalar=g_sb[:, ct:ct + 1], in1=br,
                                         op0=OP.mult, op1=OP.mult)
                writers(ct, t2)
                yield

        # ---- chain A for one 256-token block: Wo(DR) + res, LN1 ----
        src1_f = [[con.tile([128, 256], F32, tag=f"s1f{i}_{j}",
                            name=f"s1f{i}_{j}") for j in range(4)]
                  for i in range(4)]
        src1_8 = ctile([128, 4, R], FP8, "src1_8")

        def gen_chain_a(blk, xnp, stats_pool=None):
            c0 = 256 * blk
            hh = blk % 2
            xw = xnp.rearrange("p (hh m j) -> p j hh m", hh=2, j=8)
            z1 = [z_pool.tile([128, 256], F32, tag=f"z{mt}", name=f"z{mt}")
                  for mt in range(4)]
            for mt in range(4):
                for half in range(2):
                    ps = ppool.tile([64, 512], F32, tag="ps", name="pswo")
                    mcol = 64 * (2 * mt + half)
                    for t in range(2):
                        ifm = bass.AP(
                            tensor=xw.tensor,
                            offset=xw.offset + (4 * t) * xw.ap[1][0]
                            + hh * xw.ap[2][0],
                            ap=[xw.ap[0], [2 * xw.ap[1][0], 2], xw.ap[3]])
                        mm(ps[:, :256], wo8[:, t, :, mcol:mcol + 64], ifm,
                           start=(t == 0), stop=(t == 1), perf_mode=DR)
                    with tc.high_priority():
                        dve.scalar_tensor_tensor(
                            out=z1[mt][64 * half:64 * half + 64, :],
                            in0=ps[:, :256],
                            scalar=1.0 / (SX * SW),
                            in1=src_res[mt][64 * half:64 * half + 64, c0:c0 + 256],
                            op0=OP.mult, op1=OP.add)
                    yield

            def w1(ct, t2):
                gps.tensor_scalar_add(src1_f[ct][blk], t2,
                                      bt1f_sb[:, ct:ct + 1])
                gps.tensor_scalar_add(src1_8[:, ct, c0:c0 + 256], t2,
                                      bt1_sb[:, ct:ct + 1])

            yield from gen_layernorm(z1, g1_sb, w1, 256, stats_pool=stats_pool)

        # ---- FFN1 for a 512-token pair (blocks 2p, 2p+1) ----
        h18s = {}

        def gen_ffn1(p):
            c0 = 512 * p
            h18 = h1_pool.tile([128, 16, 512], FP8, tag="h1", name="h1", bufs=2)
            h18s[p] = h18
            for mt in range(16):
                for half in range(2):
                    ps = ppool.tile([64, 512], F32, tag="ps", name="psf1")
                    mcol = 64 * (2 * mt + half)
                    for t in range(2):
                        mm(ps, w18[:, t, :, mcol:mcol + 64],
                           src1_8[:, 2 * t:2 * t + 2, c0:c0 + 512],
                           start=(t == 0), stop=(t == 1), perf_mode=DR)
                    with tc.high_priority():
                        if half == 0:
                            act(h18[0:64, mt, :], ps, AF.Relu,
                                bias=b1_sb[0:64, mt:mt + 1], scale=1.0)
                        else:
                            dve.tensor_scalar(
                                out=h18[64:128, mt, :], in0=ps,
                                scalar1=b1_sb[64:128, mt:mt + 1], scalar2=0.0,
                                op0=OP.add, op1=OP.max)
                    yield

        # ---- chain B for one 256-token block: FFN2(DR) + res, LN2, out ----
        z2s = {}

        def gen_chain_b_ffn(blk):
            c0 = 256 * blk
            h18 = h18s[blk // 2]
            r0 = 256 * (blk % 2)
            z2 = [z_pool.tile([128, 256], F32, tag=f"z{ot}", name=f"z{ot}")
                  for ot in range(4)]
            z2s[blk] = z2
            for ot in range(4):
                for half in range(2):
                    ps = ppool.tile([64, 512], F32, tag="ps", name="psf2")
                    mcol = 64 * (2 * ot + half)
                    for t in range(8):
                        mm(ps[:, :256], w28[:, t, :, mcol:mcol + 64],
                           h18[:, 2 * t:2 * t + 2, r0:r0 + 256],
                           start=(t == 0), stop=(t == 7), perf_mode=DR)
                    with tc.high_priority():
                        dve.scalar_tensor_tensor(
                            out=z2[ot][64 * half:64 * half + 64, :],
                            in0=ps[:, :256],
                            scalar=1.0 / (SW * SW),
                            in1=src1_f[ot][blk][64 * half:64 * half + 64, :],
                            op0=OP.mult, op1=OP.add)
                    yield

        def gen_chain_b_ln(blk, stats_pool=None):
            c0 = 256 * blk
            z2 = z2s[blk]

            def w2(ct, t2):
                o = o_pool.tile([128, 256], F32, tag="out", name="out", bufs=6)
                gps.tensor_scalar_add(o, t2, bt2_sb[:, ct:ct + 1])
                dma(out_d[128 * ct:128 * (ct + 1), c0:c0 + 256], o)

            yield from gen_layernorm(z2, g2_sb, w2, 256, stats_pool=stats_pool)

        def gen_chain_b(blk, stats_pool=None):
            yield from gen_chain_b_ffn(blk)
            yield from gen_chain_b_ln(blk, stats_pool=stats_pool)

        def gen_seq(*gens):
            for g in gens:
                yield from g

        # ---- schedule ----
        # V' ones cols hold 1/SX so den rows accumulate den/SX and
        # rep4 = recip(den/SX) = SX/den.  (memset inside gen_vproj runs first.)
        xnp0 = xn_pool.tile([128, 2 * S], FP8, tag="xn", name="xn")
        xnp1 = xn_pool.tile([128, 2 * S], FP8, tag="xn", name="xn")

        _interleave(gen_proj(wk8, bk_sb, srcu8, sup, k_sb, 0),
                    gen_proj(wq8, bq_sb, src8, S, q_sb, 0),
                    gen_vproj())
        load_chain_weights()
        _interleave(gen_attention(0, xnp0),
                    gen_seq(gen_proj(wk8, bk_sb, srcu8, sup, k_sb, 1),
                            gen_proj(wq8, bq_sb, src8, S, q_sb, 1)))
        _interleave(gen_attention(1, xnp0), gen_chain_a(0, xnp0))
        _interleave(gen_attention(2, xnp1),
                    gen_seq(gen_chain_a(1, xnp0), gen_ffn1(0)))
        _interleave(gen_attention(3, xnp1),
                    gen_seq(gen_chain_b(0), gen_chain_a(2, xnp1),
                            gen_chain_b(1)))
        _interleave(gen_seq(gen_chain_a(3, xnp1), gen_ffn1(1),
                            gen_chain_b_ffn(2)))
        _interleave(gen_seq(gen_chain_b_ln(2, stats_pool=pe_av),
                            gen_chain_b_ln(3)),
                    gen_chain_b_ffn(3))

    nc.compile()
    return nc


def _prep_core(c, src, idxs, sup, w):
    b, hg = c // 2, c % 2
    heads = list(range(HPC * hg, HPC * hg + HPC))
    st = np.ascontiguousarray(src[b].T)                       # [512, 2048] f32
    idx = idxs[b]
    su = len(idx)
    srcu = np.zeros((HID, sup), np.float32)
    srcu[:, :su] = st[:, idx]
    wqe = np.concatenate([w["Wm"] @ w["Wq"][64 * h:64 * (h + 1), :] for h in heads])
    bqe = np.concatenate([w["Wm"] @ w["bq"][64 * h:64 * (h + 1)] + w["bm"]
                          for h in heads])
    wks = np.concatenate([w["Wk"][64 * h:64 * (h + 1), :] for h in heads])
    bks = np.concatenate([w["bk"][64 * h:64 * (h + 1)] for h in heads])
    wvs = np.concatenate([w["Wv"][64 * h:64 * (h + 1), :] for h in heads])
    bvs = np.concatenate([w["bv"][64 * h:64 * (h + 1)] for h in heads])
    mb = np.full(sup, NEG_BIG, np.float32)
    mb[:su] = 0.0
    f32 = np.float32
    src_res = np.ascontiguousarray(st[:, R * hg:R * (hg + 1)]) \
        + w["bo"][:, None].astype(f32)
    return {
        "src8": st.astype(fp8np),
        "src_res": src_res.astype(f32),
        "srcu8": srcu.astype(fp8np),
        "wq": _pack_dr(np.ascontiguousarray(wqe.T) * SQ).astype(fp8np),
        "wk": _pack_dr(np.ascontiguousarray(wks.T) * SK).astype(fp8np),
        "wv": (np.ascontiguousarray(wvs.T) * SV).astype(fp8np),
        "wo": _pack_dr(np.ascontiguousarray(w["Wo"].T) * SW).astype(fp8np),
        "w1": _pack_dr(np.ascontiguousarray(w["W1"].T) * SW).astype(fp8np),
        "w2": _pack_dr(np.ascontiguousarray(w["W2"].T) * SW).astype(fp8np),
        "bq": (bqe * SQ).reshape(2, 128).astype(f32),
        "bk": (bks * SK).reshape(2, 128).astype(f32),
        "bv": bvs.astype(f32),
        "b1": (w["b1"] * SW).reshape(16, 128).astype(f32),
        "bt1": w["ln1_b"].reshape(4, 128).astype(f32),
        "bt1f": (w["ln1_b"] + w["b2"]).reshape(4, 128).astype(f32),
        "g1": w["ln1_g"].reshape(4, 128).astype(f32),
        "g2": w["ln2_g"].reshape(4, 128).astype(f32),
        "bt2": w["ln2_b"].reshape(4, 128).astype(f32),
        "mb": mb.reshape(sup // 128, 128),
    }


def kernel(**inputs):
    global last_results
    w = {k: np.asarray(v, np.float32) for k, v in inputs.items()
         if k not in ("src", "src_mask")}
    src = np.asarray(inputs["src"], np.float32)
    mask = np.asarray(inputs["src_mask"]).reshape(B, S)
    idxs = [np.nonzero(mask[b] != 0)[0] for b in range(B)]
    sup = max(128, ((max(len(i) for i in idxs) + 127) // 128) * 128)

    if sup not in _built_cache:
        _built_cache[sup] = build_bass(sup)
    nc = _built_cache[sup]

    in_maps = [_prep_core(c, src, idxs, sup, w) for c in range(N_CORES)]
    res = bass_utils.run_bass_kernel_spmd(nc, in_maps, core_ids=list(range(N_CORES)),
                                          **run_kwargs)
    last_results = res
    out = np.empty((B, S, HID), np.float32)
    for c in range(N_CORES):
        b, hg = c // 2, c % 2
        out[b, R * hg:R * (hg + 1), :] = res.results[c]["out_t"].T
    return out


# revision 89
# speedup vs baseline: 1.1180x; 1.0015x over previous
"""Trainium2 Bass kernel for nn_EncoderLayer (multiplicative-attention encoder layer).

Sharding: 8 cores; core c handles batch b=c//2, head-group hg=c%2 (4 of 8 heads).
The reference's head-major reshape bug maps head h exactly to output rows
[256h, 256h+256), so each core owns 1024 complete output rows -> no collectives.

v3: - big GEMMs (QKV proj, Wo, FFN1, FFN2) in fp8e4 DoubleRow perf mode
      (0.5 PE cycles/out-col, K=256/pass) with host-prepacked weights and
      power-of-2 pre-scales folded into writer ops / the exp scale.
    - software-pipelined emission: chain work for head h's 256 output tokens
      is interleaved (generator round-robin) with head h+1's attention, so
      Act (exp-bound) and PE (GEMM-bound) run concurrently.
    - softmax tail: recip straight off PSUM den rows, Pool partition_broadcast
      replaces the PE broadcast matmul, xn multiply reads PSUM directly.

Per-token chain independence: LN1/FFN/LN2 normalize over features, so the
chain runs on 256-token blocks (one attention head's scrambled rows each).
FFN1 runs per 512-token pair to halve writer-instruction overhead.
"""

import numpy as np
import ml_dtypes

import concourse.bass as bass
import concourse.tile as tile
import concourse.bacc as bacc
from concourse import mybir
from concourse import bass_utils
from concourse import hw_specs as _hw_specs

_real_gat = _hw_specs.get_activation_tables


def _gat_pinned(arch):
    tabs = _real_gat(arch)
    return {name: (fns if name == "natural_log_exp_and_others" else set())
            for name, fns in tabs.items()}


bacc.get_activation_tables = _gat_pinned

B, S, HID, H, PF, D = 4, 2048, 512, 8, 2048, 64
N_CORES = 8
HPC = H // 2          # heads per core (4)
R = HPC * 256         # output rows per core (1024)
F32 = mybir.dt.float32
BF16 = mybir.dt.bfloat16
FP8 = mybir.dt.float8e4
AF = mybir.ActivationFunctionType
OP = mybir.AluOpType
DR = mybir.MatmulPerfMode.DoubleRow
NEG_BIG = -87.0
LN_EPS = 1e-5
SQ = 64.0             # wq scale
SK = 32.0             # wk scale
SV = 32.0             # wv scale
SX = 64.0             # xn scale
SW = 32.0             # wo/w1/w2 scale
fp8np = ml_dtypes.float8_e4m3

_built_cache = {}
last_results = None
run_kwargs = {}


def _bcast_ap(ap_1d, parts):
    return bass.AP(tensor=ap_1d.tensor, offset=ap_1d.offset,
                   ap=[[0, parts], *ap_1d.ap])


def _pack_dr(wT):
    """[K, M] (K mult of 256) -> DR-packed [K//256 * 128, 2 * M] host layout."""
    K, M = wT.shape
    return np.ascontiguousarray(
        wT.reshape(K // 256, 2, 128, M).transpose(0, 2, 1, 3)
    ).reshape(K // 2, 2 * M)


import os as _os
_RATIO = int(_os.environ.get("KRATIO", "1"))


def _interleave(*gens, ratio=None):
    # first generator gets `ratio` bursts per single burst of the others
    r = ratio if ratio is not None else _RATIO
    active = [iter(g) for g in gens]
    while active:
        for i, g in enumerate(list(active)):
            n = r if (i == 0 and len(active) > 1) else 1
            for _ in range(n):
                try:
                    next(g)
                except StopIteration:
                    if g in active:
                        active.remove(g)
                    break


def build_bass(sup):
    """Per-core module. sup = padded unmasked key count (mult of 128)."""
    KT = sup // 128
    nc = bacc.Bacc("TRN2", target_bir_lowering=False, debug=False,
                   num_devices=N_CORES)

    def inp(name, shape, dt=F32):
        return nc.dram_tensor(name, shape, dt, kind="ExternalInput").ap()

    src8_d = inp("src8", [HID, S], FP8)
    src_res_d = inp("src_res", [HID, R])         # fp32 src.T slice + bo
    srcu8_d = inp("srcu8", [HID, sup], FP8)
    wq_d = inp("wq", [2 * 128, 2 * 256], FP8)    # DR-packed SQ*(Wm@Wq).T
    wk_d = inp("wk", [2 * 128, 2 * 256], FP8)
    wv_d = inp("wv", [HID, 256], FP8)
    wo_d = inp("wo", [2 * 128, 2 * 512], FP8)
    w1_d = inp("w1", [2 * 128, 2 * PF], FP8)
    w2_d = inp("w2", [8 * 128, 2 * 512], FP8)
    bq_d = inp("bq", [2, 128])
    bk_d = inp("bk", [2, 128])
    bv_d = inp("bv", [256])
    b1_d = inp("b1", [16, 128])                  # SW*b1
    bt1_d = inp("bt1", [4, 128])                 # ln1_b
    bt1f_d = inp("bt1f", [4, 128])               # ln1_b + b2
    g1_d = inp("g1", [4, 128])
    g2_d = inp("g2", [4, 128])
    bt2_d = inp("bt2", [4, 128])
    mb_d = inp("mb", [KT, 128])
    out_d = nc.dram_tensor("out_t", [HID, R], F32, kind="ExternalOutput").ap()

    from contextlib import ExitStack
    with tile.TileContext(nc) as tc, ExitStack() as ctx:
        con = ctx.enter_context(tc.tile_pool(name="con", bufs=1))
        ppool = ctx.enter_context(tc.tile_pool(name="ps", bufs=2, space="PSUM"))
        pe_e = ctx.enter_context(tc.tile_pool(name="pe", bufs=2, space="PSUM"))
        pe_av = ctx.enter_context(tc.tile_pool(name="pav", bufs=2, space="PSUM"))
        att_pool = ctx.enter_context(tc.tile_pool(name="att", bufs=12))
        xn_pool = ctx.enter_context(tc.tile_pool(name="xn", bufs=2))
        rep_pool = ctx.enter_context(tc.tile_pool(name="rep", bufs=4))
        h1_pool = ctx.enter_context(tc.tile_pool(name="h1", bufs=2))
        tmp_pool = ctx.enter_context(tc.tile_pool(name="tmp", bufs=5))
        z_pool = ctx.enter_context(tc.tile_pool(name="z", bufs=2))
        o_pool = ctx.enter_context(tc.tile_pool(name="o", bufs=6))

        mm = nc.tensor.matmul
        act = nc.scalar.activation
        dve = nc.vector
        gps = nc.gpsimd

        def dma(out, in_):
            nc.sync.dma_start(out=out, in_=in_)

        def ctile(shape, dt, tag):
            return con.tile(shape, dt, tag=tag, name=tag)

        # ---- constants / weights ----
        srcu8 = ctile([128, 4, sup], FP8, "srcu8")
        wq8 = ctile([128, 2, 2, 256], FP8, "wq8")
        wk8 = ctile([128, 2, 2, 256], FP8, "wk8")
        wv8 = ctile([128, 4, 256], FP8, "wv8")
        dma(wk8, wk_d.rearrange("(t p) (i m) -> p t i m", t=2, i=2))
        dma(srcu8, srcu8_d.rearrange("(c p) n -> p c n", p=128))
        dma(wq8, wq_d.rearrange("(t p) (i m) -> p t i m", t=2, i=2))
        src8 = ctile([128, 4, S], FP8, "src8")
        dma(src8, src8_d.rearrange("(c p) n -> p c n", p=128))
        dma(wv8, wv_d.rearrange("(c p) m -> p c m", p=128))
        src_res = [ctile([128, R], F32, f"srcres{i}") for i in range(4)]
        wo8 = ctile([128, 2, 2, 512], FP8, "wo8")
        w18 = ctile([128, 2, 2, PF], FP8, "w18")
        w28 = ctile([128, 8, 2, 512], FP8, "w28")

        def load_chain_weights():
            dma(wo8, wo_d.rearrange("(t p) (i m) -> p t i m", t=2, i=2))
            for i in range(4):
                dma(src_res[i], src_res_d[128 * i:128 * (i + 1), :])
            dma(w18, w1_d.rearrange("(t p) (i m) -> p t i m", t=2, i=2))
            dma(w28, w2_d.rearrange("(t p) (i m) -> p t i m", t=8, i=2))

        def vec_in(dram, n, tag):
            t = ctile([128, n], F32, tag)
            dma(t, dram.rearrange("m p -> p m"))
            return t

        bq_sb = vec_in(bq_d, 2, "bq")
        bk_sb = vec_in(bk_d, 2, "bk")
        b1_sb = vec_in(b1_d, 16, "b1")
        g1_sb = vec_in(g1_d, 4, "g1")
        bt1_sb = vec_in(bt1_d, 4, "bt1")
        bt1f_sb = vec_in(bt1f_d, 4, "bt1f")
        g2_sb = vec_in(g2_d, 4, "g2")
        bt2_sb = vec_in(bt2_d, 4, "bt2")
        mb_sb = vec_in(mb_d, KT, "mb")
        bv_rep = ctile([128, 256], F32, "bvrep")
        dma(bv_rep, _bcast_ap(bv_d, 128))

        ones_bf = ctile([128, 128], BF16, "onesbf")
        dve.memset(ones_bf, 1.0)
        eps_t = ctile([128, 1], F32, "eps")
        dve.memset(eps_t, LN_EPS)

        q_sb = [ctile([128, S], BF16, f"q{m}") for m in range(2)]
        k_sb = [ctile([128, sup], BF16, f"k{m}") for m in range(2)]

        def gen_proj(w8, bias_sb, src_t, n_total, out_tiles, mt):
            n0 = 0
            while n0 < n_total:
                nq = min(512, n_total - n0)
                for half in range(2):
                    ps = ppool.tile([64, 512], F32, tag="ps", name="psp")
                    mcol = 64 * (2 * mt + half)
                    for t in range(2):
                        mm(ps[:, :nq],
                           w8[:, t, :, mcol:mcol + 64],
                           src_t[:, 2 * t:2 * t + 2, n0:n0 + nq],
                           start=(t == 0), stop=(t == 1), perf_mode=DR)
                    dve.tensor_scalar_add(
                        out_tiles[mt][64 * half:64 * half + 64, n0:n0 + nq],
                        ps[:, :nq],
                        bias_sb[64 * half:64 * half + 64, mt:mt + 1])
                    yield
                n0 += nq

        # ---- V natural [keys, 4*128] bf16: 64 V cols + 64 ones cols ----
        # (AV matmul cost is N-proportional, M-free: 64 ones cols give 64
        # identical den rows so recip writes [64, W] directly -- no broadcast)
        v_sb = ctile([128, KT * 4 * 128], BF16, "v")
        v_v = v_sb.rearrange("p (kt h e) -> p kt h e", kt=KT, h=4)

        def gen_vproj():
            # ones cols hold 1/SX so den rows accumulate den/SX and
            # rep = recip(den/SX) = SX/den (xn lands mid-range for fp8)
            dve.memset(v_v[:, :, :, 64:128], 1.0 / SX)
            for kt in range(KT):
                ps = ppool.tile([128, 512], F32, tag="ps", name="psv")
                for ct in range(4):
                    mm(ps[:, :256], srcu8[:, ct, 128 * kt:128 * (kt + 1)],
                       wv8[:, ct, :], start=(ct == 0), stop=(ct == 3))
                dve.scalar_tensor_tensor(
                    out=v_v[:, kt, :, 0:64],
                    in0=ps[:, :256].rearrange("p (h d) -> p h d", h=4),
                    scalar=1.0 / SV,
                    in1=bv_rep.rearrange("p (h d) -> p h d", h=4),
                    op0=OP.mult, op1=OP.add)
                yield

        # ---- attention head h -> fp8 xnp half (scaled by SX) ----
        def gen_attention(h, xnp):
            g = h // 2
            p0 = 64 * (h % 2)
            o0 = S * (h % 2)
            for q0 in range(0, S, 1024):
                avs = []
                for half in range(2):
                    avs.append(pe_av.tile([128, 512], F32, tag="av", name="av"))
                for kt in range(KT):
                    e = pe_e.tile([128, 1024], F32, tag="e", name="e")
                    with tc.high_priority():
                        for half in range(2):
                            mm(e[:, 512 * half:512 * (half + 1)],
                               k_sb[g][p0:p0 + 64, 128 * kt:128 * (kt + 1)],
                               q_sb[g][p0:p0 + 64,
                                       q0 + 512 * half:q0 + 512 * (half + 1)],
                               start=True, stop=True)
                    at = att_pool.tile([128, 1024], BF16, tag="att", name="att")
                    act(at, e, AF.Exp, bias=mb_sb[:, kt:kt + 1],
                        scale=1.0 / (SQ * SK))
                    for half in range(2):
                        mm(avs[half], v_v[:, kt, h, :],
                           at[:, 512 * half:512 * (half + 1)],
                           start=(kt == 0), stop=(kt == KT - 1),
                           skip_group_check=True)
                    yield
                # tail: rep = SX/den via recip of the 4 identical den rows;
                # broadcast row 0 to 64 partitions on Pool; xn = x' * rep.
                for half in range(2):
                    rep = rep_pool.tile([64, 512], BF16, tag="rep", name="rep")
                    with tc.high_priority(), \
                         nc.allow_low_precision(reason="softmax recip"):
                        dve.reciprocal(rep, avs[half][64:128, :])
                    with tc.high_priority():
                        dve.tensor_tensor(
                            out=xnp[0:64, o0 + q0 + 512 * half:o0 + q0 + 512 * (half + 1)],
                            in0=avs[half][0:64, :], in1=rep, op=OP.mult)
                    yield
            with tc.high_priority():
                gps.tensor_copy(out=xnp[64:128, o0:o0 + S // 2],
                                in_=xnp[0:64, o0 + 1:o0 + S // 2 + 1])
                gps.tensor_copy(out=xnp[64:128, o0 + S // 2:o0 + S - 1],
                                in_=xnp[0:64, o0 + S // 2 + 1:o0 + S])
            yield

        # ---- layernorm on 4x[128, W] f32 z-tiles ----
        def gen_layernorm(z_tiles, g_sb, writers, W, stats_pool=None):
            # s1/s2 must sit in separate PSUM banks: a start=True matmul marks
            # its whole 2KB zero-region pending-zero, wiping any sibling
            # accumulation group sharing the bank.  stats_pool lets tail LNs
            # borrow the idle attention av-tag banks so concurrent chain
            # lanes don't cycle-deadlock on the shared chain psum tag.
            sp = stats_pool
            if sp is None:
                s1 = ppool.tile([128, W], F32, tag="ps", name="s1")
                s2 = ppool.tile([128, W], F32, tag="ps", name="s2")
            else:
                s1 = sp.tile([128, W], F32, tag="av", name="s1")
                s2 = sp.tile([128, W], F32, tag="av", name="s2")
            for ct in range(4):
                zb = tmp_pool.tile([128, W], BF16, tag="zb", name="zb")
                gps.tensor_copy(out=zb, in_=z_tiles[ct])
                sq = tmp_pool.tile([128, W], BF16, tag="sq", name="sq")
                dve.tensor_tensor(out=sq, in0=zb, in1=zb, op=OP.mult)
                mm(s1, ones_bf, zb, start=(ct == 0), stop=(ct == 3),
                   skip_group_check=True)
                mm(s2, ones_bf, sq, start=(ct == 0), stop=(ct == 3),
                   skip_group_check=True)
                yield
            bm = tmp_pool.tile([128, W], F32, tag="bm", name="bm")
            br = tmp_pool.tile([128, W], F32, tag="br", name="br")
            m2 = tmp_pool.tile([128, W], BF16, tag="m2", name="m2", bufs=1)
            with tc.high_priority():
                dve.tensor_scalar_mul(bm, s1, 1.0 / HID)
                dve.tensor_tensor(out=m2, in0=bm, in1=bm, op=OP.mult)
                dve.scalar_tensor_tensor(out=br, in0=s2,
                                         scalar=1.0 / HID, in1=m2,
                                         op0=OP.mult, op1=OP.subtract)
            with tc.high_priority():
                act(br, br, AF.Ln, bias=eps_t)
                act(br, br, AF.Exp, scale=-0.5)
            yield
            for ct in range(4):
                sub = tmp_pool.tile([128, W], F32, tag="sub", name="sub")
                gps.tensor_tensor(out=sub, in0=z_tiles[ct], in1=bm,
                                  op=OP.subtract)
                t2 = tmp_pool.tile([128, W], F32, tag="t2", name="t2")
                dve.scalar_tensor_tensor(out=t2, in0=sub,
                                         scalar=g_sb[:, ct:ct + 1], in1=br,
                                         op0=OP.mult, op1=OP.mult)
                writers(ct, t2)
                yield

        # ---- chain A for one 256-token block: Wo(DR) + res, LN1 ----
        src1_f = [[con.tile([128, 256], F32, tag=f"s1f{i}_{j}",
                            name=f"s1f{i}_{j}") for j in range(4)]
                  for i in range(4)]
        src1_8 = ctile([128, 4, R], FP8, "src1_8")

        def gen_chain_a(blk, xnp, stats_pool=None):
            c0 = 256 * blk
            hh = blk % 2
            xw = xnp.rearrange("p (hh m j) -> p j hh m", hh=2, j=8)
            z1 = [z_pool.tile([128, 256], F32, tag=f"z{mt}", name=f"z{mt}")
                  for mt in range(4)]
            for mt in range(4):
                for half in range(2):
                    ps = ppool.tile([64, 512], F32, tag="ps", name="pswo")
                    mcol = 64 * (2 * mt + half)
                    for t in range(2):
                        ifm = bass.AP(
                            tensor=xw.tensor,
                            offset=xw.offset + (4 * t) * xw.ap[1][0]
                            + hh * xw.ap[2][0],
                            ap=[xw.ap[0], [2 * xw.ap[1][0], 2], xw.ap[3]])
                        mm(ps[:, :256], wo8[:, t, :, mcol:mcol + 64], ifm,
                           start=(t == 0), stop=(t == 1), perf_mode=DR)
                    with tc.high_priority():
                        dve.scalar_tensor_tensor(
                            out=z1[mt][64 * half:64 * half + 64, :],
                            in0=ps[:, :256],
                            scalar=1.0 / (SX * SW),
                            in1=src_res[mt][64 * half:64 * half + 64, c0:c0 + 256],
                            op0=OP.mult, op1=OP.add)
                    yield

            def w1(ct, t2):
                gps.tensor_scalar_add(src1_f[ct][blk], t2,
                                      bt1f_sb[:, ct:ct + 1])
                gps.tensor_scalar_add(src1_8[:, ct, c0:c0 + 256], t2,
                                      bt1_sb[:, ct:ct + 1])

            yield from gen_layernorm(z1, g1_sb, w1, 256, stats_pool=stats_pool)

        # ---- FFN1 for a 512-token pair (blocks 2p, 2p+1) ----
        h18s = {}

        def gen_ffn1(p):
            c0 = 512 * p
            h18 = h1_pool.tile([128, 16, 512], FP8, tag="h1", name="h1", bufs=2)
            h18s[p] = h18
            for mt in range(16):
                for half in range(2):
                    ps = ppool.tile([64, 512], F32, tag="ps", name="psf1")
                    mcol = 64 * (2 * mt + half)
                    for t in range(2):
                        mm(ps, w18[:, t, :, mcol:mcol + 64],
                           src1_8[:, 2 * t:2 * t + 2, c0:c0 + 512],
                           start=(t == 0), stop=(t == 1), perf_mode=DR)
                    with tc.high_priority():
                        if half == 0:
                            act(h18[0:64, mt, :], ps, AF.Relu,
                                bias=b1_sb[0:64, mt:mt + 1], scale=1.0)
                        else:
                            dve.tensor_scalar(
                                out=h18[64:128, mt, :], in0=ps,
                                scalar1=b1_sb[64:128, mt:mt + 1], scalar2=0.0,
                                op0=OP.add, op1=OP.max)
                    yield

        # ---- chain B for one 256-token block: FFN2(DR) + res, LN2, out ----
        z2s = {}

        def gen_chain_b_ffn(blk):
            c0 = 256 * blk
            h18 = h18s[blk // 2]
            r0 = 256 * (blk % 2)
            z2 = [z_pool.tile([128, 256], F32, tag=f"z{ot}", name=f"z{ot}")
                  for ot in range(4)]
            z2s[blk] = z2
            for ot in range(4):
                for half in range(2):
                    ps = ppool.tile([64, 512], F32, tag="ps", name="psf2")
                    mcol = 64 * (2 * ot + half)
                    for t in range(8):
                        mm(ps[:, :256], w28[:, t, :, mcol:mcol + 64],
                           h18[:, 2 * t:2 * t + 2, r0:r0 + 256],
                           start=(t == 0), stop=(t == 7), perf_mode=DR)
                    with tc.high_priority():
                        dve.scalar_tensor_tensor(
                            out=z2[ot][64 * half:64 * half + 64, :],
                            in0=ps[:, :256],
                            scalar=1.0 / (SW * SW),
                            in1=src1_f[ot][blk][64 * half:64 * half + 64, :],
                            op0=OP.mult, op1=OP.add)
                    yield

        def gen_chain_b_ln(blk, stats_pool=None):
            c0 = 256 * blk
            z2 = z2s[blk]

            def w2(ct, t2):
                o = o_pool.tile([128, 256], F32, tag="out", name="out", bufs=6)
                gps.tensor_scalar_add(o, t2, bt2_sb[:, ct:ct + 1])
                dma(out_d[128 * ct:128 * (ct + 1), c0:c0 + 256], o)

            yield from gen_layernorm(z2, g2_sb, w2, 256, stats_pool=stats_pool)

        def gen_chain_b(blk, stats_pool=None):
            yield from gen_chain_b_ffn(blk)
            yield from gen_chain_b_ln(blk, stats_pool=stats_pool)

        def gen_seq(*gens):
            for g in gens:
                yield from g

        # ---- schedule ----
        # V' ones cols hold 1/SX so den rows accumulate den/SX and
        # rep4 = recip(den/SX) = SX/den.  (memset inside gen_vproj runs first.)
        xnp0 = xn_pool.tile([128, 2 * S], FP8, tag="xn", name="xn")
        xnp1 = xn_pool.tile([128, 2 * S], FP8, tag="xn", name="xn")

        _interleave(gen_proj(wk8, bk_sb, srcu8, sup, k_sb, 0),
                    gen_proj(wq8, bq_sb, src8, S, q_sb, 0),
                    gen_vproj())
        load_chain_weights()
        _interleave(gen_attention(0, xnp0),
                    gen_seq(gen_proj(wk8, bk_sb, srcu8, sup, k_sb, 1),
                            gen_proj(wq8, bq_sb, src8, S, q_sb, 1)))
        _interleave(gen_attention(1, xnp0), gen_chain_a(0, xnp0))
        _interleave(gen_attention(2, xnp1),
                    gen_seq(gen_chain_a(1, xnp0), gen_ffn1(0)))
        _interleave(gen_attention(3, xnp1),
                    gen_seq(gen_chain_b(0), gen_chain_a(2, xnp1),
                            gen_chain_b(1)))
        _interleave(gen_seq(gen_chain_a(3, xnp1), gen_ffn1(1),
                            gen_chain_b_ffn(2)))
        _interleave(gen_seq(gen_chain_b_ln(2, stats_pool=pe_av),
                            gen_chain_b_ln(3)),
                    gen_chain_b_ffn(3))

    nc.compile()
    return nc


def _prep_core(c, src, idxs, sup, w):
    b, hg = c // 2, c % 2
    heads = list(range(HPC * hg, HPC * hg + HPC))
    st = np.ascontiguousarray(src[b].T)                       # [512, 2048] f32
    idx = idxs[b]
    su = len(idx)
    srcu = np.zeros((HID, sup), np.float32)
    srcu[:, :su] = st[:, idx]
    wqe = np.concatenate([w["Wm"] @ w["Wq"][64 * h:64 * (h + 1), :] for h in heads])
    bqe = np.concatenate([w["Wm"] @ w["bq"][64 * h:64 * (h + 1)] + w["bm"]
                          for h in heads])
    wks = np.concatenate([w["Wk"][64 * h:64 * (h + 1), :] for h in heads])
    bks = np.concatenate([w["bk"][64 * h:64 * (h + 1)] for h in heads])
    wvs = np.concatenate([w["Wv"][64 * h:64 * (h + 1), :] for h in heads])
    bvs = np.concatenate([w["bv"][64 * h:64 * (h + 1)] for h in heads])
    mb = np.full(sup, NEG_BIG, np.float32)
    mb[:su] = 0.0
    f32 = np.float32
    src_res = np.ascontiguousarray(st[:, R * hg:R * (hg + 1)]) \
        + w["bo"][:, None].astype(f32)
    return {
        "src8": st.astype(fp8np),
        "src_res": src_res.astype(f32),
        "srcu8": srcu.astype(fp8np),
        "wq": _pack_dr(np.ascontiguousarray(wqe.T) * SQ).astype(fp8np),
        "wk": _pack_dr(np.ascontiguousarray(wks.T) * SK).astype(fp8np),
        "wv": (np.ascontiguousarray(wvs.T) * SV).astype(fp8np),
        "wo": _pack_dr(np.ascontiguousarray(w["Wo"].T) * SW).astype(fp8np),
        "w1": _pack_dr(np.ascontiguousarray(w["W1"].T) * SW).astype(fp8np),
        "w2": _pack_dr(np.ascontiguousarray(w["W2"].T) * SW).astype(fp8np),
        "bq": (bqe * SQ).reshape(2, 128).astype(f32),
        "bk": (bks * SK).reshape(2, 128).astype(f32),
        "bv": bvs.astype(f32),
        "b1": (w["b1"] * SW).reshape(16, 128).astype(f32),
        "bt1": w["ln1_b"].reshape(4, 128).astype(f32),
        "bt1f": (w["ln1_b"] + w["b2"]).reshape(4, 128).astype(f32),
        "g1": w["ln1_g"].reshape(4, 128).astype(f32),
        "g2": w["ln2_g"].reshape(4, 128).astype(f32),
        "bt2": w["ln2_b"].reshape(4, 128).astype(f32),
        "mb": mb.reshape(sup // 128, 128),
    }


def kernel(**inputs):
    global last_results
    w = {k: np.asarray(v, np.float32) for k, v in inputs.items()
         if k not in ("src", "src_mask")}
    src = np.asarray(inputs["src"], np.float32)
    mask = np.asarray(inputs["src_mask"]).reshape(B, S)
    idxs = [np.nonzero(mask[b] != 0)[0] for b in range(B)]
    sup = max(128, ((max(len(i) for i in idxs) + 127) // 128) * 128)

    if sup not in _built_cache:
        _built_cache[sup] = build_bass(sup)
    nc = _built_cache[sup]

    in_maps = [_prep_core(c, src, idxs, sup, w) for c in range(N_CORES)]
    res = bass_utils.run_bass_kernel_spmd(nc, in_maps, core_ids=list(range(N_CORES)),
                                          **run_kwargs)
    last_results = res
    out = np.empty((B, S, HID), np.float32)
    for c in range(N_CORES):
        b, hg = c // 2, c % 2
        out[b, R * hg:R * (hg + 1), :] = res.results[c]["out_t"].T
    return out
